# revision 1
# baseline (speedup 1.0000x reference)
"""Self-contained TRN2 Bass kernel for the DNC (NeuCom) recurrence.

kernel(**inputs) takes FULL inputs (B=16), shards batch across 8 NeuronCores
(2 per core), runs the Bass/Tile kernel SPMD, and gathers the full output.
"""
import math
from contextlib import ExitStack

import numpy as np

import concourse.bass as bass
import concourse.mybir as mybir
import concourse.tile as tile
from concourse.bass import ds, ts
from concourse.bass_utils import run_bass_kernel_spmd
from concourse.tile_scheduler import DMAInst

# ---------------------------------------------------------------------------
# Post-pass: the walrus build in this container accepts at most ONE sync-wait
# command per instruction; Tile attaches more. Split extras into NoOps.
# ---------------------------------------------------------------------------
_CTRL_TYPES = (mybir.InstDrain, mybir.InstEventSemaphore, mybir.InstNoOp)
_ctr = [0]


def _limit_for(inst):
    return 1


def fix_sync_waits(nc):
    for f in nc.m.functions:
        for bb in f.blocks:
            new_insts = []
            for inst in bb.instructions:
                si = inst.sync_info
                waits = list(si.on_wait) if si is not None else []
                lim = _limit_for(inst)
                if len(waits) > lim:
                    extra = waits[:-lim]
                    keep = waits[-lim:]
                    while extra:
                        chunk, extra = extra[:1], extra[1:]
                        _ctr[0] += 1
                        nop = mybir.InstNoOp(
                            name=f"WFIX-{_ctr[0]}",
                            engine=inst.engine,
                            sync_info=mybir.SyncInfo(on_wait=chunk, on_update=[]),
                            text_hint="waitfix",
                        )
                        new_insts.append(nop)
                    si.on_wait = keep
                new_insts.append(inst)
            bb.instructions = new_insts
    return nc


FP = mybir.dt.float32
AF = mybir.ActivationFunctionType
OP = mybir.AluOpType
AX = mybir.AxisListType

N, Wd, R, B = 256, 64, 4, 2
H, I, O, IF = 512, 512, 512, 471
EPS = 1e-6

C_RK, C_RB, C_WK, C_WB, C_EV, C_WV, C_FG, C_AG, C_WG, C_RM = (
    0, 256, 260, 324, 325, 389, 453, 457, 458, 459)


def build(nc: bass.Bass, T: int, debug: bool = False):
    x_d = nc.dram_tensor("x", [T, B, I], FP, kind="ExternalInput")
    wh_d = nc.dram_tensor("W_hid", [I + R * Wd, H], FP, kind="ExternalInput")
    bh_d = nc.dram_tensor("b_hid", [H], FP, kind="ExternalInput")
    wi_d = nc.dram_tensor("W_iface", [H, IF], FP, kind="ExternalInput")
    wo_d = nc.dram_tensor("W_out", [H, O], FP, kind="ExternalInput")
    wm_d = nc.dram_tensor("W_memout", [R * Wd, O], FP, kind="ExternalInput")
    out_d = nc.dram_tensor("out", [T, B, O], FP, kind="ExternalOutput")
    dbg = None
    if debug:
        dbg = {k: nc.dram_tensor(f"dbg_{k}", s, FP, kind="ExternalOutput")
               for k, s in [("h", [2, H]), ("cw", [2, 256]), ("ww", [2, 256]),
                            ("rc", [8, 256]), ("rv", [8, 64]), ("ifc", [2, IF]),
                            ("mt", [128, 256]), ("rn", [2, 256])]}
    with tile.TileContext(nc) as tc:
        with ExitStack() as ctx:
            _build(ctx, tc, nc, T, x_d, wh_d, bh_d, wi_d, wo_d, wm_d, out_d, dbg)
    return nc


def _build(ctx, tc, nc, T, x_d, wh_d, bh_d, wi_d, wo_d, wm_d, out_d, dbg=None):
    per = ctx.enter_context(tc.tile_pool(name="persist", bufs=1))
    car = ctx.enter_context(tc.tile_pool(name="carry", bufs=2))
    tmp = ctx.enter_context(tc.tile_pool(name="tmp", bufs=2))
    psA = ctx.enter_context(tc.tile_pool(name="psA", bufs=2, space="PSUM"))
    psB = ctx.enter_context(tc.tile_pool(name="psB", bufs=2, space="PSUM"))
    psC = ctx.enter_context(tc.tile_pool(name="psC", bufs=2, space="PSUM"))
    psD = ctx.enter_context(tc.tile_pool(name="psD", bufs=2, space="PSUM"))

    dma = nc.gpsimd.dma_start
    v = nc.vector
    sc = nc.scalar
    te = nc.tensor
    mm = te.matmul
    tp = te.transpose

    def T_(shape, tag):
        return tmp.tile(shape, FP, tag=tag, name=tag)

    def C_(shape, tag):
        return car.tile(shape, FP, tag=tag, name=tag)

    def P_(shape, tag):
        return per.tile(shape, FP, tag=tag, name=tag)

    # ---------------- constants ----------------
    ones_full = P_([128, 256], "ones_full")
    v.memset(ones_full[:], 1.0)
    ident = P_([128, 128], "ident")
    v.tensor_copy(ident[:], ones_full[:, 0:128])
    nc.gpsimd.affine_select(ident[:], ident[:], pattern=[[-1, 128]],
                            compare_op=OP.is_equal, fill=0.0, base=0,
                            channel_multiplier=1)
    iota_row = P_([128, 256], "iota_row")
    nc.gpsimd.iota(iota_row[:], pattern=[[1, 256]], base=0, channel_multiplier=0,
                   allow_small_or_imprecise_dtypes=True)
    jmask = []
    for c in range(2):
        jm = P_([128, 256], f"jmask{c}")
        nc.gpsimd.affine_select(jm[:], ones_full[:], pattern=[[-1, 256]],
                                compare_op=OP.is_ge, fill=0.0, base=128 * c - 1,
                                channel_multiplier=1)
        jmask.append(jm)
    onespad = P_([128, 2], "onespad")
    v.memset(onespad[:], 0.0)
    v.memset(onespad[0:64, 0:1], 1.0)
    v.memset(onespad[64:128, 1:2], 1.0)
    # selrowB[b]: [2, 256] with row b = ones
    sel0 = P_([2, 256], "sel0")
    v.memset(sel0[:], 0.0)
    v.memset(sel0[0:1, :], 1.0)
    sel1 = P_([2, 256], "sel1")
    v.tensor_sub(sel1[:], ones_full[0:2, :], sel0[:])
    selrowB = [sel0, sel1]
    selcolB = [sel0[:, 0:1], sel1[:, 0:1]]

    # ---------------- weights ----------------
    def load_w(dram, n_tiles, cols, name, row0=0, rows=128):
        out = []
        for k in range(n_tiles):
            t = P_([rows, cols], f"{name}{k}")
            dma(out=t[:], in_=dram.ap()[ds(row0 + k * rows, rows), :])
            out.append(t)
        return out

    wh_sb = load_w(wh_d, 4, H, "wh")
    wrv_sb = load_w(wh_d, 4, H, "wrv", row0=512, rows=64)
    wi_sb = load_w(wi_d, 4, IF, "wi")
    wo_sb = load_w(wo_d, 4, O, "wo")
    wm_sb = load_w(wm_d, 4, O, "wm", rows=64)
    bh_sb = P_([1, H], "bh")
    dma(out=bh_sb[:], in_=bh_d.ap()[None, :])

    # ---------------- Xp ----------------
    TB = T * B
    assert TB <= 128
    xnat = P_([128, I], "xnat")
    dma(out=xnat[:TB, :], in_=x_d.ap().rearrange("t b i -> (t b) i"))
    xt_sb = []
    for k in range(4):
        t = P_([128, TB], f"xt{k}")
        xtp = psC.tile([128, 256], FP, tag="bcast", name="xtp")
        tp(xtp[:, 0:TB], xnat[:TB, ts(k, 128)], ident[:TB, :TB])
        v.tensor_copy(t[:], xtp[:, 0:TB])
        xt_sb.append(t)
    xp_sb = P_([128, H], "xp")
    xp_ps = psA.tile([128, H], FP, tag="ctrl", name="xp_ps")
    for k in range(4):
        mm(xp_ps[:TB, :], xt_sb[k][:, :TB], wh_sb[k][:], start=(k == 0), stop=False)
    mm(xp_ps[:TB, :], ones_full[0:1, :TB], bh_sb[:], start=False, stop=True)
    v.tensor_copy(xp_sb[:TB, :], xp_ps[:TB, :])

    # ---------------- carries ----------------
    MT = C_([128, 256], "MT")
    v.memset(MT[:], 1e-6)
    Ms = []
    for c in range(2):
        m = C_([128, 128], f"Ms{c}")
        v.memset(m[:], 1e-6)
        Ms.append(m)
    L = {}
    for b in range(B):
        for c in range(2):
            l = C_([128, 256], f"L{b}{c}")
            v.memset(l[:], 0.0)
            L[(b, c)] = l
    u_col = C_([128, 4], "u_col")
    v.memset(u_col[:], 0.0)
    ww_col = C_([128, 4], "ww_col")
    v.memset(ww_col[:], 0.0)
    wwrowB = []
    pB = []
    for b in range(B):
        w = C_([1, 256], f"wwrow{b}")
        v.memset(w[:], 0.0)
        wwrowB.append(w)
        p = C_([1, 256], f"p{b}")
        v.memset(p[:], 0.0)
        pB.append(p)
    rwCol = []
    for c in range(2):
        t = C_([128, 8], f"rwCol{c}")
        v.memset(t[:], 0.0)
        rwCol.append(t)
    rvT = C_([64, 8], "rvT")
    v.memset(rvT[:], 0.0)
    rnorm_row = C_([2, 256], "rnorm_row")
    v.memset(rnorm_row[:], 1.0 / (math.sqrt(Wd * 1e-12) + EPS))

    # ---------------- steps ----------------
    for t_step in range(T):
        # ===== controller =====
        h_ps = psA.tile([2, H], FP, tag="ctrl", name="h_ps")
        for r in range(R):
            lhs = rvT[:].rearrange("w (b r) -> w b r", r=4)[:, :, r]
            mm(h_ps[:], lhs, wrv_sb[r][:], start=(r == 0), stop=False)
        mm(h_ps[:], ident[:, ds(2 * t_step, 2)], xp_sb[:], start=False, stop=True)
        h_sb = T_([2, H], "h_sb")
        sc.activation(h_sb[:], h_ps[:], AF.Relu)
        hT = T_([128, 8], "hT")
        for k in range(4):
            htp = psD.tile([128, 512], FP, tag="sm", name="htp")
            tp(htp[:, 0:2], h_sb[:, ts(k, 128)], ident[0:2, 0:2])
            v.tensor_copy(hT[:, ts(k, 2)], htp[:, 0:2])

        # ===== iface + packed activations =====
        if_ps = psA.tile([2, IF], FP, tag="ctrl", name="if_ps")
        for k in range(4):
            mm(if_ps[:], hT[:, ts(k, 2)], wi_sb[k][:], start=(k == 0), stop=(k == 3))
        ifc = T_([2, IF], "ifc")
        # oneplus(rb|wb) = 1 + softplus = 1 + relu(x) + ln(1 + exp(-|x|))
        bw5 = T_([2, 5], "bw5")
        v.tensor_copy(bw5[:, 0:4], if_ps[:, C_RB:C_RB + 4])
        v.tensor_copy(bw5[:, 4:5], if_ps[:, C_WB:C_WB + 1])
        bwa = T_([2, 5], "bwa")
        sc.activation(bwa[:], bw5[:], AF.Abs)
        sc.activation(bwa[:], bwa[:], AF.Exp, scale=-1.0)
        sc.activation(bwa[:], bwa[:], AF.Ln, bias=1.0)
        sc.activation(bw5[:], bw5[:], AF.Relu)
        v.tensor_add(bw5[:], bw5[:], bwa[:])
        v.tensor_scalar_add(bw5[:], bw5[:], 1.0)
        sc.activation(ifc[:, C_EV:C_WV], if_ps[:, C_EV:C_WV], AF.Sigmoid)
        sc.activation(ifc[:, C_WV:C_FG], if_ps[:, C_WV:C_FG], AF.Copy)
        sc.activation(ifc[:, C_FG:C_RM], if_ps[:, C_FG:C_RM], AF.Sigmoid)
        # rm softmax -> rmM [4, 6] cols (m*2+b)
        rme = T_([2, 12], "rme")
        sc.activation(rme[:], if_ps[:, C_RM:C_RM + 12], AF.Exp)
        rmden = T_([2, 4], "rmden")
        v.tensor_reduce(rmden[:], rme[:].rearrange("b (r m) -> b r m", m=3),
                        axis=AX.X, op=OP.add)
        v.reciprocal(rmden[:], rmden[:])
        rmG = T_([2, 12], "rmG")
        v.tensor_tensor(
            out=rmG[:].rearrange("b (m r) -> b m r", r=4),
            in0=rme[:].rearrange("b (r m) -> b m r", m=3),
            in1=rmden[:].rearrange("b (u r) -> b u r", u=1).broadcast_to([2, 3, 4]),
            op=OP.mult)
        rmM_ps = psD.tile([128, 512], FP, tag="sm", name="rmM_ps")
        for m3 in range(3):
            tp(rmM_ps[0:4, ds(m3 * 2, 2)], rmG[:, ds(m3 * 4, 4)], ident[0:2, 0:2])
        rmM = T_([4, 6], "rmM")
        v.tensor_copy(rmM[:], rmM_ps[0:4, 0:6])
        # ww blend coefficients: c1 = ag*wg, c2 = (1-ag)*wg
        c1 = T_([2, 1], "c1")
        v.tensor_mul(c1[:], ifc[:, C_AG:C_AG + 1], ifc[:, C_WG:C_WG + 1])
        c2 = T_([2, 1], "c2")
        v.tensor_scalar(c2[:], ifc[:, C_AG:C_AG + 1], -1.0, 1.0, op0=OP.mult,
                        op1=OP.add)
        v.tensor_mul(c2[:], c2[:], ifc[:, C_WG:C_WG + 1])
        c1t_ps = psD.tile([128, 512], FP, tag="sm", name="c1t_ps")
        tp(c1t_ps[0:1, 0:2], c1[:], ident[0:2, 0:2])
        c1T = T_([1, 2], "c1T")
        v.tensor_copy(c1T[:], c1t_ps[0:1, 0:2])
        c2m = []
        for b in range(B):
            cm = T_([2, 1], f"c2m{b}")
            v.tensor_mul(cm[:], c2[:], selcolB[b])
            c2m.append(cm)

        # per-batch ev|wv [1,128] and fg [1,4] via selector matmuls
        exg_ps = psD.tile([128, 512], FP, tag="sm", name="exg_ps")
        for b in range(B):
            mm(exg_ps[0:1, ds(b * 256, 128)], selcolB[b], ifc[:, C_EV:C_EV + 128],
               start=True, stop=True, skip_group_check=True)
            mm(exg_ps[0:1, ds(b * 256 + 128, 4)], selcolB[b],
               ifc[:, C_FG:C_FG + 4], start=True, stop=True,
               skip_group_check=True)
        evwvB = []
        fgrowB = []
        for b in range(B):
            ev = T_([1, 128], f"evwv{b}")
            v.tensor_copy(ev[:], exg_ps[0:1, ds(b * 256, 128)])
            evwvB.append(ev)
            fg = T_([1, 4], f"fgrow{b}")
            v.tensor_copy(fg[:], exg_ps[0:1, ds(b * 256 + 128, 4)])
            fgrowB.append(fg)

        # scaled keys
        ksq = T_([2, 320], "ksq")
        sc.activation(ksq[:, 0:256], if_ps[:, C_RK:C_RK + 256], AF.Square)
        sc.activation(ksq[:, 256:320], if_ps[:, C_WK:C_WK + 64], AF.Square)
        kn = T_([2, 5], "kn")
        v.tensor_reduce(kn[:], ksq[:].rearrange("b (k w) -> b k w", w=64),
                        axis=AX.X, op=OP.add)
        sc.activation(kn[:], kn[:], AF.Sqrt)
        v.tensor_scalar_add(kn[:], kn[:], EPS)
        v.reciprocal(kn[:], kn[:])
        scl = T_([2, 5], "scl")
        v.tensor_mul(scl[:, 0:4], kn[:, 0:4], bw5[:, 0:4])
        v.tensor_mul(scl[:, 4:5], kn[:, 4:5], bw5[:, 4:5])
        krow = T_([2, 320], "krow")
        v.tensor_tensor(
            out=krow[:, 0:256].rearrange("b (k w) -> b k w", w=64),
            in0=if_ps[:, C_RK:C_RK + 256].rearrange("b (k w) -> b k w", w=64),
            in1=scl[:, 0:4].rearrange("b (k u) -> b k u", u=1).broadcast_to(
                [2, 4, 64]),
            op=OP.mult)
        v.tensor_tensor(out=krow[:, 256:320], in0=if_ps[:, C_WK:C_WK + 64],
                        in1=scl[:, 4:5].broadcast_to([2, 64]), op=OP.mult)
        keysT = T_([128, 10], "keysT")
        v.memset(keysT[:], 0.0)
        kt_ps = psD.tile([128, 512], FP, tag="sm", name="kt_ps")
        for b in range(B):
            for k in range(5):
                mm(kt_ps[ds(b * 64, 64), ds(b * 5 + k, 1)], krow[:, ts(k, 64)],
                   selcolB[b], start=True, stop=True, skip_group_check=True)
        for b in range(B):
            v.tensor_copy(keysT[ds(b * 64, 64), ds(b * 5, 5)],
                          kt_ps[ds(b * 64, 64), ds(b * 5, 5)])

        # ===== cw on old M (packed [2, 256]) =====
        simw_ps = psD.tile([128, 512], FP, tag="sm", name="simw_ps")
        mm(simw_ps[0:2, 0:256],
           keysT[:].rearrange("p (b k) -> p b k", k=5)[:, :, 4], MT[:],
           start=True, stop=True)
        cwl = T_([2, 256], "cwl")
        v.tensor_mul(cwl[:], simw_ps[0:2, 0:256], rnorm_row[:])
        cwden = T_([2, 1], "cwden")
        cwe = T_([2, 256], "cwe")
        sc.activation(cwe[:], cwl[:], AF.Exp, accum_out=cwden[:])
        v.reciprocal(cwden[:], cwden[:])
        cw_row = T_([2, 256], "cw_row")
        v.tensor_scalar_mul(cw_row[:], cwe[:], cwden[:])

        # ===== usage =====
        ret_col = T_([128, 4], "ret_col")
        fgb_ps = psC.tile([128, 256], FP, tag="bcast", name="fgb_ps")
        for b in range(B):
            mm(fgb_ps[:, ds(b * 4, 4)], ones_full[0:1, 0:128], fgrowB[b][:],
               start=True, stop=True, skip_group_check=True)
        for c in range(2):
            m1 = T_([128, 8], "m1")
            v.tensor_mul(m1[:], rwCol[c][:], fgb_ps[:, 0:8])
            sc.activation(m1[:], m1[:], AF.Identity, bias=1.0, scale=-1.0)
            q = T_([128, 4], "qq")
            v.tensor_tensor(out=q[:].rearrange("p (b u) -> p b u", u=2),
                            in0=m1[:].rearrange("p (b r) -> p b r", r=4)[:, :, 0:2],
                            in1=m1[:].rearrange("p (b r) -> p b r", r=4)[:, :, 2:4],
                            op=OP.mult)
            v.tensor_tensor(
                out=ret_col[:].rearrange("p (b c) -> p b c", c=2)[:, :, c],
                in0=q[:].rearrange("p (b u) -> p b u", u=2)[:, :, 0],
                in1=q[:].rearrange("p (b u) -> p b u", u=2)[:, :, 1],
                op=OP.mult)
        un_col = C_([128, 4], "u_col")
        t1 = T_([128, 4], "t1")
        v.tensor_mul(t1[:], u_col[:], ww_col[:])
        t2 = T_([128, 4], "t2")
        v.tensor_add(t2[:], u_col[:], ww_col[:])
        v.tensor_sub(t2[:], t2[:], t1[:])
        v.tensor_mul(un_col[:], t2[:], ret_col[:])

        # ===== allocation (per batch) =====
        a_col = T_([128, 4], "a_col")
        aRowB = []
        for b in range(B):
            ur_ps = psD.tile([128, 512], FP, tag="sm", name="ur_ps")
            for c in range(2):
                tp(ur_ps[0:1, ts(c, 128)], un_col[:, ds(b * 2 + c, 1)], ident[:])
            u_rowb = T_([1, 256], f"u_row{b}")
            v.tensor_copy(u_rowb[:], ur_ps[0:1, 0:256])
            ubc_ps = psC.tile([128, 256], FP, tag="bcast", name="ubc_ps")
            mm(ubc_ps[:], ones_full[0:1, 0:128], u_rowb[:], start=True, stop=True)
            ubc = T_([128, 256], "ubc")
            v.tensor_copy(ubc[:], ubc_ps[:])
            pi = []
            for c in range(2):
                ucol_bc = un_col[:, ds(b * 2 + c, 1)]
                scr = T_([128, 256], "scr")
                rA = T_([128, 2], "rA")
                v.tensor_scalar(scr[:], ubc[:], ucol_bc, 0.0, op0=OP.is_lt,
                                op1=OP.add, accum_out=rA[:, 0:1])
                v.scalar_tensor_tensor(scr[:], ubc[:], ucol_bc, jmask[c][:],
                                       op0=OP.is_equal, op1=OP.mult,
                                       accum_out=rA[:, 1:2])
                r_col = T_([128, 1], "r_col")
                v.tensor_add(r_col[:], rA[:, 0:1], rA[:, 1:2])
                pic = T_([128, 256], f"pi{c}")
                v.tensor_scalar(pic[:], iota_row[:], r_col[:], None,
                                op0=OP.is_equal)
                pi.append(pic)
            su_ps = psD.tile([128, 512], FP, tag="sm", name="su_ps")
            for c in range(2):
                mm(su_ps[0:1, 0:256], un_col[:, ds(b * 2 + c, 1)], pi[c][:],
                   start=(c == 0), stop=(c == 1))
            asc = T_([1, 257], "asc")
            v.memset(asc[:, 0:1], 1.0)
            v.tensor_tensor_scan(asc[:, 1:257], su_ps[0:1, 0:256],
                                 ones_full[0:1, 0:256], initial=1.0,
                                 op0=OP.mult, op1=OP.bypass)
            asr = T_([1, 256], "asr")
            v.tensor_sub(asr[:], asc[:, 0:256], asc[:, 1:257])
            abc_ps = psC.tile([128, 256], FP, tag="bcast", name="abc_ps")
            mm(abc_ps[:], ones_full[0:1, 0:128], asr[:], start=True, stop=True)
            for c in range(2):
                scr2 = T_([128, 256], "scr")
                v.scalar_tensor_tensor(scr2[:], pi[c][:], 1.0, abc_ps[:],
                                       op0=OP.mult, op1=OP.mult,
                                       accum_out=a_col[:, ds(b * 2 + c, 1)])
            ar_ps = psD.tile([128, 512], FP, tag="sm", name="ar_ps")
            for c in range(2):
                tp(ar_ps[0:1, ts(c, 128)], a_col[:, ds(b * 2 + c, 1)], ident[:])
            arow = T_([1, 256], f"arow{b}")
            v.tensor_copy(arow[:], ar_ps[0:1, 0:256])
            aRowB.append(arow)

        # ===== ww rows (PE blend), cols, p =====
        wwrowBn = []
        negwwB = []
        wwsumB = []
        for b in range(B):
            ww_ps = psD.tile([128, 512], FP, tag="sm", name="ww_ps")
            mm(ww_ps[0:1, 0:256], c1T[:, ds(b, 1)], aRowB[b][:], start=True,
               stop=False, skip_group_check=True)
            mm(ww_ps[0:1, 0:256], c2m[b][:], cw_row[:], start=False, stop=True,
               skip_group_check=True)
            wwn = C_([1, 256], f"wwrow{b}")
            wwsum = T_([1, 1], f"wwsum{b}")
            sc.activation(wwn[:], ww_ps[0:1, 0:256], AF.Copy, accum_out=wwsum[:])
            wwsumB.append(wwsum)
            wwrowBn.append(wwn)
            nw = T_([1, 256], f"negww{b}")
            v.tensor_scalar_mul(nw[:], wwn[:], -1.0)
            negwwB.append(nw)
        wwn_col = C_([128, 4], "ww_col")
        wc_ps = psD.tile([128, 512], FP, tag="sm", name="wc_ps")
        for b in range(B):
            for c in range(2):
                mm(wc_ps[:, ds(b * 2 + c, 1)], wwrowBn[b][0:1, ts(c, 128)],
                   ones_full[0:1, 0:1], start=True, stop=True,
                   skip_group_check=True)
        v.tensor_copy(wwn_col[:], wc_ps[:, 0:4])
        pBn = []
        for b in range(B):
            nws = T_([1, 1], f"nws{b}")
            v.tensor_scalar(nws[:], wwsumB[b][:], -1.0, 1.0, op0=OP.mult,
                            op1=OP.add)
            pn = C_([1, 256], f"p{b}")
            v.scalar_tensor_tensor(pn[:], pB[b][:], nws[:], wwrowBn[b][:],
                                   op0=OP.mult, op1=OP.add)
            pBn.append(pn)

        # ===== M update =====
        q1t_ps = psB.tile([128, 256], FP, tag="aux", name="q1t_ps")
        q2t_ps = psB.tile([128, 256], FP, tag="aux", name="q2t_ps")
        for b in range(B):
            negev = T_([1, 64], f"negev{b}")
            v.tensor_scalar_mul(negev[:], evwvB[b][:, 0:64], -1.0)
            mm(q1t_ps[ds(b * 64, 64), :], negev[:], wwrowBn[b][:], start=True,
               stop=True, skip_group_check=True)
            mm(q2t_ps[ds(b * 64, 64), :], evwvB[b][:, 64:128], wwrowBn[b][:],
               start=True, stop=True, skip_group_check=True)
        MTn = C_([128, 256], "MT")
        v.scalar_tensor_tensor(MTn[:], q1t_ps[:], 1.0, MT[:], op0=OP.add,
                               op1=OP.mult)
        v.tensor_add(MTn[:], MTn[:], q2t_ps[:])
        Msn = []
        for c in range(2):
            q1s_ps = psB.tile([128, 256], FP, tag="aux", name="q1s_ps")
            q2s_ps = psB.tile([128, 256], FP, tag="aux", name="q2s_ps")
            for b in range(B):
                mm(q1s_ps[:, ds(b * 64, 64)], negwwB[b][0:1, ts(c, 128)],
                   evwvB[b][:, 0:64], start=True, stop=True,
                   skip_group_check=True)
                mm(q2s_ps[:, ds(b * 64, 64)], wwrowBn[b][0:1, ts(c, 128)],
                   evwvB[b][:, 64:128], start=True, stop=True,
                   skip_group_check=True)
            msn = C_([128, 128], f"Ms{c}")
            v.scalar_tensor_tensor(msn[:], q1s_ps[:, 0:128], 1.0, Ms[c][:],
                                   op0=OP.add, op1=OP.mult)
            v.tensor_add(msn[:], msn[:], q2s_ps[:, 0:128])
            Msn.append(msn)

        # ===== L update + transient LT =====
        Ln = {}
        for b in range(B):
            for c in range(2):
                a2_ps = psB.tile([128, 256], FP, tag="aux", name="a2_ps")
                mm(a2_ps[:], negwwB[b][0:1, ts(c, 128)], ones_full[0:1, :],
                   start=True, stop=False)
                mm(a2_ps[:], ones_full[0:1, 0:128], negwwB[b][:],
                   start=False, stop=True)
                b_ps = psB.tile([128, 256], FP, tag="aux", name="b_ps")
                mm(b_ps[:], wwrowBn[b][0:1, ts(c, 128)], pB[b][:],
                   start=True, stop=True)
                ln = C_([128, 256], f"L{b}{c}")
                v.scalar_tensor_tensor(ln[:], a2_ps[:], 1.0, L[(b, c)][:],
                                       op0=OP.add, op1=OP.mult)
                v.tensor_add(ln[:], ln[:], b_ps[:])
                nc.gpsimd.affine_select(ln[:], ln[:], pattern=[[-1, 256]],
                                        compare_op=OP.not_equal, fill=0.0,
                                        base=128 * c, channel_multiplier=1)
                Ln[(b, c)] = ln
        LT = {}
        for b in range(B):
            for jc in range(2):
                lt = T_([128, 256], f"LT{b}{jc}")
                for ic in range(2):
                    lt_ps = psC.tile([128, 256], FP, tag="bcast", name="lt_ps")
                    tp(lt_ps[:, 0:128], Ln[(b, ic)][:, ts(jc, 128)], ident[:])
                    sc.activation(lt[:, ts(ic, 128)], lt_ps[:, 0:128], AF.Copy)
                LT[(b, jc)] = lt

        # ===== rc on new M (per batch [4, 256]) =====
        mt2 = T_([128, 256], "mt2")
        sc.activation(mt2[:], MTn[:], AF.Square)
        nq_ps = psD.tile([128, 512], FP, tag="sm", name="nq_ps")
        mm(nq_ps[0:2, 0:256], onespad[:], mt2[:], start=True, stop=True)
        rnN = C_([2, 256], "rnorm_row")
        sc.activation(rnN[:], nq_ps[0:2, 0:256], AF.Sqrt)
        v.tensor_scalar_add(rnN[:], rnN[:], EPS)
        v.reciprocal(rnN[:], rnN[:])
        rcB = []
        for b in range(B):
            simr_ps = psD.tile([128, 512], FP, tag="sm", name="simr_ps")
            mm(simr_ps[0:4, 0:256],
               keysT[:].rearrange("p (b k) -> p b k", k=5)[:, b, 0:4], MTn[:],
               start=True, stop=True)
            rn4_ps = psC.tile([128, 256], FP, tag="bcast", name="rn4_ps")
            mm(rn4_ps[0:4, :], selrowB[b][:, 0:4], rnN[:], start=True, stop=True)
            rn4 = T_([4, 256], "rn4")
            v.tensor_copy(rn4[:], rn4_ps[0:4, :])
            rcl = T_([4, 256], "rcl")
            v.tensor_mul(rcl[:], simr_ps[0:4, 0:256], rn4[:])
            rcden = T_([4, 1], "rcden")
            rce = T_([4, 256], "rce")
            sc.activation(rce[:], rcl[:], AF.Exp, accum_out=rcden[:])
            v.reciprocal(rcden[:], rcden[:])
            rc = T_([4, 256], f"rc{b}")
            v.tensor_scalar_mul(rc[:], rce[:], rcden[:])
            rcB.append(rc)

        # ===== fwd / bwd / rw_new (per batch) =====
        rwnB = []
        for b in range(B):
            bwd_ps = psD.tile([128, 512], FP, tag="sm", name="bwd_ps")
            for c in range(2):
                mm(bwd_ps[0:4, 0:256],
                   rwCol[c][:].rearrange("p (b r) -> p b r", r=4)[:, b, :],
                   Ln[(b, c)][:], start=(c == 0), stop=(c == 1))
            fwd_ps = psD.tile([128, 512], FP, tag="sm", name="fwd_ps")
            for c in range(2):
                mm(fwd_ps[0:4, 0:256],
                   rwCol[c][:].rearrange("p (b r) -> p b r", r=4)[:, b, :],
                   LT[(b, c)][:], start=(c == 0), stop=(c == 1))
            rwn = T_([4, 256], f"rwn{b}")
            v.tensor_scalar_mul(rwn[:], bwd_ps[0:4, 0:256], rmM[:, ds(b, 1)])
            v.scalar_tensor_tensor(rwn[:], rcB[b][:], rmM[:, ds(2 + b, 1)],
                                   rwn[:], op0=OP.mult, op1=OP.add)
            v.scalar_tensor_tensor(rwn[:], fwd_ps[0:4, 0:256],
                                   rmM[:, ds(4 + b, 1)], rwn[:], op0=OP.mult,
                                   op1=OP.add)
            rwnB.append(rwn)
        rwColn = []
        for c in range(2):
            rwc = C_([128, 8], f"rwCol{c}")
            rwColn.append(rwc)
        for b in range(B):
            for c in range(2):
                rwc_ps = psD.tile([128, 512], FP, tag="sm", name="rwc_ps")
                tp(rwc_ps[:, 0:4], rwnB[b][:, ts(c, 128)], ident[0:4, 0:4])
                v.tensor_copy(rwColn[c][:].rearrange(
                    "p (b r) -> p b r", r=4)[:, b, :], rwc_ps[:, 0:4])

        # ===== rv =====
        rvTn = C_([64, 8], "rvT")
        for b in range(B):
            rv_ps = psD.tile([128, 512], FP, tag="sm", name="rv_ps")
            for c in range(2):
                mm(rv_ps[0:4, 0:64],
                   rwColn[c][:].rearrange("p (b r) -> p b r", r=4)[:, b, :],
                   Msn[c][:, ds(b * 64, 64)], start=(c == 0), stop=(c == 1))
            rvb = T_([4, 64], f"rvb{b}")
            v.tensor_copy(rvb[:], rv_ps[0:4, 0:64])
            rvt_ps = psD.tile([128, 512], FP, tag="sm", name="rvt_ps")
            tp(rvt_ps[0:64, 0:4], rvb[:], ident[0:4, 0:4])
            v.tensor_copy(rvTn[:].rearrange("w (b r) -> w b r", r=4)[:, b, :],
                          rvt_ps[0:64, 0:4])

        # ===== output =====
        po_ps = psA.tile([2, H], FP, tag="ctrl", name="po_ps")
        for k in range(4):
            mm(po_ps[:], hT[:, ts(k, 2)], wo_sb[k][:], start=(k == 0), stop=False)
        for r in range(R):
            lhs = rvTn[:].rearrange("w (b r) -> w b r", r=4)[:, :, r]
            mm(po_ps[:], lhs, wm_sb[r][:], start=False, stop=(r == 3))
        if dbg is not None and t_step == T - 1:
            dma(out=dbg["h"].ap(), in_=h_sb[:])
            dma(out=dbg["cw"].ap(), in_=cw_row[:])
            dma(out=dbg["ww"].ap()[0:1], in_=wwrowBn[0][:])
            dma(out=dbg["ww"].ap()[1:2], in_=wwrowBn[1][:])
            dma(out=dbg["rc"].ap()[0:4], in_=rcB[0][:])
            dma(out=dbg["rc"].ap()[4:8], in_=rcB[1][:])
            dma(out=dbg["rv"].ap()[0:4], in_=rvTn[:].rearrange("w (b r) -> w b r", r=4)[:, 0, :].rearrange("w r -> r w") if False else rvTn[:, 0:4].rearrange("w r -> r w") if False else rvTn[:, 0:4])
            dma(out=dbg["ifc"].ap(), in_=ifc[:])
            dma(out=dbg["mt"].ap(), in_=MTn[:])
            dma(out=dbg["rn"].ap(), in_=rnN[:])
        out_sb = T_([2, O], "out_sb")
        sc.activation(out_sb[:], po_ps[:], AF.Copy)
        dma(out=out_d.ap()[t_step], in_=out_sb[:])

        MT, Ms, L, u_col, ww_col, rwCol, rvT, rnorm_row = (
            MTn, Msn, Ln, un_col, wwn_col, rwColn, rvTn, rnN)
        wwrowB, pB = wwrowBn, pBn


# ---------------------------------------------------------------------------
# Public entry point
# ---------------------------------------------------------------------------
_T, _BFULL, _NCORES = 64, 16, 8
_cache = {}


def _get_nc():
    if "nc" not in _cache:
        nc = bass.Bass("TRN2")
        build(nc, _T)
        fix_sync_waits(nc)
        _cache["nc"] = nc
    return _cache["nc"]


def kernel(**inputs):
    x = np.ascontiguousarray(np.asarray(inputs["x"], dtype=np.float32))
    shared = {
        k: np.ascontiguousarray(np.asarray(inputs[k], dtype=np.float32))
        for k in ("W_hid", "b_hid", "W_iface", "W_out", "W_memout")
    }
    assert x.shape == (_T, _BFULL, I)
    nc = _get_nc()
    in_maps = []
    for core in range(_NCORES):
        shard = np.ascontiguousarray(x[:, core * B:(core + 1) * B, :])
        m = {"x": shard}
        m.update(shared)
        in_maps.append(m)
    res = run_bass_kernel_spmd(nc, in_maps, core_ids=list(range(_NCORES)))
    out = np.empty((_T, _BFULL, O), dtype=np.float32)
    for core in range(_NCORES):
        out[:, core * B:(core + 1) * B, :] = res.results[core]["out"]
    return out



# revision 14
# speedup vs baseline: 1.3472x; 1.3472x over previous
"""Optimized TRN2 Bass kernel for the DNC (NeuCom) recurrence — v2.

Key changes vs v1 baseline:
- Single activation table (natural_log_exp): sigmoid via exp + DVE reciprocal,
  inverse norms via exp(-0.5*ln(q+eps)), oneplus via ln(1+exp(x)).
- float32r matmuls for all large-free matmuls (4x fewer PE cycles/row).
- Block-diagonal fused matmuls: both batches in one instruction for sims,
  M update, L/LT updates, fwd/bwd.
- L^T maintained as a carry with elementwise updates (no per-step transposes).
- Allocation (usage sort) via masked log-sum instead of explicit permutation
  matmuls + scan: a_i = (1-u_i) * exp(sum_{j sorted before i} ln u_j).
  Exact ties (which persist among never-written slots) are handled by an
  equality tie-count term; compares run in ln-space so lt/eq stay consistent.
- Engine rebalance: copies on Activation, some elementwise on Pool.

Hardware constraint honored throughout: every SBUF operand of a non-DMA
instruction must start at partition 0/32/64/96 (PSUM operands are exempt),
so per-batch row data lives in separate base-0 tiles and [2,X] stacked tiles
are built via one-hot selector matmuls accumulated in PSUM.
"""
from contextlib import ExitStack

import numpy as np

import concourse.bass as bass
import concourse.mybir as mybir
import concourse.tile as tile
from concourse.bass import ds, ts
from concourse.bass_utils import run_bass_kernel_spmd

_ctr = [0]


def fix_sync_waits(nc):
    """walrus accepts at most ONE sync-wait per instruction; split extras."""
    for f in nc.m.functions:
        for bb in f.blocks:
            new_insts = []
            for inst in bb.instructions:
                si = inst.sync_info
                waits = list(si.on_wait) if si is not None else []
                if len(waits) > 1:
                    extra, keep = waits[:-1], waits[-1:]
                    while extra:
                        chunk, extra = extra[:1], extra[1:]
                        _ctr[0] += 1
                        nop = mybir.InstNoOp(
                            name=f"WFIX-{_ctr[0]}",
                            engine=inst.engine,
                            sync_info=mybir.SyncInfo(on_wait=chunk, on_update=[]),
                            text_hint="waitfix",
                        )
                        new_insts.append(nop)
                    si.on_wait = keep
                new_insts.append(inst)
            bb.instructions = new_insts
    return nc


FP = mybir.dt.float32
FPR = mybir.dt.float32r
AF = mybir.ActivationFunctionType
OP = mybir.AluOpType
AX = mybir.AxisListType

N, Wd, R, B = 256, 64, 4, 2
H, I, O, IF = 512, 512, 512, 471

C_RK, C_RB, C_WK, C_WB, C_EV, C_WV, C_FG, C_AG, C_WG, C_RM = (
    0, 256, 260, 324, 325, 389, 453, 457, 458, 459)

EQ_ON_POOL = True       # tie-count stt ops on Pool (else DVE)
LT_ADD_ON_POOL = True   # LT "+b2" adds on Pool (else DVE)


def r_(ap):
    return ap.bitcast(FPR)


def build(nc: bass.Bass, T: int, debug: bool = False):
    x_d = nc.dram_tensor("x", [T, B, I], FP, kind="ExternalInput")
    wh_d = nc.dram_tensor("W_hid", [I + R * Wd, H], FP, kind="ExternalInput")
    bh_d = nc.dram_tensor("b_hid", [H], FP, kind="ExternalInput")
    wi_d = nc.dram_tensor("W_iface", [H, IF], FP, kind="ExternalInput")
    wo_d = nc.dram_tensor("W_out", [H, O], FP, kind="ExternalInput")
    wm_d = nc.dram_tensor("W_memout", [R * Wd, O], FP, kind="ExternalInput")
    out_d = nc.dram_tensor("out", [T, B, O], FP, kind="ExternalOutput")
    dbg = None
    if debug:
        dbg = {k: nc.dram_tensor(f"dbg_{k}", s, FP, kind="ExternalOutput")
               for k, s in [("h", [2, H]), ("sig", [2, 134]),
                            ("cw", [2, 256]), ("ret", [128, 4]),
                            ("u", [128, 4]), ("a", [128, 4]),
                            ("ww", [1, 512]), ("mt", [64, 512]),
                            ("rn", [1, 512]), ("rc", [8, 256]),
                            ("rw", [8, 256]), ("rv", [8, 64]),
                            ("L0", [128, 512]), ("LT0", [128, 512]),
                            ("p", [1, 512]), ("lnu", [128, 4]),
                            ("eqc", [128, 4]), ("A1", [128, 4])]}
    with tile.TileContext(nc) as tc:
        with ExitStack() as ctx:
            _build(ctx, tc, nc, T, x_d, wh_d, bh_d, wi_d, wo_d, wm_d, out_d,
                   dbg)
    return nc


def _build(ctx, tc, nc, T, x_d, wh_d, bh_d, wi_d, wo_d, wm_d, out_d, dbg):
    per = ctx.enter_context(tc.tile_pool(name="persist", bufs=1))
    car = ctx.enter_context(tc.tile_pool(name="carry", bufs=2))
    tmp = ctx.enter_context(tc.tile_pool(name="tmp", bufs=2))
    ps = ctx.enter_context(tc.tile_pool(name="ps", bufs=2, space="PSUM"))

    dma = nc.sync.dma_start
    v = nc.vector
    sc = nc.scalar
    gp = nc.gpsimd
    te = nc.tensor
    mm = te.matmul

    def mmr(out, lhsT, rhs, **kw):
        mm(out, r_(lhsT), r_(rhs), **kw)

    def tp_(out, in_, idn, **kw):
        mm(out, in_, idn, is_transpose=True, **kw)

    def T_(shape, tag, dt=FP):
        return tmp.tile(shape, dt, tag=tag, name=tag)

    def C_(shape, tag, dt=FP):
        return car.tile(shape, dt, tag=tag, name=tag)

    def P_(shape, tag, dt=FP):
        return per.tile(shape, dt, tag=tag, name=tag)

    def PS(shape, tag, bufs=None):
        return ps.tile(shape, FP, tag=tag, name=tag, bufs=bufs)

    # ---------------- constants ----------------
    ones_full = P_([128, 512], "ones_full")
    v.memset(ones_full[:], 1.0)
    ident = P_([128, 128], "ident")
    v.tensor_copy(ident[:], ones_full[:, 0:128])
    gp.affine_select(ident[:], ident[:], pattern=[[-1, 128]],
                     compare_op=OP.is_equal, fill=0.0, base=0,
                     channel_multiplier=1)
    # blockmask[b, n] = 1 if n in batch-b block
    blockmask = P_([2, 512], "blockmask")
    v.tensor_copy(blockmask[:], ones_full[0:2, :])
    gp.affine_select(blockmask[:], blockmask[:], pattern=[[1, 512]],
                     compare_op=OP.is_ge, fill=0.0, base=0,
                     channel_multiplier=-256)
    gp.affine_select(blockmask[:], blockmask[:], pattern=[[-1, 512]],
                     compare_op=OP.is_ge, fill=0.0, base=255,
                     channel_multiplier=256)
    jmask = []
    for c in range(2):
        jm = P_([128, 256], f"jmask{c}")
        gp.affine_select(jm[:], ones_full[:, 0:256], pattern=[[-1, 256]],
                         compare_op=OP.is_ge, fill=0.0, base=128 * c - 1,
                         channel_multiplier=1)
        jmask.append(jm)
    negblockmask = P_([2, 512], "negblockmask", FPR)
    v.tensor_scalar_mul(negblockmask[:], blockmask[:], -1.0)
    negones_row = P_([1, 128], "negones_row", FPR)
    v.tensor_scalar_mul(negones_row[:], ones_full[0:1, 0:128], -1.0)
    onesR = P_([128, 512], "onesR", FPR)
    v.tensor_copy(onesR[:], ones_full[:])
    identR = P_([128, 128], "identR", FPR)
    v.tensor_copy(identR[:], ident[:])
    cE12 = P_([128, 1], "cE12")
    v.memset(cE12[:], 1e-12)
    cE37 = P_([128, 1], "cE37")
    v.memset(cE37[:], 1e-37)
    # one-hot selectors
    selrow = []  # [1,2] rows for scatter (lhsT)
    for b in range(B):
        sf = P_([1, 2], f"selrowF{b}")
        v.memset(sf[:], 0.0)
        v.memset(sf[0:1, b:b + 1], 1.0)
        s = P_([1, 2], f"selrow{b}", FPR)
        v.tensor_copy(s[:], sf[:])
        selrow.append(s)
    selcol0 = P_([2, 1], "selcol0")
    v.memset(selcol0[:], 0.0)
    v.memset(selcol0[0:1, 0:1], 1.0)
    selcol1 = P_([2, 1], "selcol1")
    v.tensor_sub(selcol1[:], ones_full[0:2, 0:1], selcol0[:])
    selcol = [selcol0, selcol1]

    # ---------------- weights ----------------
    def load_w(dram, n_tiles, cols, name, row0=0, rows=128):
        out = []
        for k in range(n_tiles):
            t = P_([rows, cols], f"{name}{k}", FPR)
            nc.gpsimd.dma_start(out=t[:],
                                in_=dram.ap()[ds(row0 + k * rows, rows), :])
            out.append(t)
        return out

    wh_sb = load_w(wh_d, 4, H, "wh")
    wrv2 = load_w(wh_d, 2, H, "wrv2", row0=512, rows=128)
    # W_iface padded to even free size (f32r matmul ISA constraint)
    wi_sb = []
    for k in range(4):
        t = P_([128, IF + 1], f"wi{k}", FPR)
        v.tensor_scalar_mul(t[:], ones_full[:, 0:IF + 1], 0.0)
        nc.gpsimd.dma_start(out=t[:, 0:IF],
                            in_=wi_d.ap()[ds(k * 128, 128), :])
        wi_sb.append(t)
    wo_sb = load_w(wo_d, 4, O, "wo")
    wm2 = load_w(wm_d, 2, O, "wm2", rows=128)
    bh_sb = P_([1, H], "bh")
    dma(out=bh_sb[:], in_=bh_d.ap()[None, :])

    # ---------------- Xp precompute ----------------
    TB = T * B
    assert TB <= 128
    xnat = P_([128, I], "xnat")
    dma(out=xnat[:TB, :], in_=x_d.ap().rearrange("t b i -> (t b) i"))
    xt_sb = []
    for k in range(4):
        t = P_([128, 128], f"xt{k}", FPR)
        xtp = PS([128, 512], "ctrl")
        tp_(xtp[:, 0:TB], xnat[:TB, ts(k, 128)], ident[:TB, :TB])
        v.tensor_copy(t[:, :TB], xtp[:, 0:TB])
        xt_sb.append(t)
    xp_sb = P_([128, H], "xp", FPR)
    xp_ps = PS([128, H], "ctrl")
    for k in range(4):
        mmr(xp_ps[:TB, :], xt_sb[k][:, :TB], wh_sb[k][:], start=(k == 0),
            stop=False)
    mm(xp_ps[:TB, :], ones_full[0:1, :TB], bh_sb[:], start=False, stop=True)
    v.tensor_copy(xp_sb[:TB, :], xp_ps[:TB, :])

    # ---------------- carries (initial) ----------------
    MT = C_([64, 512], "MT", FPR)
    v.tensor_scalar_mul(MT[:], ones_full[0:64, :], 1e-6)
    Ms = []
    for c in range(2):
        m = C_([128, 128], f"Ms{c}", FPR)
        v.tensor_scalar_mul(m[:], ones_full[:, 0:128], 1e-6)
        Ms.append(m)
    L = []
    LT = []
    for c in range(2):
        l = C_([128, 512], f"L{c}", FPR)
        v.tensor_scalar_mul(l[:], ones_full[:], 0.0)
        L.append(l)
        lt = C_([128, 512], f"LT{c}", FPR)
        v.tensor_scalar_mul(lt[:], ones_full[:], 0.0)
        LT.append(lt)
    u_col = C_([128, 4], "u_col")
    v.memset(u_col[:], 0.0)
    ww_col = C_([128, 4], "ww_col")
    v.memset(ww_col[:], 0.0)
    ww_2r = C_([2, 256], "ww_2r", FPR)
    v.tensor_scalar_mul(ww_2r[:], ones_full[0:2, 0:256], 0.0)
    ww_row2 = C_([1, 512], "ww_row2", FPR)
    v.tensor_scalar_mul(ww_row2[:], ones_full[0:1, :], 0.0)
    ww_blk = C_([2, 512], "ww_blk", FPR)
    v.tensor_scalar_mul(ww_blk[:], ones_full[0:2, :], 0.0)
    p_2r = C_([2, 256], "p_2r", FPR)
    v.tensor_scalar_mul(p_2r[:], ones_full[0:2, 0:256], 0.0)
    p_blk = C_([2, 512], "p_blk", FPR)
    v.tensor_scalar_mul(p_blk[:], ones_full[0:2, :], 0.0)
    p_row2 = C_([1, 512], "p_row2", FPR)
    v.tensor_scalar_mul(p_row2[:], ones_full[0:1, :], 0.0)
    rw16 = C_([128, 16], "rw16", FPR)
    v.tensor_scalar_mul(rw16[:], ones_full[:, 0:16], 0.0)
    rvT128 = C_([128, 4], "rvT128", FPR)
    v.tensor_scalar_mul(rvT128[:], ones_full[:, 0:4], 0.0)
    rn_row2 = C_([1, 512], "rn_row2")
    v.memset(rn_row2[:], float((Wd * 1e-12 + 1e-12) ** -0.5))

    # smT column map (scratch PSUM bank, tag "sm"):
    SM_HTP, SM_RMG, SM_C12, SM_RST, SM_KT, SM_FGB = 0, 8, 14, 18, 22, 32
    SM_LNU, SM_AT, SM_WC, SM_RWT, SM_RVT = 48, 176, 304, 308, 324

    # ---------------- steps ----------------
    for t_step in range(T):
        last = (t_step == T - 1)
        smT = PS([128, 512], "sm", bufs=1)

        # ===== controller h =====
        h_ps = PS([2, H], "ctrl")
        for j in range(2):
            lhs = rvT128[:].rearrange("p (b j) -> p j b", j=2)[:, j, :]
            mmr(h_ps[:], lhs, wrv2[j][:], start=(j == 0), stop=False)
        mmr(h_ps[:], identR[:TB, ds(2 * t_step, 2)], xp_sb[:TB, :],
            start=False, stop=True)
        h_sb = T_([2, H], "h_sb")
        sc.activation(h_sb[:], h_ps[:], AF.Relu)
        for k in range(4):
            tp_(smT[:, ds(SM_HTP + 2 * k, 2)], h_sb[:, ts(k, 128)],
                ident[0:2, 0:2], skip_group_check=True)
        hT = T_([128, 8], "hT", FPR)
        v.tensor_copy(hT[:], smT[:, ds(SM_HTP, 8)])

        # ===== iface =====
        if_ps = PS([2, IF + 1], "ctrl")
        for k in range(4):
            mmr(if_ps[:], hT[:, ds(2 * k, 2)], wi_sb[k][:], start=(k == 0),
                stop=(k == 3))

        # -- iface activations (full 2-row ops only) --
        esig = T_([2, 134], "esig")
        sc.activation(esig[:], if_ps[:, C_EV:C_RM], AF.Exp, scale=-1.0)
        v.tensor_scalar_add(esig[:], esig[:], 1.0)
        sig = T_([2, 134], "sig")
        v.reciprocal(sig[:], esig[:])
        # sig: [,0:64]=ev  [,128:132]=fg  [,132:133]=ag  [,133:134]=wg

        rme = T_([2, 12], "rme")
        sc.activation(rme[:], if_ps[:, C_RM:C_RM + 12], AF.Exp)
        rmden = T_([2, 4], "rmden")
        v.tensor_reduce(rmden[:], rme[:].rearrange("b (r m) -> b r m", m=3),
                        axis=AX.X, op=OP.add)
        v.reciprocal(rmden[:], rmden[:])
        rmG = T_([2, 12], "rmG")
        v.tensor_tensor(
            out=rmG[:].rearrange("b (m r) -> b m r", r=4),
            in0=rme[:].rearrange("b (r m) -> b m r", m=3),
            in1=rmden[:].rearrange("b (u r) -> b u r", u=1).broadcast_to(
                [2, 3, 4]),
            op=OP.mult)
        for m3 in range(3):
            tp_(smT[0:4, ds(SM_RMG + 2 * m3, 2)], rmG[:, ds(4 * m3, 4)],
                ident[0:2, 0:2], skip_group_check=True)
        rm_m = []
        for m3 in range(3):
            rmt = T_([4, 2], f"rm_m{m3}")
            if m3 == 1:
                sc.activation(rmt[:], smT[0:4, ds(SM_RMG + 2 * m3, 2)],
                              AF.Copy)
            else:
                v.tensor_copy(rmt[:], smT[0:4, ds(SM_RMG + 2 * m3, 2)])
            rm_m.append(rmt)

        # gates -> transposed rows [1,2]
        c1 = T_([2, 1], "c1")
        v.tensor_tensor(c1[:], sig[:, 132:133], sig[:, 133:134], op=OP.mult)
        c2 = T_([2, 1], "c2")
        v.tensor_scalar(c2[:], sig[:, 132:133], -1.0, 1.0, op0=OP.mult,
                        op1=OP.add)
        v.tensor_mul(c2[:], c2[:], sig[:, 133:134])
        tp_(smT[0:1, ds(SM_C12, 2)], c1[:, 0:1], ident[0:2, 0:2],
            skip_group_check=True)
        tp_(smT[0:1, ds(SM_C12 + 2, 2)], c2[:, 0:1], ident[0:2, 0:2],
            skip_group_check=True)
        c1T = T_([1, 2], "c1T")
        c2T = T_([1, 2], "c2T")

        # oneplus(rb|wb) = 1 + ln(1+exp(x)); key norms; rs = (1+sp)/||k||
        bw5 = T_([2, 5], "bw5")
        sc.activation(bw5[:, 0:4], if_ps[:, C_RB:C_RB + 4], AF.Copy)
        sc.activation(bw5[:, 4:5], if_ps[:, C_WB:C_WB + 1], AF.Copy)
        sc.activation(bw5[:], bw5[:], AF.Exp)
        sc.activation(bw5[:], bw5[:], AF.Ln, bias=1.0)
        ifk = T_([2, 325], "ifk")
        v.tensor_copy(ifk[:], if_ps[:, 0:C_EV])
        ksq = T_([2, 325], "ksq")
        v.tensor_tensor(ksq[:], ifk[:], ifk[:], op=OP.mult)
        kn2 = T_([2, 5], "kn2")
        v.tensor_reduce(kn2[:, 0:4],
                        ksq[:, 0:256].rearrange("b (k w) -> b k w", w=64),
                        axis=AX.X, op=OP.add)
        v.tensor_reduce(kn2[:, 4:5], ksq[:, C_WK:C_WK + 64], axis=AX.X,
                        op=OP.add)
        sc.activation(kn2[:], kn2[:], AF.Ln, bias=cE12[0:2, 0:1])
        invkn = T_([2, 5], "invkn")
        sc.activation(invkn[:], kn2[:], AF.Exp, scale=-0.5)
        rs = T_([2, 5], "rs")
        v.scalar_tensor_tensor(rs[:], bw5[:], 1.0, invkn[:], op0=OP.add,
                               op1=OP.mult)
        # transpose read scales [2,4]->[4,2] and write scale [2,1]->[1,2]
        tp_(smT[0:4, ds(SM_RST, 2)], rs[:, 0:4], ident[0:2, 0:2],
            skip_group_check=True)
        tp_(smT[0:1, ds(SM_RST + 2, 2)], rs[:, 4:5], ident[0:2, 0:2],
            skip_group_check=True)
        rsRT = T_([4, 2], "rsRT")
        sc.activation(rsRT[:], smT[0:4, ds(SM_RST, 2)], AF.Copy)
        rsWT = T_([1, 2], "rsWT")
        sc.activation(rsWT[:], smT[0:1, ds(SM_RST + 2, 2)], AF.Copy)
        sc.activation(c1T[:], smT[0:1, ds(SM_C12, 2)], AF.Copy)
        sc.activation(c2T[:], smT[0:1, ds(SM_C12 + 2, 2)], AF.Copy)

        # ===== keys (raw; scales applied to sims) =====
        tp_(smT[:, ds(SM_KT, 2)], ifk[:, 0:128], ident[0:2, 0:2],
            skip_group_check=True)
        tp_(smT[:, ds(SM_KT + 2, 2)], ifk[:, 128:256], ident[0:2, 0:2],
            skip_group_check=True)
        tp_(smT[0:64, ds(SM_KT + 4, 2)], ifk[:, C_WK:C_WK + 64],
            ident[0:2, 0:2], skip_group_check=True)
        keysR = T_([64, 8], "keysR", FPR)
        keysW = T_([64, 8], "keysW", FPR)
        v.tensor_scalar_mul(keysW[:], ones_full[0:64, 0:8], 0.0)
        key_engs = [v, sc, v, sc]
        for kk in range(4):
            src = smT[ds(64 * (kk % 2), 64), ds(SM_KT + 2 * (kk // 2), 2)]
            eng = key_engs[kk]
            if eng is sc:
                sc.activation(
                    keysR[:].rearrange("w (b r) -> w r b", r=4)[:, kk, :],
                    src, AF.Copy)
            else:
                eng.tensor_copy(
                    keysR[:].rearrange("w (b r) -> w r b", r=4)[:, kk, :],
                    src)
        for b in range(B):
            v.tensor_copy(keysW[:, ds(4 * b, 1)],
                          smT[0:64, ds(SM_KT + 4 + b, 1)])

        # ===== cw on old M =====
        simw = []
        for b in range(B):
            swb = PS([2, 512], "ctrl")
            mmr(swb[:], keysW[:, ds(4 * b, 2)], MT[:], start=True, stop=True)
            simw.append(swb)
        shx = PS([128, 512], "shx", bufs=1)  # cw/rc rows 0:4,64:68; rn8 r32
        c2cw = []
        for b in range(B):
            r0 = ds(64 * b, 1)
            cwdb = T_([1, 1], f"cwd{b}")
            v.scalar_tensor_tensor(shx[r0, 0:256],
                                   simw[b][0:1, ds(256 * b, 256)],
                                   rsWT[0:1, b:b + 1],
                                   rn_row2[0:1, ds(256 * b, 256)],
                                   op0=OP.mult, op1=OP.mult)
            sc.activation(shx[r0, 256:512], shx[r0, 0:256], AF.Exp,
                          accum_out=cwdb[:])
            v.reciprocal(cwdb[:], cwdb[:])
            c2cwb = T_([1, 256], f"c2cw{b}")
            v.tensor_scalar(c2cwb[:], shx[r0, 256:512], cwdb[:],
                            c2T[0:1, b:b + 1], op0=OP.mult, op1=OP.mult)
            c2cw.append(c2cwb)

        # ===== usage =====
        fgrow = []
        for b in range(B):
            fgp = PS([1, 4], "ctrl")
            mm(fgp[:], selcol[b][:], sig[:, 128:132], start=True, stop=True)
            fgs = T_([1, 4], f"fgrow{b}")
            v.tensor_copy(fgs[:], fgp[:])
            fgrow.append(fgs)
        for c in range(2):
            for b in range(B):
                mm(smT[:, ds(SM_FGB + 8 * c + 4 * b, 4)],
                   ones_full[0:1, 0:128], fgrow[b][:], start=True, stop=True,
                   skip_group_check=True)
        m1 = T_([128, 16], "m1")
        v.scalar_tensor_tensor(m1[:], smT[:, ds(SM_FGB, 16)], -1.0, rw16[:],
                               op0=OP.mult, op1=OP.mult)
        m2 = T_([128, 16], "m2")
        sc.activation(m2[:], m1[:], AF.Identity, bias=1.0)
        q8 = T_([128, 8], "q8")
        gp.tensor_tensor(q8[:],
                        m2[:].rearrange("p (g r) -> p g r", r=2)[:, :, 0],
                        m2[:].rearrange("p (g r) -> p g r", r=2)[:, :, 1],
                        op=OP.mult)
        ret4 = T_([128, 4], "ret4")
        v.tensor_tensor(ret4[:],
                        q8[:].rearrange("p (h u) -> p h u", u=2)[:, :, 0],
                        q8[:].rearrange("p (h u) -> p h u", u=2)[:, :, 1],
                        op=OP.mult)
        t1 = T_([128, 4], "t1")
        gp.tensor_tensor(t1[:], u_col[:], ww_col[:], op=OP.mult)
        t2 = T_([128, 4], "t2")
        gp.tensor_add(t2[:], u_col[:], ww_col[:])
        v.tensor_sub(t2[:], t2[:], t1[:])
        un_col = C_([128, 4], "u_col")
        v.tensor_tensor(un_col[:], t2[:], ret4[:], op=OP.mult)

        # ===== allocation =====
        lnu_col = T_([128, 4], "lnu_col")
        sc.activation(lnu_col[:], un_col[:], AF.Ln, bias=cE37[:, 0:1])
        ut_ps = PS([1, 512], "ctrl")
        for j in range(4):
            b, c = j // 2, j % 2
            tp_(ut_ps[0:1, ds(128 * j, 128)],
                un_col[:, ds(2 * c + b, 1)], ident[:],
                skip_group_check=True)
        u_row2 = T_([1, 512], "u_row2")
        sc.activation(u_row2[:], ut_ps[:], AF.Copy)
        # per-batch PSUM bank: broadcast u_b; ln(u) goes to SBUF
        lnubc_sb = T_([128, 512], "lnubc_sb")
        ubcln = []
        for b in range(B):
            ub = PS([128, 256], "ubcln", bufs=1)
            mm(ub[:], ones_full[0:1, 0:128],
               u_row2[0:1, ds(256 * b, 256)], start=True, stop=True)
            sc.activation(lnubc_sb[:, ds(256 * b, 256)], ub[:], AF.Ln,
                          bias=cE37[:, 0:1])
            ubcln.append(ub)
        A1 = T_([128, 4], "A1")
        eqc = T_([128, 4], "eqc")
        for c in range(2):
            for b in range(B):
                col = ds(2 * c + b, 1)
                scr = T_([128, 256], f"scr{c}{b}")
                v.scalar_tensor_tensor(scr[:], ubcln[b][:],
                                       un_col[:, col],
                                       lnubc_sb[:, ds(256 * b, 256)],
                                       op0=OP.is_lt, op1=OP.mult,
                                       accum_out=A1[:, col])
                scr2 = T_([128, 256], f"scr2{c}{b}")
                v.scalar_tensor_tensor(scr2[:], ubcln[b][:],
                                       un_col[:, col], jmask[c][:],
                                       op0=OP.is_equal, op1=OP.mult,
                                       accum_out=eqc[:, col])
        A = T_([128, 4], "A")
        v.tensor_tensor(A[:], eqc[:], lnu_col[:], op=OP.mult)
        v.tensor_add(A[:], A[:], A1[:])
        cpx = T_([128, 4], "cpx")
        sc.activation(cpx[:], A[:], AF.Exp)
        onemu = T_([128, 4], "onemu")
        v.tensor_scalar(onemu[:], un_col[:], -1.0, 1.0, op0=OP.mult,
                        op1=OP.add)
        a_col = T_([128, 4], "a_col")
        v.tensor_tensor(a_col[:], onemu[:], cpx[:], op=OP.mult)

        # ===== ww (row space, written into [1,512] row) =====
        at_ps = PS([1, 512], "ctrl")
        for j in range(4):
            b, c = j // 2, j % 2
            tp_(at_ps[0:1, ds(128 * j, 128)], a_col[:, ds(2 * c + b, 1)],
                ident[:], skip_group_check=True)
        wwn_row2 = C_([1, 512], "ww_row2", FPR)
        wws4 = T_([1, 4], "wws4")
        for b in range(B):
            for c in range(2):
                v.scalar_tensor_tensor(
                    wwn_row2[0:1, ds(256 * b + 128 * c, 128)],
                    at_ps[0:1, ds(128 * (2 * b + c), 128)],
                    c1T[0:1, b:b + 1],
                    c2cw[b][0:1, ds(128 * c, 128)],
                    op0=OP.mult, op1=OP.add,
                    accum_out=wws4[0:1, ds(2 * b + c, 1)])
        wws2 = T_([1, 2], "wws2")
        v.tensor_reduce(wws2[:], wws4[:].rearrange("o (b c) -> o b c", c=2),
                        axis=AX.X, op=OP.add)
        # stacked [2,256] / [2,512] forms via selector-scatter in PSUM
        ww2r_ps = PS([2, 256], "ctrl")
        for b in range(B):
            mmr(ww2r_ps[:], selrow[b][:], wwn_row2[0:1, ds(256 * b, 256)],
                start=(b == 0), stop=(b == 1))
        wwn_2r = C_([2, 256], "ww_2r", FPR)
        v.tensor_copy(wwn_2r[:], ww2r_ps[:])
        wwblk_ps = PS([2, 512], "ctrl")
        for b in range(B):
            mmr(wwblk_ps[:, ds(256 * b, 256)], selrow[b][:],
                wwn_row2[0:1, ds(256 * b, 256)], start=True, stop=True,
                skip_group_check=True)
        wwn_blk = C_([2, 512], "ww_blk", FPR)
        sc.activation(wwn_blk[:], wwblk_ps[:], AF.Copy)
        # ww_col via transposes of stacked halves (cols come out as (b))
        for c in range(2):
            tp_(smT[:, ds(SM_WC + 2 * c, 2)].bitcast(FPR),
                wwn_2r[:, ds(128 * c, 128)], identR[0:2, 0:2],
                skip_group_check=True)
        wwn_col = C_([128, 4], "ww_col")
        v.tensor_copy(wwn_col[:], smT[:, ds(SM_WC, 4)])

        # ===== L / LT updates (old p as rhs) =====
        a2 = []
        for c in range(2):
            a2c = PS([128, 512], "a2", bufs=1)
            mmr(a2c[:], wwn_2r[:, ds(128 * c, 128)], negblockmask[:],
                start=True, stop=False)
            mmr(a2c[:], negones_row[:], wwn_row2[:], start=False,
                stop=True)
            a2.append(a2c)
        Ln = []
        LTn = []
        for c in range(2):
            b_c = PS([128, 512], "aux")
            mmr(b_c[:], wwn_2r[:, ds(128 * c, 128)], p_blk[:], start=True,
                stop=True)
            b2_c = PS([128, 512], "aux")
            mmr(b2_c[:], p_2r[:, ds(128 * c, 128)], wwn_blk[:], start=True,
                stop=True)
            lnc = C_([128, 512], f"L{c}", FPR)
            v.scalar_tensor_tensor(lnc[:], a2[c][:], 1.0, L[c][:], op0=OP.add,
                                   op1=OP.mult)
            v.tensor_add(lnc[:], lnc[:], b_c[:])
            gp.affine_select(lnc[:], lnc[:], pattern=[[0, 2], [-1, 256]],
                             compare_op=OP.not_equal, fill=0.0, base=128 * c,
                             channel_multiplier=1)
            Ln.append(lnc)
            ltc = C_([128, 512], f"LT{c}", FPR)
            v.scalar_tensor_tensor(ltc[:], a2[c][:], 1.0, LT[c][:],
                                   op0=OP.add, op1=OP.mult)
            v.tensor_add(ltc[:], ltc[:], b2_c[:])
            gp.affine_select(ltc[:], ltc[:], pattern=[[0, 2], [-1, 256]],
                             compare_op=OP.not_equal, fill=0.0, base=128 * c,
                             channel_multiplier=1)
            LTn.append(ltc)

        # ===== p update (row space + stacked forms) =====
        pn_row2 = C_([1, 512], "p_row2", FPR)
        nws2 = T_([1, 2], "nws2")
        sc.activation(nws2[:], wws2[:], AF.Identity, bias=1.0, scale=-1.0)
        for b in range(B):
            v.scalar_tensor_tensor(pn_row2[0:1, ds(256 * b, 256)],
                                   p_row2[0:1, ds(256 * b, 256)],
                                   nws2[0:1, b:b + 1],
                                   wwn_row2[0:1, ds(256 * b, 256)],
                                   op0=OP.mult, op1=OP.add)
        p2r_ps = PS([2, 256], "ctrl")
        for b in range(B):
            mmr(p2r_ps[:], selrow[b][:], pn_row2[0:1, ds(256 * b, 256)],
                start=(b == 0), stop=(b == 1))
        pn_2r = C_([2, 256], "p_2r", FPR)
        v.tensor_copy(pn_2r[:], p2r_ps[:])
        pblk_ps = PS([2, 512], "ctrl")
        for b in range(B):
            mmr(pblk_ps[:, ds(256 * b, 256)], selrow[b][:],
                pn_row2[0:1, ds(256 * b, 256)], start=True, stop=True,
                skip_group_check=True)
        pn_blk = C_([2, 512], "p_blk", FPR)
        sc.activation(pn_blk[:], pblk_ps[:], AF.Copy)

        # ===== M update =====
        negev_2r = T_([2, 64], "negev_2r", FPR)
        v.tensor_scalar_mul(negev_2r[:], sig[:, 0:64], -1.0)
        wv_2r = T_([2, 64], "wv_2r", FPR)
        v.tensor_copy(wv_2r[:], if_ps[:, C_WV:C_WV + 64])
        q1 = PS([64, 512], "aux")
        mmr(q1[:], negev_2r[:], wwn_blk[:], start=True, stop=True)
        q2 = PS([64, 512], "aux")
        mmr(q2[:], wv_2r[:], wwn_blk[:], start=True, stop=True)
        MTn = C_([64, 512], "MT", FPR)
        v.scalar_tensor_tensor(MTn[:], q1[:], 1.0, MT[:], op0=OP.add,
                               op1=OP.mult)
        v.tensor_add(MTn[:], MTn[:], q2[:])
        # Ms via transposes of MTn
        mst = PS([128, 512], "aux")
        for c in range(2):
            for b in range(B):
                tp_(mst[:, ds(64 * (2 * c + b), 64)].bitcast(FPR),
                    MTn[0:64, ds(256 * b + 128 * c, 128)],
                    identR[0:64, 0:64], skip_group_check=True)
        Msn = []
        for c in range(2):
            msc = C_([128, 128], f"Ms{c}", FPR)
            eng = v if c == 0 else sc
            if eng is sc:
                sc.activation(msc[:], mst[:, ds(128 * c, 128)], AF.Copy)
            else:
                v.tensor_copy(msc[:], mst[:, ds(128 * c, 128)])
            Msn.append(msc)

        # ===== rnorm (new M) =====
        mt2 = T_([64, 512], "mt2", FPR)
        sc.activation(mt2[:], MTn[:], AF.Square)
        nq = PS([2, 512], "aux")
        mmr(nq[:], onesR[0:64, 0:2], mt2[:], start=True, stop=True)
        rnln = T_([1, 512], "rnln")
        sc.activation(rnln[:], nq[0:1, :], AF.Ln, bias=cE12[0:1, 0:1])
        rnn_row2 = C_([1, 512], "rn_row2")
        sc.activation(rnn_row2[:], rnln[:], AF.Exp, scale=-0.5)

        # ===== rc on new M =====
        simr = []
        for b in range(B):
            srb = PS([4, 512], "ctrl")
            mmr(srb[:], keysR[:, ds(4 * b, 4)], MTn[:], start=True, stop=True)
            simr.append(srb)
        for b in range(B):
            mm(shx[ds(32, 4), ds(256 * b, 256)], ones_full[0:1, 0:4],
               rnn_row2[0:1, ds(256 * b, 256)], start=True, stop=True,
               skip_group_check=True)
        rn8_sb = T_([4, 512], "rn8_sb")
        sc.activation(rn8_sb[:], shx[ds(32, 4), :], AF.Copy)
        for b in range(B):
            rr = ds(64 * b, 4)  # rc rows reuse cw rows (consumed)
            v.scalar_tensor_tensor(shx[rr, 0:256],
                                   simr[b][:, ds(256 * b, 256)],
                                   rsRT[:, b:b + 1],
                                   rn8_sb[:, ds(256 * b, 256)],
                                   op0=OP.mult, op1=OP.mult)
            sc.activation(shx[rr, 256:512], shx[rr, 0:256], AF.Exp,
                          accum_out=smT[ds(64 * b, 4), ds(SM_LNU, 1)])
            v.reciprocal(smT[ds(64 * b, 4), ds(SM_LNU, 1)],
                         smT[ds(64 * b, 4), ds(SM_LNU, 1)])

        # ===== fwd / bwd / rw blend =====
        bwd = []
        fwd = []
        for b in range(B):
            bwb = PS([4, 512], "aux")
            for c in range(2):
                mmr(bwb[:], rw16[:, ds(8 * c + 4 * b, 4)], Ln[c][:],
                    start=(c == 0), stop=(c == 1))
            bwd.append(bwb)
        for b in range(B):
            fwb = PS([4, 512], "aux")
            for c in range(2):
                mmr(fwb[:], rw16[:, ds(8 * c + 4 * b, 4)], LTn[c][:],
                    start=(c == 0), stop=(c == 1))
            fwd.append(fwb)
        rwb = []
        for b in range(B):
            blk = ds(256 * b, 256)
            rwbb = T_([4, 256], f"rwb{b}")
            rm1c = T_([4, 1], f"rm1c{b}")
            v.tensor_tensor(rm1c[:], rm_m[1][:, b:b + 1],
                            smT[ds(64 * b, 4), ds(SM_LNU, 1)], op=OP.mult)
            v.tensor_scalar_mul(rwbb[:], bwd[b][:, blk],
                                rm_m[0][:, b:b + 1])
            v.scalar_tensor_tensor(rwbb[:], shx[ds(64 * b, 4), 256:512],
                                   rm1c[:], rwbb[:], op0=OP.mult, op1=OP.add)
            v.scalar_tensor_tensor(rwbb[:], fwd[b][:, blk],
                                   rm_m[2][:, b:b + 1], rwbb[:],
                                   op0=OP.mult, op1=OP.add)
            rwb.append(rwbb)
        for c in range(2):
            for b in range(B):
                tp_(smT[:, ds(SM_RWT + 8 * c + 4 * b, 4)],
                    rwb[b][:, ds(128 * c, 128)], ident[0:4, 0:4],
                    skip_group_check=True)
        rwn16 = C_([128, 16], "rw16", FPR)
        v.tensor_copy(rwn16[:], smT[:, ds(SM_RWT, 16)])

        # ===== rv =====
        rv_sb = []
        for b in range(B):
            rvb = PS([4, 64], "ctrl")
            for c in range(2):
                mmr(rvb[:], rwn16[:, ds(8 * c + 4 * b, 4)],
                    Msn[c][:, ds(64 * b, 64)], start=(c == 0), stop=(c == 1))
            rvsb = T_([4, 64], f"rv_sb{b}")
            v.tensor_copy(rvsb[:], rvb[:])
            rv_sb.append(rvsb)
        for b in range(B):
            tp_(smT[0:64, ds(SM_RVT + 4 * b, 4)], rv_sb[b][:],
                ident[0:4, 0:4], skip_group_check=True)
        rvn128 = C_([128, 4], "rvT128", FPR)
        for b in range(B):
            quad = smT[0:64, ds(SM_RVT + 4 * b, 4)].rearrange(
                "w (j k) -> w k j", k=2)
            v.tensor_copy(rvn128[0:64, ds(2 * b, 2)], quad[:, 0, :])
            v.tensor_copy(rvn128[64:128, ds(2 * b, 2)], quad[:, 1, :])

        # ===== output =====
        po = PS([2, O], "ctrl")
        for k in range(4):
            mmr(po[:], hT[:, ds(2 * k, 2)], wo_sb[k][:], start=(k == 0),
                stop=False)
        for j in range(2):
            lhs = rvn128[:].rearrange("p (b j) -> p j b", j=2)[:, j, :]
            mmr(po[:], lhs, wm2[j][:], start=False, stop=(j == 1))
        out_sb = T_([2, O], "out_sb")
        sc.activation(out_sb[:], po[:], AF.Copy)
        dma(out=out_d.ap().rearrange("t b o -> (t b) o")[ds(2 * t_step, 2), :],
            in_=out_sb[:])

        if dbg is not None and last:
            dma(out=dbg["h"].ap(), in_=h_sb[:])
            dma(out=dbg["sig"].ap(), in_=sig[:])
            dma(out=dbg["cw"].ap()[0:1], in_=c2cw[0][:])
            dma(out=dbg["cw"].ap()[1:2], in_=c2cw[1][:])
            dma(out=dbg["ret"].ap(), in_=ret4[:])
            dma(out=dbg["u"].ap(), in_=un_col[:])
            dma(out=dbg["a"].ap(), in_=a_col[:])
            dma(out=dbg["ww"].ap(), in_=wwn_row2[:])
            dma(out=dbg["mt"].ap(), in_=MTn[:])
            dma(out=dbg["rn"].ap(), in_=rnn_row2[:])
            dma(out=dbg["rc"].ap()[0:4], in_=shx[0:4, 256:512])
            dma(out=dbg["rc"].ap()[4:8], in_=shx[64:68, 256:512])
            dma(out=dbg["rw"].ap()[0:4], in_=rwb[0][:])
            dma(out=dbg["rw"].ap()[4:8], in_=rwb[1][:])
            dma(out=dbg["rv"].ap()[0:4], in_=rv_sb[0][:])
            dma(out=dbg["rv"].ap()[4:8], in_=rv_sb[1][:])
            dma(out=dbg["L0"].ap(), in_=Ln[0][:])
            dma(out=dbg["LT0"].ap(), in_=LTn[0][:])
            dma(out=dbg["p"].ap(), in_=pn_row2[:])
            dma(out=dbg["lnu"].ap(), in_=lnu_col[:])
            dma(out=dbg["eqc"].ap(), in_=eqc[:])
            dma(out=dbg["A1"].ap(), in_=A1[:])

        MT, Ms, L, LT = MTn, Msn, Ln, LTn
        u_col, ww_col = un_col, wwn_col
        ww_2r, ww_row2, ww_blk = wwn_2r, wwn_row2, wwn_blk
        p_2r, p_blk, p_row2 = pn_2r, pn_blk, pn_row2
        rw16, rvT128, rn_row2 = rwn16, rvn128, rnn_row2


# ---------------------------------------------------------------------------
# Public entry point
# ---------------------------------------------------------------------------
_T, _BFULL, _NCORES = 64, 16, 8
_cache = {}


def _get_nc(T=_T, debug=False, fix=True):
    key = ("nc", T, debug, fix)
    if key not in _cache:
        nc = bass.Bass("TRN2")
        build(nc, T, debug=debug)
        if fix:
            fix_sync_waits(nc)
        _cache[key] = nc
    return _cache[key]


def kernel(**inputs):
    x = np.ascontiguousarray(np.asarray(inputs["x"], dtype=np.float32))
    shared = {
        k: np.ascontiguousarray(np.asarray(inputs[k], dtype=np.float32))
        for k in ("W_hid", "b_hid", "W_iface", "W_out", "W_memout")
    }
    assert x.shape == (_T, _BFULL, I)
    nc = _get_nc()
    in_maps = []
    for core in range(_NCORES):
        shard = np.ascontiguousarray(x[:, core * B:(core + 1) * B, :])
        m = {"x": shard}
        m.update(shared)
        in_maps.append(m)
    res = run_bass_kernel_spmd(nc, in_maps, core_ids=list(range(_NCORES)))
    out = np.empty((_T, _BFULL, O), dtype=np.float32)
    for core in range(_NCORES):
        out[:, core * B:(core + 1) * B, :] = res.results[core]["out"]
    return out


# revision 15
# speedup vs baseline: 1.3877x; 1.0300x over previous
"""Optimized TRN2 Bass kernel for the DNC (NeuCom) recurrence — v2.

Key changes vs v1 baseline:
- Single activation table (natural_log_exp): sigmoid via exp + DVE reciprocal,
  inverse norms via exp(-0.5*ln(q+eps)), oneplus via ln(1+exp(x)).
- float32r matmuls for all large-free matmuls (4x fewer PE cycles/row).
- Block-diagonal fused matmuls: both batches in one instruction for sims,
  M update, L/LT updates, fwd/bwd.
- L^T maintained as a carry with elementwise updates (no per-step transposes).
- Allocation (usage sort) via masked log-sum instead of explicit permutation
  matmuls + scan: a_i = (1-u_i) * exp(sum_{j sorted before i} ln u_j).
  Exact ties (which persist among never-written slots) are handled by an
  equality tie-count term; compares run in ln-space so lt/eq stay consistent.
- Engine rebalance: copies on Activation, some elementwise on Pool.

Hardware constraint honored throughout: every SBUF operand of a non-DMA
instruction must start at partition 0/32/64/96 (PSUM operands are exempt),
so per-batch row data lives in separate base-0 tiles and [2,X] stacked tiles
are built via one-hot selector matmuls accumulated in PSUM.
"""
from contextlib import ExitStack

import numpy as np

import concourse.bass as bass
import concourse.mybir as mybir
import concourse.tile as tile
from concourse.bass import ds, ts
from concourse.bass_utils import run_bass_kernel_spmd

_ctr = [0]


def fix_sync_waits(nc):
    """walrus accepts at most ONE sync-wait per instruction; split extras."""
    for f in nc.m.functions:
        for bb in f.blocks:
            new_insts = []
            for inst in bb.instructions:
                si = inst.sync_info
                waits = list(si.on_wait) if si is not None else []
                if len(waits) > 1:
                    extra, keep = waits[:-1], waits[-1:]
                    while extra:
                        chunk, extra = extra[:1], extra[1:]
                        _ctr[0] += 1
                        nop = mybir.InstNoOp(
                            name=f"WFIX-{_ctr[0]}",
                            engine=inst.engine,
                            sync_info=mybir.SyncInfo(on_wait=chunk, on_update=[]),
                            text_hint="waitfix",
                        )
                        new_insts.append(nop)
                    si.on_wait = keep
                new_insts.append(inst)
            bb.instructions = new_insts
    return nc


FP = mybir.dt.float32
FPR = mybir.dt.float32r
AF = mybir.ActivationFunctionType
OP = mybir.AluOpType
AX = mybir.AxisListType

N, Wd, R, B = 256, 64, 4, 2
H, I, O, IF = 512, 512, 512, 471

C_RK, C_RB, C_WK, C_WB, C_EV, C_WV, C_FG, C_AG, C_WG, C_RM = (
    0, 256, 260, 324, 325, 389, 453, 457, 458, 459)

EQ_ON_POOL = True       # tie-count stt ops on Pool (else DVE)
LT_ADD_ON_POOL = True   # LT "+b2" adds on Pool (else DVE)


def r_(ap):
    return ap.bitcast(FPR)


def build(nc: bass.Bass, T: int, debug: bool = False):
    x_d = nc.dram_tensor("x", [T, B, I], FP, kind="ExternalInput")
    wh_d = nc.dram_tensor("W_hid", [I + R * Wd, H], FP, kind="ExternalInput")
    bh_d = nc.dram_tensor("b_hid", [H], FP, kind="ExternalInput")
    wi_d = nc.dram_tensor("W_iface", [H, IF], FP, kind="ExternalInput")
    wo_d = nc.dram_tensor("W_out", [H, O], FP, kind="ExternalInput")
    wm_d = nc.dram_tensor("W_memout", [R * Wd, O], FP, kind="ExternalInput")
    out_d = nc.dram_tensor("out", [T, B, O], FP, kind="ExternalOutput")
    dbg = None
    if debug:
        dbg = {k: nc.dram_tensor(f"dbg_{k}", s, FP, kind="ExternalOutput")
               for k, s in [("h", [2, H]), ("sig", [2, 134]),
                            ("cw", [2, 256]), ("ret", [128, 4]),
                            ("u", [128, 4]), ("a", [128, 4]),
                            ("ww", [1, 512]), ("mt", [64, 512]),
                            ("rn", [1, 512]), ("rc", [8, 256]),
                            ("rw", [8, 256]), ("rv", [8, 64]),
                            ("L0", [128, 512]), ("LT0", [128, 512]),
                            ("p", [1, 512]), ("lnu", [128, 4]),
                            ("eqc", [128, 4]), ("A1", [128, 4])]}
    with tile.TileContext(nc) as tc:
        with ExitStack() as ctx:
            _build(ctx, tc, nc, T, x_d, wh_d, bh_d, wi_d, wo_d, wm_d, out_d,
                   dbg)
    return nc


def _build(ctx, tc, nc, T, x_d, wh_d, bh_d, wi_d, wo_d, wm_d, out_d, dbg):
    per = ctx.enter_context(tc.tile_pool(name="persist", bufs=1))
    car = ctx.enter_context(tc.tile_pool(name="carry", bufs=2))
    tmp = ctx.enter_context(tc.tile_pool(name="tmp", bufs=2))
    ps = ctx.enter_context(tc.tile_pool(name="ps", bufs=2, space="PSUM"))

    dma = nc.sync.dma_start
    v = nc.vector
    sc = nc.scalar
    gp = nc.gpsimd
    te = nc.tensor
    mm = te.matmul

    def mmr(out, lhsT, rhs, **kw):
        mm(out, r_(lhsT), r_(rhs), **kw)

    def tp_(out, in_, idn, **kw):
        mm(out, in_, idn, is_transpose=True, **kw)

    def T_(shape, tag, dt=FP):
        return tmp.tile(shape, dt, tag=tag, name=tag)

    def C_(shape, tag, dt=FP):
        return car.tile(shape, dt, tag=tag, name=tag)

    def P_(shape, tag, dt=FP):
        return per.tile(shape, dt, tag=tag, name=tag)

    def PS(shape, tag, bufs=None):
        return ps.tile(shape, FP, tag=tag, name=tag, bufs=bufs)

    # ---------------- constants ----------------
    ones_full = P_([128, 512], "ones_full")
    v.memset(ones_full[:], 1.0)
    ident = P_([128, 128], "ident")
    v.tensor_copy(ident[:], ones_full[:, 0:128])
    gp.affine_select(ident[:], ident[:], pattern=[[-1, 128]],
                     compare_op=OP.is_equal, fill=0.0, base=0,
                     channel_multiplier=1)
    # blockmask[b, n] = 1 if n in batch-b block
    blockmask = P_([2, 512], "blockmask")
    v.tensor_copy(blockmask[:], ones_full[0:2, :])
    gp.affine_select(blockmask[:], blockmask[:], pattern=[[1, 512]],
                     compare_op=OP.is_ge, fill=0.0, base=0,
                     channel_multiplier=-256)
    gp.affine_select(blockmask[:], blockmask[:], pattern=[[-1, 512]],
                     compare_op=OP.is_ge, fill=0.0, base=255,
                     channel_multiplier=256)
    jmask = []
    for c in range(2):
        jm = P_([128, 256], f"jmask{c}")
        gp.affine_select(jm[:], ones_full[:, 0:256], pattern=[[-1, 256]],
                         compare_op=OP.is_ge, fill=0.0, base=128 * c - 1,
                         channel_multiplier=1)
        jmask.append(jm)
    negblockmask = P_([2, 512], "negblockmask", FPR)
    v.tensor_scalar_mul(negblockmask[:], blockmask[:], -1.0)
    negones_row = P_([1, 128], "negones_row", FPR)
    v.tensor_scalar_mul(negones_row[:], ones_full[0:1, 0:128], -1.0)
    onesR = P_([128, 512], "onesR", FPR)
    v.tensor_copy(onesR[:], ones_full[:])
    identR = P_([128, 128], "identR", FPR)
    v.tensor_copy(identR[:], ident[:])
    cE12 = P_([128, 1], "cE12")
    v.memset(cE12[:], 1e-12)
    cE37 = P_([128, 1], "cE37")
    v.memset(cE37[:], 1e-37)
    # one-hot selectors
    selrow = []  # [1,2] rows for scatter (lhsT)
    for b in range(B):
        sf = P_([1, 2], f"selrowF{b}")
        v.memset(sf[:], 0.0)
        v.memset(sf[0:1, b:b + 1], 1.0)
        s = P_([1, 2], f"selrow{b}", FPR)
        v.tensor_copy(s[:], sf[:])
        selrow.append(s)
    selcol0 = P_([2, 1], "selcol0")
    v.memset(selcol0[:], 0.0)
    v.memset(selcol0[0:1, 0:1], 1.0)
    selcol1 = P_([2, 1], "selcol1")
    v.tensor_sub(selcol1[:], ones_full[0:2, 0:1], selcol0[:])
    selcol = [selcol0, selcol1]

    # ---------------- weights ----------------
    def load_w(dram, n_tiles, cols, name, row0=0, rows=128):
        out = []
        for k in range(n_tiles):
            t = P_([rows, cols], f"{name}{k}", FPR)
            nc.gpsimd.dma_start(out=t[:],
                                in_=dram.ap()[ds(row0 + k * rows, rows), :])
            out.append(t)
        return out

    wh_sb = load_w(wh_d, 4, H, "wh")
    wrv2 = load_w(wh_d, 2, H, "wrv2", row0=512, rows=128)
    # W_iface padded to even free size (f32r matmul ISA constraint)
    wi_sb = []
    for k in range(4):
        t = P_([128, IF + 1], f"wi{k}", FPR)
        v.tensor_scalar_mul(t[:], ones_full[:, 0:IF + 1], 0.0)
        nc.gpsimd.dma_start(out=t[:, 0:IF],
                            in_=wi_d.ap()[ds(k * 128, 128), :])
        wi_sb.append(t)
    wo_sb = load_w(wo_d, 4, O, "wo")
    wm2 = load_w(wm_d, 2, O, "wm2", rows=128)
    bh_sb = P_([1, H], "bh")
    dma(out=bh_sb[:], in_=bh_d.ap()[None, :])

    # ---------------- Xp precompute ----------------
    TB = T * B
    assert TB <= 128
    xnat = P_([128, I], "xnat")
    dma(out=xnat[:TB, :], in_=x_d.ap().rearrange("t b i -> (t b) i"))
    xt_sb = []
    for k in range(4):
        t = P_([128, 128], f"xt{k}", FPR)
        xtp = PS([128, 512], "ctrl")
        tp_(xtp[:, 0:TB], xnat[:TB, ts(k, 128)], ident[:TB, :TB])
        v.tensor_copy(t[:, :TB], xtp[:, 0:TB])
        xt_sb.append(t)
    xp_sb = P_([128, H], "xp", FPR)
    xp_ps = PS([128, H], "ctrl")
    for k in range(4):
        mmr(xp_ps[:TB, :], xt_sb[k][:, :TB], wh_sb[k][:], start=(k == 0),
            stop=False)
    mm(xp_ps[:TB, :], ones_full[0:1, :TB], bh_sb[:], start=False, stop=True)
    v.tensor_copy(xp_sb[:TB, :], xp_ps[:TB, :])

    # ---------------- carries (initial) ----------------
    MT = C_([64, 512], "MT", FPR)
    v.tensor_scalar_mul(MT[:], ones_full[0:64, :], 1e-6)
    Ms = []
    for c in range(2):
        m = C_([128, 128], f"Ms{c}", FPR)
        v.tensor_scalar_mul(m[:], ones_full[:, 0:128], 1e-6)
        Ms.append(m)
    L = []
    LT = []
    for c in range(2):
        l = C_([128, 512], f"L{c}", FPR)
        v.tensor_scalar_mul(l[:], ones_full[:], 0.0)
        L.append(l)
        lt = C_([128, 512], f"LT{c}", FPR)
        v.tensor_scalar_mul(lt[:], ones_full[:], 0.0)
        LT.append(lt)
    u_col = C_([128, 4], "u_col")
    v.memset(u_col[:], 0.0)
    ww_col = C_([128, 4], "ww_col")
    v.memset(ww_col[:], 0.0)
    ww_2r = C_([2, 256], "ww_2r", FPR)
    v.tensor_scalar_mul(ww_2r[:], ones_full[0:2, 0:256], 0.0)
    ww_row2 = C_([1, 512], "ww_row2", FPR)
    v.tensor_scalar_mul(ww_row2[:], ones_full[0:1, :], 0.0)
    ww_blk = C_([2, 512], "ww_blk", FPR)
    v.tensor_scalar_mul(ww_blk[:], ones_full[0:2, :], 0.0)
    p_2r = C_([2, 256], "p_2r", FPR)
    v.tensor_scalar_mul(p_2r[:], ones_full[0:2, 0:256], 0.0)
    p_blk = C_([2, 512], "p_blk", FPR)
    v.tensor_scalar_mul(p_blk[:], ones_full[0:2, :], 0.0)
    p_row2 = C_([1, 512], "p_row2", FPR)
    v.tensor_scalar_mul(p_row2[:], ones_full[0:1, :], 0.0)
    rw16 = C_([128, 16], "rw16", FPR)
    v.tensor_scalar_mul(rw16[:], ones_full[:, 0:16], 0.0)
    rvT128 = C_([128, 4], "rvT128", FPR)
    v.tensor_scalar_mul(rvT128[:], ones_full[:, 0:4], 0.0)
    rn_row2 = C_([1, 512], "rn_row2")
    v.memset(rn_row2[:], float((Wd * 1e-12 + 1e-12) ** -0.5))

    # smT column map (scratch PSUM bank, tag "sm"):
    SM_HTP, SM_RMG, SM_C12, SM_RST, SM_KT, SM_FGB = 0, 8, 14, 18, 22, 32
    SM_LNU, SM_AT, SM_WC, SM_RWT, SM_RVT = 48, 176, 304, 308, 324

    # ---------------- steps ----------------
    for t_step in range(T):
        last = (t_step == T - 1)
        smT = PS([128, 512], "sm", bufs=1)

        # ===== controller h =====
        h_ps = PS([2, H], "ctrl")
        for j in range(2):
            lhs = rvT128[:].rearrange("p (b j) -> p j b", j=2)[:, j, :]
            mmr(h_ps[:], lhs, wrv2[j][:], start=(j == 0), stop=False)
        mmr(h_ps[:], identR[:TB, ds(2 * t_step, 2)], xp_sb[:TB, :],
            start=False, stop=True)
        h_sb = T_([2, H], "h_sb")
        sc.activation(h_sb[:], h_ps[:], AF.Relu)
        for k in range(4):
            tp_(smT[:, ds(SM_HTP + 2 * k, 2)], h_sb[:, ts(k, 128)],
                ident[0:2, 0:2], skip_group_check=True)
        hT = T_([128, 8], "hT", FPR)
        v.tensor_copy(hT[:], smT[:, ds(SM_HTP, 8)])

        # ===== iface =====
        if_ps = PS([2, IF + 1], "ctrl")
        for k in range(4):
            mmr(if_ps[:], hT[:, ds(2 * k, 2)], wi_sb[k][:], start=(k == 0),
                stop=(k == 3))

        # -- iface activations (full 2-row ops only) --
        esig = T_([2, 134], "esig")
        sc.activation(esig[:], if_ps[:, C_EV:C_RM], AF.Exp, scale=-1.0)
        v.tensor_scalar_add(esig[:], esig[:], 1.0)
        sig = T_([2, 134], "sig")
        v.reciprocal(sig[:], esig[:])
        # sig: [,0:64]=ev  [,128:132]=fg  [,132:133]=ag  [,133:134]=wg

        rme = T_([2, 12], "rme")
        sc.activation(rme[:], if_ps[:, C_RM:C_RM + 12], AF.Exp)
        rmden = T_([2, 4], "rmden")
        v.tensor_reduce(rmden[:], rme[:].rearrange("b (r m) -> b r m", m=3),
                        axis=AX.X, op=OP.add)
        v.reciprocal(rmden[:], rmden[:])
        rmG = T_([2, 12], "rmG")
        v.tensor_tensor(
            out=rmG[:].rearrange("b (m r) -> b m r", r=4),
            in0=rme[:].rearrange("b (r m) -> b m r", m=3),
            in1=rmden[:].rearrange("b (u r) -> b u r", u=1).broadcast_to(
                [2, 3, 4]),
            op=OP.mult)
        for m3 in range(3):
            tp_(smT[0:4, ds(SM_RMG + 2 * m3, 2)], rmG[:, ds(4 * m3, 4)],
                ident[0:2, 0:2], skip_group_check=True)
        rm_m = []
        for m3 in range(3):
            rmt = T_([4, 2], f"rm_m{m3}")
            if m3 == 1:
                sc.activation(rmt[:], smT[0:4, ds(SM_RMG + 2 * m3, 2)],
                              AF.Copy)
            else:
                v.tensor_copy(rmt[:], smT[0:4, ds(SM_RMG + 2 * m3, 2)])
            rm_m.append(rmt)

        # gates -> transposed rows [1,2]
        c1 = T_([2, 1], "c1")
        v.tensor_tensor(c1[:], sig[:, 132:133], sig[:, 133:134], op=OP.mult)
        c2 = T_([2, 1], "c2")
        v.tensor_scalar(c2[:], sig[:, 132:133], -1.0, 1.0, op0=OP.mult,
                        op1=OP.add)
        v.tensor_mul(c2[:], c2[:], sig[:, 133:134])
        tp_(smT[0:1, ds(SM_C12, 2)], c1[:, 0:1], ident[0:2, 0:2],
            skip_group_check=True)
        tp_(smT[0:1, ds(SM_C12 + 2, 2)], c2[:, 0:1], ident[0:2, 0:2],
            skip_group_check=True)
        c1T = T_([1, 2], "c1T")
        c2T = T_([1, 2], "c2T")

        # oneplus(rb|wb) = 1 + ln(1+exp(x)); key norms; rs = (1+sp)/||k||
        bw5 = T_([2, 5], "bw5")
        sc.activation(bw5[:, 0:4], if_ps[:, C_RB:C_RB + 4], AF.Copy)
        sc.activation(bw5[:, 4:5], if_ps[:, C_WB:C_WB + 1], AF.Copy)
        sc.activation(bw5[:], bw5[:], AF.Exp)
        sc.activation(bw5[:], bw5[:], AF.Ln, bias=1.0)
        ifk = T_([2, 325], "ifk")
        v.tensor_copy(ifk[:], if_ps[:, 0:C_EV])
        ksq = T_([2, 325], "ksq")
        v.tensor_tensor(ksq[:], ifk[:], ifk[:], op=OP.mult)
        kn2 = T_([2, 5], "kn2")
        v.tensor_reduce(kn2[:, 0:4],
                        ksq[:, 0:256].rearrange("b (k w) -> b k w", w=64),
                        axis=AX.X, op=OP.add)
        v.tensor_reduce(kn2[:, 4:5], ksq[:, C_WK:C_WK + 64], axis=AX.X,
                        op=OP.add)
        sc.activation(kn2[:], kn2[:], AF.Ln, bias=cE12[0:2, 0:1])
        invkn = T_([2, 5], "invkn")
        sc.activation(invkn[:], kn2[:], AF.Exp, scale=-0.5)
        rs = T_([2, 5], "rs")
        v.scalar_tensor_tensor(rs[:], bw5[:], 1.0, invkn[:], op0=OP.add,
                               op1=OP.mult)
        # transpose read scales [2,4]->[4,2] and write scale [2,1]->[1,2]
        tp_(smT[0:4, ds(SM_RST, 2)], rs[:, 0:4], ident[0:2, 0:2],
            skip_group_check=True)
        tp_(smT[0:1, ds(SM_RST + 2, 2)], rs[:, 4:5], ident[0:2, 0:2],
            skip_group_check=True)
        rsRT = T_([4, 2], "rsRT")
        sc.activation(rsRT[:], smT[0:4, ds(SM_RST, 2)], AF.Copy)
        rsWT = T_([1, 2], "rsWT")
        sc.activation(rsWT[:], smT[0:1, ds(SM_RST + 2, 2)], AF.Copy)
        sc.activation(c1T[:], smT[0:1, ds(SM_C12, 2)], AF.Copy)
        sc.activation(c2T[:], smT[0:1, ds(SM_C12 + 2, 2)], AF.Copy)

        # ===== keys (raw; scales applied to sims) =====
        tp_(smT[:, ds(SM_KT, 2)], ifk[:, 0:128], ident[0:2, 0:2],
            skip_group_check=True)
        tp_(smT[:, ds(SM_KT + 2, 2)], ifk[:, 128:256], ident[0:2, 0:2],
            skip_group_check=True)
        tp_(smT[0:64, ds(SM_KT + 4, 2)], ifk[:, C_WK:C_WK + 64],
            ident[0:2, 0:2], skip_group_check=True)
        keysR = T_([64, 8], "keysR", FPR)
        keysW = T_([64, 8], "keysW", FPR)
        v.tensor_scalar_mul(keysW[:], ones_full[0:64, 0:8], 0.0)
        key_engs = [v, sc, v, sc]
        for kk in range(4):
            src = smT[ds(64 * (kk % 2), 64), ds(SM_KT + 2 * (kk // 2), 2)]
            eng = key_engs[kk]
            if eng is sc:
                sc.activation(
                    keysR[:].rearrange("w (b r) -> w r b", r=4)[:, kk, :],
                    src, AF.Copy)
            else:
                eng.tensor_copy(
                    keysR[:].rearrange("w (b r) -> w r b", r=4)[:, kk, :],
                    src)
        for b in range(B):
            v.tensor_copy(keysW[:, ds(4 * b, 1)],
                          smT[0:64, ds(SM_KT + 4 + b, 1)])

        # ===== cw on old M =====
        simw = []
        for b in range(B):
            swb = PS([2, 512], "ctrl")
            mmr(swb[:], keysW[:, ds(4 * b, 2)], MT[:], start=True, stop=True)
            simw.append(swb)
        shx = PS([128, 512], "shx", bufs=1)  # cw/rc rows 0:4,64:68; rn8 r32
        c2cw = []
        for b in range(B):
            r0 = ds(64 * b, 1)
            cwdb = T_([1, 1], f"cwd{b}")
            v.scalar_tensor_tensor(shx[r0, 0:256],
                                   simw[b][0:1, ds(256 * b, 256)],
                                   rsWT[0:1, b:b + 1],
                                   rn_row2[0:1, ds(256 * b, 256)],
                                   op0=OP.mult, op1=OP.mult)
            sc.activation(shx[r0, 256:512], shx[r0, 0:256], AF.Exp,
                          accum_out=cwdb[:])
            v.reciprocal(cwdb[:], cwdb[:])
            c2cwb = T_([1, 256], f"c2cw{b}")
            v.tensor_scalar(c2cwb[:], shx[r0, 256:512], cwdb[:],
                            c2T[0:1, b:b + 1], op0=OP.mult, op1=OP.mult)
            c2cw.append(c2cwb)

        # ===== usage =====
        fgrow = []
        for b in range(B):
            fgp = PS([1, 4], "ctrl")
            mm(fgp[:], selcol[b][:], sig[:, 128:132], start=True, stop=True)
            fgs = T_([1, 4], f"fgrow{b}")
            v.tensor_copy(fgs[:], fgp[:])
            fgrow.append(fgs)
        for c in range(2):
            for b in range(B):
                mm(smT[:, ds(SM_FGB + 8 * c + 4 * b, 4)],
                   ones_full[0:1, 0:128], fgrow[b][:], start=True, stop=True,
                   skip_group_check=True)
        m1 = T_([128, 16], "m1")
        v.scalar_tensor_tensor(m1[:], smT[:, ds(SM_FGB, 16)], -1.0, rw16[:],
                               op0=OP.mult, op1=OP.mult)
        m2 = T_([128, 16], "m2")
        sc.activation(m2[:], m1[:], AF.Identity, bias=1.0)
        q8 = T_([128, 8], "q8")
        gp.tensor_tensor(q8[:],
                        m2[:].rearrange("p (g r) -> p g r", r=2)[:, :, 0],
                        m2[:].rearrange("p (g r) -> p g r", r=2)[:, :, 1],
                        op=OP.mult)
        ret4 = T_([128, 4], "ret4")
        v.tensor_tensor(ret4[:],
                        q8[:].rearrange("p (h u) -> p h u", u=2)[:, :, 0],
                        q8[:].rearrange("p (h u) -> p h u", u=2)[:, :, 1],
                        op=OP.mult)
        t1 = T_([128, 4], "t1")
        gp.tensor_tensor(t1[:], u_col[:], ww_col[:], op=OP.mult)
        t2 = T_([128, 4], "t2")
        gp.tensor_add(t2[:], u_col[:], ww_col[:])
        v.tensor_sub(t2[:], t2[:], t1[:])
        un_col = C_([128, 4], "u_col")
        v.tensor_tensor(un_col[:], t2[:], ret4[:], op=OP.mult)

        # ===== allocation =====
        lnu_col = T_([128, 4], "lnu_col")
        sc.activation(lnu_col[:], un_col[:], AF.Ln, bias=cE37[:, 0:1])
        ut_ps = PS([1, 512], "ctrl")
        for j in range(4):
            b, c = j // 2, j % 2
            tp_(ut_ps[0:1, ds(128 * j, 128)],
                un_col[:, ds(2 * c + b, 1)], ident[:],
                skip_group_check=True)
        u_row2 = T_([1, 512], "u_row2")
        sc.activation(u_row2[:], ut_ps[:], AF.Copy)
        # per-batch PSUM bank: broadcast u_b; ln(u) goes to SBUF
        lnubc_sb = T_([128, 512], "lnubc_sb")
        ubcln = []
        for b in range(B):
            ub = PS([128, 256], "ubcln", bufs=1)
            mm(ub[:], ones_full[0:1, 0:128],
               u_row2[0:1, ds(256 * b, 256)], start=True, stop=True)
            sc.activation(lnubc_sb[:, ds(256 * b, 256)], ub[:], AF.Ln,
                          bias=cE37[:, 0:1])
            ubcln.append(ub)
        A1 = T_([128, 4], "A1")
        eqc = T_([128, 4], "eqc")
        for c in range(2):
            for b in range(B):
                col = ds(2 * c + b, 1)
                scr = T_([128, 256], f"scr{c}{b}")
                v.scalar_tensor_tensor(scr[:], ubcln[b][:],
                                       un_col[:, col],
                                       lnubc_sb[:, ds(256 * b, 256)],
                                       op0=OP.is_lt, op1=OP.mult,
                                       accum_out=A1[:, col])
                scr2 = T_([128, 256], f"scr2{c}{b}")
                v.scalar_tensor_tensor(scr2[:], ubcln[b][:],
                                       un_col[:, col], jmask[c][:],
                                       op0=OP.is_equal, op1=OP.mult,
                                       accum_out=eqc[:, col])
        A = T_([128, 4], "A")
        v.tensor_tensor(A[:], eqc[:], lnu_col[:], op=OP.mult)
        v.tensor_add(A[:], A[:], A1[:])
        cpx = T_([128, 4], "cpx")
        sc.activation(cpx[:], A[:], AF.Exp)
        onemu = T_([128, 4], "onemu")
        v.tensor_scalar(onemu[:], un_col[:], -1.0, 1.0, op0=OP.mult,
                        op1=OP.add)
        a_col = T_([128, 4], "a_col")
        v.tensor_tensor(a_col[:], onemu[:], cpx[:], op=OP.mult)

        # ===== ww (row space, written into [1,512] row) =====
        at_ps = PS([1, 512], "ctrl")
        for j in range(4):
            b, c = j // 2, j % 2
            tp_(at_ps[0:1, ds(128 * j, 128)], a_col[:, ds(2 * c + b, 1)],
                ident[:], skip_group_check=True)
        wwn_row2 = C_([1, 512], "ww_row2", FPR)
        wws4 = T_([1, 4], "wws4")
        for b in range(B):
            for c in range(2):
                v.scalar_tensor_tensor(
                    wwn_row2[0:1, ds(256 * b + 128 * c, 128)],
                    at_ps[0:1, ds(128 * (2 * b + c), 128)],
                    c1T[0:1, b:b + 1],
                    c2cw[b][0:1, ds(128 * c, 128)],
                    op0=OP.mult, op1=OP.add,
                    accum_out=wws4[0:1, ds(2 * b + c, 1)])
        wws2 = T_([1, 2], "wws2")
        v.tensor_reduce(wws2[:], wws4[:].rearrange("o (b c) -> o b c", c=2),
                        axis=AX.X, op=OP.add)
        # stacked [2,256] / [2,512] forms via selector-scatter in PSUM
        ww2r_ps = PS([2, 256], "ctrl")
        for b in range(B):
            mmr(ww2r_ps[:], selrow[b][:], wwn_row2[0:1, ds(256 * b, 256)],
                start=(b == 0), stop=(b == 1))
        wwn_2r = C_([2, 256], "ww_2r", FPR)
        v.tensor_copy(wwn_2r[:], ww2r_ps[:])
        wwblk_ps = PS([2, 512], "ctrl")
        for b in range(B):
            mmr(wwblk_ps[:, ds(256 * b, 256)], selrow[b][:],
                wwn_row2[0:1, ds(256 * b, 256)], start=True, stop=True,
                skip_group_check=True)
        wwn_blk = C_([2, 512], "ww_blk", FPR)
        sc.activation(wwn_blk[:], wwblk_ps[:], AF.Copy)
        # ww_col via transposes of stacked halves (cols come out as (b))
        for c in range(2):
            tp_(smT[:, ds(SM_WC + 2 * c, 2)].bitcast(FPR),
                wwn_2r[:, ds(128 * c, 128)], identR[0:2, 0:2],
                skip_group_check=True)
        wwn_col = C_([128, 4], "ww_col")
        v.tensor_copy(wwn_col[:], smT[:, ds(SM_WC, 4)])

        # ===== L / LT updates (old p as rhs) =====
        a2 = []
        for c in range(2):
            a2c = PS([128, 512], "a2", bufs=1)
            mmr(a2c[:], wwn_2r[:, ds(128 * c, 128)], negblockmask[:],
                start=True, stop=False)
            mmr(a2c[:], negones_row[:], wwn_row2[:], start=False,
                stop=True)
            a2.append(a2c)
        Ln = []
        LTn = []
        for c in range(2):
            b_c = PS([128, 512], "aux")
            mmr(b_c[:], wwn_2r[:, ds(128 * c, 128)], p_blk[:], start=True,
                stop=True)
            b2_c = PS([128, 512], "aux")
            mmr(b2_c[:], p_2r[:, ds(128 * c, 128)], wwn_blk[:], start=True,
                stop=True)
            lnc = C_([128, 512], f"L{c}", FPR)
            v.scalar_tensor_tensor(lnc[:], a2[c][:], 1.0, L[c][:], op0=OP.add,
                                   op1=OP.mult)
            v.tensor_add(lnc[:], lnc[:], b_c[:])
            gp.affine_select(lnc[:], lnc[:], pattern=[[0, 2], [-1, 256]],
                             compare_op=OP.not_equal, fill=0.0, base=128 * c,
                             channel_multiplier=1)
            Ln.append(lnc)
            ltc = C_([128, 512], f"LT{c}", FPR)
            v.scalar_tensor_tensor(ltc[:], a2[c][:], 1.0, LT[c][:],
                                   op0=OP.add, op1=OP.mult)
            v.tensor_add(ltc[:], ltc[:], b2_c[:])
            gp.affine_select(ltc[:], ltc[:], pattern=[[0, 2], [-1, 256]],
                             compare_op=OP.not_equal, fill=0.0, base=128 * c,
                             channel_multiplier=1)
            LTn.append(ltc)

        # ===== p update (row space + stacked forms) =====
        pn_row2 = C_([1, 512], "p_row2", FPR)
        nws2 = T_([1, 2], "nws2")
        sc.activation(nws2[:], wws2[:], AF.Identity, bias=1.0, scale=-1.0)
        for b in range(B):
            v.scalar_tensor_tensor(pn_row2[0:1, ds(256 * b, 256)],
                                   p_row2[0:1, ds(256 * b, 256)],
                                   nws2[0:1, b:b + 1],
                                   wwn_row2[0:1, ds(256 * b, 256)],
                                   op0=OP.mult, op1=OP.add)
        p2r_ps = PS([2, 256], "ctrl")
        for b in range(B):
            mmr(p2r_ps[:], selrow[b][:], pn_row2[0:1, ds(256 * b, 256)],
                start=(b == 0), stop=(b == 1))
        pn_2r = C_([2, 256], "p_2r", FPR)
        v.tensor_copy(pn_2r[:], p2r_ps[:])
        pblk_ps = PS([2, 512], "ctrl")
        for b in range(B):
            mmr(pblk_ps[:, ds(256 * b, 256)], selrow[b][:],
                pn_row2[0:1, ds(256 * b, 256)], start=True, stop=True,
                skip_group_check=True)
        pn_blk = C_([2, 512], "p_blk", FPR)
        sc.activation(pn_blk[:], pblk_ps[:], AF.Copy)

        # ===== M update =====
        negev_2r = T_([2, 64], "negev_2r", FPR)
        v.tensor_scalar_mul(negev_2r[:], sig[:, 0:64], -1.0)
        wv_2r = T_([2, 64], "wv_2r", FPR)
        v.tensor_copy(wv_2r[:], if_ps[:, C_WV:C_WV + 64])
        q1 = PS([64, 512], "aux")
        mmr(q1[:], negev_2r[:], wwn_blk[:], start=True, stop=True)
        q2 = PS([64, 512], "aux")
        mmr(q2[:], wv_2r[:], wwn_blk[:], start=True, stop=True)
        MTn = C_([64, 512], "MT", FPR)
        v.scalar_tensor_tensor(MTn[:], q1[:], 1.0, MT[:], op0=OP.add,
                               op1=OP.mult)
        v.tensor_add(MTn[:], MTn[:], q2[:])
        # Ms via transposes of MTn
        mst = PS([128, 512], "aux")
        for c in range(2):
            for b in range(B):
                tp_(mst[:, ds(64 * (2 * c + b), 64)].bitcast(FPR),
                    MTn[0:64, ds(256 * b + 128 * c, 128)],
                    identR[0:64, 0:64], skip_group_check=True)
        Msn = []
        for c in range(2):
            msc = C_([128, 128], f"Ms{c}", FPR)
            eng = v if c == 0 else sc
            if eng is sc:
                sc.activation(msc[:], mst[:, ds(128 * c, 128)], AF.Copy)
            else:
                v.tensor_copy(msc[:], mst[:, ds(128 * c, 128)])
            Msn.append(msc)

        # ===== rnorm (new M) =====
        mt2 = T_([64, 512], "mt2", FPR)
        sc.activation(mt2[:], MTn[:], AF.Square)
        nq = PS([2, 512], "aux")
        mmr(nq[:], onesR[0:64, 0:2], mt2[:], start=True, stop=True)
        rnln = T_([1, 512], "rnln")
        sc.activation(rnln[:], nq[0:1, :], AF.Ln, bias=cE12[0:1, 0:1])
        rnn_row2 = C_([1, 512], "rn_row2")
        sc.activation(rnn_row2[:], rnln[:], AF.Exp, scale=-0.5)

        # ===== rc on new M =====
        simr = []
        for b in range(B):
            srb = PS([4, 512], "ctrl")
            mmr(srb[:], keysR[:, ds(4 * b, 4)], MTn[:], start=True, stop=True)
            simr.append(srb)
        for b in range(B):
            mm(shx[ds(32, 4), ds(256 * b, 256)], ones_full[0:1, 0:4],
               rnn_row2[0:1, ds(256 * b, 256)], start=True, stop=True,
               skip_group_check=True)
        rn8_sb = T_([4, 512], "rn8_sb")
        sc.activation(rn8_sb[:], shx[ds(32, 4), :], AF.Copy)
        for b in range(B):
            rr = ds(64 * b, 4)  # rc rows reuse cw rows (consumed)
            v.scalar_tensor_tensor(shx[rr, 0:256],
                                   simr[b][:, ds(256 * b, 256)],
                                   rsRT[:, b:b + 1],
                                   rn8_sb[:, ds(256 * b, 256)],
                                   op0=OP.mult, op1=OP.mult)
            sc.activation(shx[rr, 256:512], shx[rr, 0:256], AF.Exp,
                          accum_out=smT[ds(64 * b, 4), ds(SM_LNU, 1)])
            v.reciprocal(smT[ds(64 * b, 4), ds(SM_LNU, 1)],
                         smT[ds(64 * b, 4), ds(SM_LNU, 1)])

        # ===== fwd / bwd / rw blend =====
        bwd = []
        fwd = []
        for b in range(B):
            bwb = PS([4, 512], "aux")
            for c in range(2):
                mmr(bwb[:], rw16[:, ds(8 * c + 4 * b, 4)], Ln[c][:],
                    start=(c == 0), stop=(c == 1))
            bwd.append(bwb)
        for b in range(B):
            fwb = PS([4, 512], "aux")
            for c in range(2):
                mmr(fwb[:], rw16[:, ds(8 * c + 4 * b, 4)], LTn[c][:],
                    start=(c == 0), stop=(c == 1))
            fwd.append(fwb)
        rwb = []
        for b in range(B):
            blk = ds(256 * b, 256)
            rwbb = T_([4, 256], f"rwb{b}")
            rm1c = T_([4, 1], f"rm1c{b}")
            v.tensor_tensor(rm1c[:], rm_m[1][:, b:b + 1],
                            smT[ds(64 * b, 4), ds(SM_LNU, 1)], op=OP.mult)
            v.tensor_scalar_mul(rwbb[:], bwd[b][:, blk],
                                rm_m[0][:, b:b + 1])
            v.scalar_tensor_tensor(rwbb[:], shx[ds(64 * b, 4), 256:512],
                                   rm1c[:], rwbb[:], op0=OP.mult, op1=OP.add)
            v.scalar_tensor_tensor(rwbb[:], fwd[b][:, blk],
                                   rm_m[2][:, b:b + 1], rwbb[:],
                                   op0=OP.mult, op1=OP.add)
            rwb.append(rwbb)
        for c in range(2):
            for b in range(B):
                tp_(smT[:, ds(SM_RWT + 8 * c + 4 * b, 4)],
                    rwb[b][:, ds(128 * c, 128)], ident[0:4, 0:4],
                    skip_group_check=True)
        rwn16 = C_([128, 16], "rw16", FPR)
        v.tensor_copy(rwn16[:], smT[:, ds(SM_RWT, 16)])

        # ===== rv =====
        rv_sb = []
        for b in range(B):
            rvb = PS([4, 64], "ctrl")
            for c in range(2):
                mmr(rvb[:], rwn16[:, ds(8 * c + 4 * b, 4)],
                    Msn[c][:, ds(64 * b, 64)], start=(c == 0), stop=(c == 1))
            rvsb = T_([4, 64], f"rv_sb{b}")
            v.tensor_copy(rvsb[:], rvb[:])
            rv_sb.append(rvsb)
        for b in range(B):
            tp_(smT[0:64, ds(SM_RVT + 4 * b, 4)], rv_sb[b][:],
                ident[0:4, 0:4], skip_group_check=True)
        rvn128 = C_([128, 4], "rvT128", FPR)
        for b in range(B):
            quad = smT[0:64, ds(SM_RVT + 4 * b, 4)].rearrange(
                "w (j k) -> w k j", k=2)
            v.tensor_copy(rvn128[0:64, ds(2 * b, 2)], quad[:, 0, :])
            v.tensor_copy(rvn128[64:128, ds(2 * b, 2)], quad[:, 1, :])

        # ===== output =====
        po = PS([2, O], "ctrl")
        for k in range(4):
            mmr(po[:], hT[:, ds(2 * k, 2)], wo_sb[k][:], start=(k == 0),
                stop=False)
        for j in range(2):
            lhs = rvn128[:].rearrange("p (b j) -> p j b", j=2)[:, j, :]
            mmr(po[:], lhs, wm2[j][:], start=False, stop=(j == 1))
        out_sb = T_([2, O], "out_sb")
        sc.activation(out_sb[:], po[:], AF.Copy)
        dma(out=out_d.ap().rearrange("t b o -> (t b) o")[ds(2 * t_step, 2), :],
            in_=out_sb[:])

        if dbg is not None and last:
            dma(out=dbg["h"].ap(), in_=h_sb[:])
            dma(out=dbg["sig"].ap(), in_=sig[:])
            dma(out=dbg["cw"].ap()[0:1], in_=c2cw[0][:])
            dma(out=dbg["cw"].ap()[1:2], in_=c2cw[1][:])
            dma(out=dbg["ret"].ap(), in_=ret4[:])
            dma(out=dbg["u"].ap(), in_=un_col[:])
            dma(out=dbg["a"].ap(), in_=a_col[:])
            dma(out=dbg["ww"].ap(), in_=wwn_row2[:])
            dma(out=dbg["mt"].ap(), in_=MTn[:])
            dma(out=dbg["rn"].ap(), in_=rnn_row2[:])
            dma(out=dbg["rc"].ap()[0:4], in_=shx[0:4, 256:512])
            dma(out=dbg["rc"].ap()[4:8], in_=shx[64:68, 256:512])
            dma(out=dbg["rw"].ap()[0:4], in_=rwb[0][:])
            dma(out=dbg["rw"].ap()[4:8], in_=rwb[1][:])
            dma(out=dbg["rv"].ap()[0:4], in_=rv_sb[0][:])
            dma(out=dbg["rv"].ap()[4:8], in_=rv_sb[1][:])
            dma(out=dbg["L0"].ap(), in_=Ln[0][:])
            dma(out=dbg["LT0"].ap(), in_=LTn[0][:])
            dma(out=dbg["p"].ap(), in_=pn_row2[:])
            dma(out=dbg["lnu"].ap(), in_=lnu_col[:])
            dma(out=dbg["eqc"].ap(), in_=eqc[:])
            dma(out=dbg["A1"].ap(), in_=A1[:])

        MT, Ms, L, LT = MTn, Msn, Ln, LTn
        u_col, ww_col = un_col, wwn_col
        ww_2r, ww_row2, ww_blk = wwn_2r, wwn_row2, wwn_blk
        p_2r, p_blk, p_row2 = pn_2r, pn_blk, pn_row2
        rw16, rvT128, rn_row2 = rwn16, rvn128, rnn_row2


# ---------------------------------------------------------------------------
# Public entry point
# ---------------------------------------------------------------------------
_T, _BFULL, _NCORES = 64, 16, 8
_cache = {}


def _get_nc(T=_T, debug=False, fix=True):
    key = ("nc", T, debug, fix)
    if key not in _cache:
        nc = bass.Bass("TRN2")
        build(nc, T, debug=debug)
        if fix:
            fix_sync_waits(nc)
        _cache[key] = nc
    return _cache[key]


def _get_jit():
    """Build the sharded PJRT executable once and reuse it across calls
    (run_bass_kernel_spmd re-traces jax.jit on every call)."""
    if "jit" in _cache:
        return _cache["jit"]
    import jax
    import numpy as _np
    from jax.sharding import Mesh, PartitionSpec
    try:
        from jax import shard_map
    except ImportError:
        from jax.experimental.shard_map import shard_map
    from concourse import bass2jax as _b2j
    from concourse import mybir as _mybir
    _b2j.install_neuronx_cc_hook()
    nc = _get_nc()
    partition_name = (nc.partition_id_tensor.name
                      if nc.partition_id_tensor else None)
    in_names, out_names, out_avals, zero_shapes = [], [], [], []
    for alloc in nc.m.functions[0].allocations:
        if not isinstance(alloc, _mybir.MemoryLocationSet):
            continue
        name = alloc.memorylocations[0].name
        if alloc.kind == "ExternalInput":
            if name != partition_name:
                in_names.append(name)
        elif alloc.kind == "ExternalOutput":
            shape = tuple(alloc.tensor_shape)
            dtype = _mybir.dt.np(alloc.dtype)
            out_names.append(name)
            out_avals.append(jax.core.ShapedArray(shape, dtype))
            zero_shapes.append((shape, dtype))
    n_params = len(in_names)
    n_outs = len(out_avals)
    all_names = list(in_names) + out_names
    if partition_name is not None:
        all_names.append(partition_name)

    def _body(*args):
        operands = list(args)
        if partition_name is not None:
            operands.append(_b2j.partition_id_tensor())
        outs = _b2j._bass_exec_p.bind(
            *operands, out_avals=tuple(out_avals), in_names=tuple(all_names),
            out_names=tuple(out_names), lowering_input_output_aliases=(),
            sim_require_finite=True, sim_require_nnan=True, nc=nc)
        return tuple(outs)

    devices = jax.devices()[:_NCORES]
    mesh = Mesh(_np.asarray(devices), ("core",))
    in_specs = (PartitionSpec("core"),) * (n_params + n_outs)
    out_specs = (PartitionSpec("core"),) * n_outs
    donate = tuple(range(n_params, n_params + n_outs))
    fn = jax.jit(shard_map(_body, mesh=mesh, in_specs=in_specs,
                           out_specs=out_specs, check_rep=False),
                 donate_argnums=donate, keep_unused=True)
    _cache["jit"] = (fn, in_names, out_names, out_avals, zero_shapes)
    return _cache["jit"]


def kernel(**inputs):
    x = np.ascontiguousarray(np.asarray(inputs["x"], dtype=np.float32))
    shared = {
        k: np.ascontiguousarray(np.asarray(inputs[k], dtype=np.float32))
        for k in ("W_hid", "b_hid", "W_iface", "W_out", "W_memout")
    }
    assert x.shape == (_T, _BFULL, I)
    in_maps = []
    for core in range(_NCORES):
        shard = np.ascontiguousarray(x[:, core * B:(core + 1) * B, :])
        m = {"x": shard}
        m.update(shared)
        in_maps.append(m)
    try:
        fn, in_names, out_names, out_avals, zero_shapes = _get_jit()
        concat_in = [
            np.concatenate([in_maps[c][name] for c in range(_NCORES)], axis=0)
            for name in in_names
        ]
        concat_zeros = [np.zeros((_NCORES * sh[0],) + tuple(sh[1:]), dt)
                        for sh, dt in zero_shapes]
        out_arrs = fn(*concat_in, *concat_zeros)
        oi = out_names.index("out")
        res = np.asarray(out_arrs[oi]).reshape(_NCORES, _T, B, O)
        out = np.empty((_T, _BFULL, O), dtype=np.float32)
        for core in range(_NCORES):
            out[:, core * B:(core + 1) * B, :] = res[core]
        return out
    except Exception:
        nc = _get_nc()
        res = run_bass_kernel_spmd(nc, in_maps,
                                   core_ids=list(range(_NCORES)))
        out = np.empty((_T, _BFULL, O), dtype=np.float32)
        for core in range(_NCORES):
            out[:, core * B:(core + 1) * B, :] = res.results[core]["out"]
        return out


# revision 16
# speedup vs baseline: 3.7470x; 2.7002x over previous
"""Optimized TRN2 Bass kernel for the DNC (NeuCom) recurrence — v2.

Key changes vs v1 baseline:
- Single activation table (natural_log_exp): sigmoid via exp + DVE reciprocal,
  inverse norms via exp(-0.5*ln(q+eps)), oneplus via ln(1+exp(x)).
- float32r matmuls for all large-free matmuls (4x fewer PE cycles/row).
- Block-diagonal fused matmuls: both batches in one instruction for sims,
  M update, L/LT updates, fwd/bwd.
- L^T maintained as a carry with elementwise updates (no per-step transposes).
- Allocation (usage sort) via masked log-sum instead of explicit permutation
  matmuls + scan: a_i = (1-u_i) * exp(sum_{j sorted before i} ln u_j).
  Exact ties (which persist among never-written slots) are handled by an
  equality tie-count term; compares run in ln-space so lt/eq stay consistent.
- Engine rebalance: copies on Activation, some elementwise on Pool.

Hardware constraint honored throughout: every SBUF operand of a non-DMA
instruction must start at partition 0/32/64/96 (PSUM operands are exempt),
so per-batch row data lives in separate base-0 tiles and [2,X] stacked tiles
are built via one-hot selector matmuls accumulated in PSUM.
"""
from contextlib import ExitStack

import numpy as np

import concourse.bass as bass
import concourse.mybir as mybir
import concourse.tile as tile
from concourse.bass import ds, ts
from concourse.bass_utils import run_bass_kernel_spmd

_ctr = [0]


def fix_sync_waits(nc):
    """walrus accepts at most ONE sync-wait per instruction; split extras."""
    for f in nc.m.functions:
        for bb in f.blocks:
            new_insts = []
            for inst in bb.instructions:
                si = inst.sync_info
                waits = list(si.on_wait) if si is not None else []
                if len(waits) > 1:
                    extra, keep = waits[:-1], waits[-1:]
                    while extra:
                        chunk, extra = extra[:1], extra[1:]
                        _ctr[0] += 1
                        nop = mybir.InstNoOp(
                            name=f"WFIX-{_ctr[0]}",
                            engine=inst.engine,
                            sync_info=mybir.SyncInfo(on_wait=chunk, on_update=[]),
                            text_hint="waitfix",
                        )
                        new_insts.append(nop)
                    si.on_wait = keep
                new_insts.append(inst)
            bb.instructions = new_insts
    return nc


FP = mybir.dt.float32
FPR = mybir.dt.float32r
AF = mybir.ActivationFunctionType
OP = mybir.AluOpType
AX = mybir.AxisListType

N, Wd, R, B = 256, 64, 4, 2
H, I, O, IF = 512, 512, 512, 471

C_RK, C_RB, C_WK, C_WB, C_EV, C_WV, C_FG, C_AG, C_WG, C_RM = (
    0, 256, 260, 324, 325, 389, 453, 457, 458, 459)

EQ_ON_POOL = True       # tie-count stt ops on Pool (else DVE)
LT_ADD_ON_POOL = True   # LT "+b2" adds on Pool (else DVE)


def r_(ap):
    return ap.bitcast(FPR)


def build(nc: bass.Bass, T: int, debug: bool = False):
    x_d = nc.dram_tensor("x", [T, B, I], FP, kind="ExternalInput")
    wh_d = nc.dram_tensor("W_hid", [I + R * Wd, H], FP, kind="ExternalInput")
    bh_d = nc.dram_tensor("b_hid", [H], FP, kind="ExternalInput")
    wi_d = nc.dram_tensor("W_iface", [H, IF], FP, kind="ExternalInput")
    wo_d = nc.dram_tensor("W_out", [H, O], FP, kind="ExternalInput")
    wm_d = nc.dram_tensor("W_memout", [R * Wd, O], FP, kind="ExternalInput")
    out_d = nc.dram_tensor("out", [T, B, O], FP, kind="ExternalOutput")
    dbg = None
    if debug:
        dbg = {k: nc.dram_tensor(f"dbg_{k}", s, FP, kind="ExternalOutput")
               for k, s in [("h", [2, H]), ("sig", [2, 134]),
                            ("cw", [2, 256]), ("ret", [128, 4]),
                            ("u", [128, 4]), ("a", [128, 4]),
                            ("ww", [1, 512]), ("mt", [64, 512]),
                            ("rn", [1, 512]), ("rc", [8, 256]),
                            ("rw", [8, 256]), ("rv", [8, 64]),
                            ("L0", [128, 512]), ("LT0", [128, 512]),
                            ("p", [1, 512]), ("lnu", [128, 4]),
                            ("eqc", [128, 4]), ("A1", [128, 4])]}
    with tile.TileContext(nc) as tc:
        with ExitStack() as ctx:
            _build(ctx, tc, nc, T, x_d, wh_d, bh_d, wi_d, wo_d, wm_d, out_d,
                   dbg)
    return nc


def _build(ctx, tc, nc, T, x_d, wh_d, bh_d, wi_d, wo_d, wm_d, out_d, dbg):
    per = ctx.enter_context(tc.tile_pool(name="persist", bufs=1))
    car = ctx.enter_context(tc.tile_pool(name="carry", bufs=2))
    tmp = ctx.enter_context(tc.tile_pool(name="tmp", bufs=2))
    ps = ctx.enter_context(tc.tile_pool(name="ps", bufs=2, space="PSUM"))

    dma = nc.sync.dma_start
    v = nc.vector
    sc = nc.scalar
    gp = nc.gpsimd
    te = nc.tensor
    mm = te.matmul

    def mmr(out, lhsT, rhs, **kw):
        mm(out, r_(lhsT), r_(rhs), **kw)

    def tp_(out, in_, idn, **kw):
        mm(out, in_, idn, is_transpose=True, **kw)

    def T_(shape, tag, dt=FP):
        return tmp.tile(shape, dt, tag=tag, name=tag)

    def C_(shape, tag, dt=FP):
        return car.tile(shape, dt, tag=tag, name=tag)

    def P_(shape, tag, dt=FP):
        return per.tile(shape, dt, tag=tag, name=tag)

    def PS(shape, tag, bufs=None):
        return ps.tile(shape, FP, tag=tag, name=tag, bufs=bufs)

    # ---------------- constants ----------------
    ones_full = P_([128, 512], "ones_full")
    v.memset(ones_full[:], 1.0)
    ident = P_([128, 128], "ident")
    v.tensor_copy(ident[:], ones_full[:, 0:128])
    gp.affine_select(ident[:], ident[:], pattern=[[-1, 128]],
                     compare_op=OP.is_equal, fill=0.0, base=0,
                     channel_multiplier=1)
    # blockmask[b, n] = 1 if n in batch-b block
    blockmask = P_([2, 512], "blockmask")
    v.tensor_copy(blockmask[:], ones_full[0:2, :])
    gp.affine_select(blockmask[:], blockmask[:], pattern=[[1, 512]],
                     compare_op=OP.is_ge, fill=0.0, base=0,
                     channel_multiplier=-256)
    gp.affine_select(blockmask[:], blockmask[:], pattern=[[-1, 512]],
                     compare_op=OP.is_ge, fill=0.0, base=255,
                     channel_multiplier=256)
    jmask = []
    for c in range(2):
        jm = P_([128, 256], f"jmask{c}")
        gp.affine_select(jm[:], ones_full[:, 0:256], pattern=[[-1, 256]],
                         compare_op=OP.is_ge, fill=0.0, base=128 * c - 1,
                         channel_multiplier=1)
        jmask.append(jm)
    negblockmask = P_([2, 512], "negblockmask", FPR)
    v.tensor_scalar_mul(negblockmask[:], blockmask[:], -1.0)
    negones_row = P_([1, 128], "negones_row", FPR)
    v.tensor_scalar_mul(negones_row[:], ones_full[0:1, 0:128], -1.0)
    onesR = P_([128, 512], "onesR", FPR)
    v.tensor_copy(onesR[:], ones_full[:])
    identR = P_([128, 128], "identR", FPR)
    v.tensor_copy(identR[:], ident[:])
    cE12 = P_([128, 1], "cE12")
    v.memset(cE12[:], 1e-12)
    cE37 = P_([128, 1], "cE37")
    v.memset(cE37[:], 1e-37)
    # one-hot selectors
    selrow = []  # [1,2] rows for scatter (lhsT)
    for b in range(B):
        sf = P_([1, 2], f"selrowF{b}")
        v.memset(sf[:], 0.0)
        v.memset(sf[0:1, b:b + 1], 1.0)
        s = P_([1, 2], f"selrow{b}", FPR)
        v.tensor_copy(s[:], sf[:])
        selrow.append(s)
    selcol0 = P_([2, 1], "selcol0")
    v.memset(selcol0[:], 0.0)
    v.memset(selcol0[0:1, 0:1], 1.0)
    selcol1 = P_([2, 1], "selcol1")
    v.tensor_sub(selcol1[:], ones_full[0:2, 0:1], selcol0[:])
    selcol = [selcol0, selcol1]

    # ---------------- weights ----------------
    def load_w(dram, n_tiles, cols, name, row0=0, rows=128):
        out = []
        for k in range(n_tiles):
            t = P_([rows, cols], f"{name}{k}", FPR)
            nc.gpsimd.dma_start(out=t[:],
                                in_=dram.ap()[ds(row0 + k * rows, rows), :])
            out.append(t)
        return out

    wh_sb = load_w(wh_d, 4, H, "wh")
    wrv2 = load_w(wh_d, 2, H, "wrv2", row0=512, rows=128)
    # W_iface padded to even free size (f32r matmul ISA constraint)
    wi_sb = []
    for k in range(4):
        t = P_([128, IF + 1], f"wi{k}", FPR)
        v.tensor_scalar_mul(t[:], ones_full[:, 0:IF + 1], 0.0)
        nc.gpsimd.dma_start(out=t[:, 0:IF],
                            in_=wi_d.ap()[ds(k * 128, 128), :])
        wi_sb.append(t)
    wo_sb = load_w(wo_d, 4, O, "wo")
    wm2 = load_w(wm_d, 2, O, "wm2", rows=128)
    bh_sb = P_([1, H], "bh")
    dma(out=bh_sb[:], in_=bh_d.ap()[None, :])

    # ---------------- Xp precompute ----------------
    TB = T * B
    assert TB <= 128
    xnat = P_([128, I], "xnat")
    dma(out=xnat[:TB, :], in_=x_d.ap().rearrange("t b i -> (t b) i"))
    xt_sb = []
    for k in range(4):
        t = P_([128, 128], f"xt{k}", FPR)
        xtp = PS([128, 512], "ctrl")
        tp_(xtp[:, 0:TB], xnat[:TB, ts(k, 128)], ident[:TB, :TB])
        v.tensor_copy(t[:, :TB], xtp[:, 0:TB])
        xt_sb.append(t)
    xp_sb = P_([128, H], "xp", FPR)
    xp_ps = PS([128, H], "ctrl")
    for k in range(4):
        mmr(xp_ps[:TB, :], xt_sb[k][:, :TB], wh_sb[k][:], start=(k == 0),
            stop=False)
    mm(xp_ps[:TB, :], ones_full[0:1, :TB], bh_sb[:], start=False, stop=True)
    v.tensor_copy(xp_sb[:TB, :], xp_ps[:TB, :])

    # ---------------- carries (initial) ----------------
    MT = C_([64, 512], "MT", FPR)
    v.tensor_scalar_mul(MT[:], ones_full[0:64, :], 1e-6)
    Ms = []
    for c in range(2):
        m = C_([128, 128], f"Ms{c}", FPR)
        v.tensor_scalar_mul(m[:], ones_full[:, 0:128], 1e-6)
        Ms.append(m)
    L = []
    LT = []
    for c in range(2):
        l = C_([128, 512], f"L{c}", FPR)
        v.tensor_scalar_mul(l[:], ones_full[:], 0.0)
        L.append(l)
        lt = C_([128, 512], f"LT{c}", FPR)
        v.tensor_scalar_mul(lt[:], ones_full[:], 0.0)
        LT.append(lt)
    u_col = C_([128, 4], "u_col")
    v.memset(u_col[:], 0.0)
    ww_col = C_([128, 4], "ww_col")
    v.memset(ww_col[:], 0.0)
    ww_2r = C_([2, 256], "ww_2r", FPR)
    v.tensor_scalar_mul(ww_2r[:], ones_full[0:2, 0:256], 0.0)
    ww_row2 = C_([1, 512], "ww_row2", FPR)
    v.tensor_scalar_mul(ww_row2[:], ones_full[0:1, :], 0.0)
    ww_blk = C_([2, 512], "ww_blk", FPR)
    v.tensor_scalar_mul(ww_blk[:], ones_full[0:2, :], 0.0)
    p_2r = C_([2, 256], "p_2r", FPR)
    v.tensor_scalar_mul(p_2r[:], ones_full[0:2, 0:256], 0.0)
    p_blk = C_([2, 512], "p_blk", FPR)
    v.tensor_scalar_mul(p_blk[:], ones_full[0:2, :], 0.0)
    p_row2 = C_([1, 512], "p_row2", FPR)
    v.tensor_scalar_mul(p_row2[:], ones_full[0:1, :], 0.0)
    rw16 = C_([128, 16], "rw16", FPR)
    v.tensor_scalar_mul(rw16[:], ones_full[:, 0:16], 0.0)
    rvT128 = C_([128, 4], "rvT128", FPR)
    v.tensor_scalar_mul(rvT128[:], ones_full[:, 0:4], 0.0)
    rn_row2 = C_([1, 512], "rn_row2")
    v.memset(rn_row2[:], float((Wd * 1e-12 + 1e-12) ** -0.5))

    # smT column map (scratch PSUM bank, tag "sm"):
    SM_HTP, SM_RMG, SM_C12, SM_RST, SM_KT, SM_FGB = 0, 8, 14, 18, 22, 32
    SM_LNU, SM_AT, SM_WC, SM_RWT, SM_RVT = 48, 176, 304, 308, 324

    # ---------------- steps ----------------
    for t_step in range(T):
        last = (t_step == T - 1)
        smT = PS([128, 512], "sm", bufs=1)

        # ===== controller h =====
        h_ps = PS([2, H], "ctrl")
        for j in range(2):
            lhs = rvT128[:].rearrange("p (b j) -> p j b", j=2)[:, j, :]
            mmr(h_ps[:], lhs, wrv2[j][:], start=(j == 0), stop=False)
        mmr(h_ps[:], identR[:TB, ds(2 * t_step, 2)], xp_sb[:TB, :],
            start=False, stop=True)
        h_sb = T_([2, H], "h_sb")
        sc.activation(h_sb[:], h_ps[:], AF.Relu)
        for k in range(4):
            tp_(smT[:, ds(SM_HTP + 2 * k, 2)], h_sb[:, ts(k, 128)],
                ident[0:2, 0:2], skip_group_check=True)
        hT = T_([128, 8], "hT", FPR)
        v.tensor_copy(hT[:], smT[:, ds(SM_HTP, 8)])

        # ===== iface =====
        if_ps = PS([2, IF + 1], "ctrl")
        for k in range(4):
            mmr(if_ps[:], hT[:, ds(2 * k, 2)], wi_sb[k][:], start=(k == 0),
                stop=(k == 3))

        # -- iface activations (full 2-row ops only) --
        esig = T_([2, 134], "esig")
        sc.activation(esig[:], if_ps[:, C_EV:C_RM], AF.Exp, scale=-1.0)
        v.tensor_scalar_add(esig[:], esig[:], 1.0)
        sig = T_([2, 134], "sig")
        v.reciprocal(sig[:], esig[:])
        # sig: [,0:64]=ev  [,128:132]=fg  [,132:133]=ag  [,133:134]=wg

        rme = T_([2, 12], "rme")
        sc.activation(rme[:], if_ps[:, C_RM:C_RM + 12], AF.Exp)
        rmden = T_([2, 4], "rmden")
        v.tensor_reduce(rmden[:], rme[:].rearrange("b (r m) -> b r m", m=3),
                        axis=AX.X, op=OP.add)
        v.reciprocal(rmden[:], rmden[:])
        rmG = T_([2, 12], "rmG")
        v.tensor_tensor(
            out=rmG[:].rearrange("b (m r) -> b m r", r=4),
            in0=rme[:].rearrange("b (r m) -> b m r", m=3),
            in1=rmden[:].rearrange("b (u r) -> b u r", u=1).broadcast_to(
                [2, 3, 4]),
            op=OP.mult)
        for m3 in range(3):
            tp_(smT[0:4, ds(SM_RMG + 2 * m3, 2)], rmG[:, ds(4 * m3, 4)],
                ident[0:2, 0:2], skip_group_check=True)
        rm_m = []
        for m3 in range(3):
            rmt = T_([4, 2], f"rm_m{m3}")
            if m3 == 1:
                sc.activation(rmt[:], smT[0:4, ds(SM_RMG + 2 * m3, 2)],
                              AF.Copy)
            else:
                v.tensor_copy(rmt[:], smT[0:4, ds(SM_RMG + 2 * m3, 2)])
            rm_m.append(rmt)

        # gates -> transposed rows [1,2]
        c1 = T_([2, 1], "c1")
        v.tensor_tensor(c1[:], sig[:, 132:133], sig[:, 133:134], op=OP.mult)
        c2 = T_([2, 1], "c2")
        v.tensor_scalar(c2[:], sig[:, 132:133], -1.0, 1.0, op0=OP.mult,
                        op1=OP.add)
        v.tensor_mul(c2[:], c2[:], sig[:, 133:134])
        tp_(smT[0:1, ds(SM_C12, 2)], c1[:, 0:1], ident[0:2, 0:2],
            skip_group_check=True)
        tp_(smT[0:1, ds(SM_C12 + 2, 2)], c2[:, 0:1], ident[0:2, 0:2],
            skip_group_check=True)
        c1T = T_([1, 2], "c1T")
        c2T = T_([1, 2], "c2T")

        # oneplus(rb|wb) = 1 + ln(1+exp(x)); key norms; rs = (1+sp)/||k||
        bw5 = T_([2, 5], "bw5")
        sc.activation(bw5[:, 0:4], if_ps[:, C_RB:C_RB + 4], AF.Copy)
        sc.activation(bw5[:, 4:5], if_ps[:, C_WB:C_WB + 1], AF.Copy)
        sc.activation(bw5[:], bw5[:], AF.Exp)
        sc.activation(bw5[:], bw5[:], AF.Ln, bias=1.0)
        ifk = T_([2, 325], "ifk")
        v.tensor_copy(ifk[:], if_ps[:, 0:C_EV])
        ksq = T_([2, 325], "ksq")
        v.tensor_tensor(ksq[:], ifk[:], ifk[:], op=OP.mult)
        kn2 = T_([2, 5], "kn2")
        v.tensor_reduce(kn2[:, 0:4],
                        ksq[:, 0:256].rearrange("b (k w) -> b k w", w=64),
                        axis=AX.X, op=OP.add)
        v.tensor_reduce(kn2[:, 4:5], ksq[:, C_WK:C_WK + 64], axis=AX.X,
                        op=OP.add)
        sc.activation(kn2[:], kn2[:], AF.Ln, bias=cE12[0:2, 0:1])
        invkn = T_([2, 5], "invkn")
        sc.activation(invkn[:], kn2[:], AF.Exp, scale=-0.5)
        rs = T_([2, 5], "rs")
        v.scalar_tensor_tensor(rs[:], bw5[:], 1.0, invkn[:], op0=OP.add,
                               op1=OP.mult)
        # transpose read scales [2,4]->[4,2] and write scale [2,1]->[1,2]
        tp_(smT[0:4, ds(SM_RST, 2)], rs[:, 0:4], ident[0:2, 0:2],
            skip_group_check=True)
        tp_(smT[0:1, ds(SM_RST + 2, 2)], rs[:, 4:5], ident[0:2, 0:2],
            skip_group_check=True)
        rsRT = T_([4, 2], "rsRT")
        sc.activation(rsRT[:], smT[0:4, ds(SM_RST, 2)], AF.Copy)
        rsWT = T_([1, 2], "rsWT")
        sc.activation(rsWT[:], smT[0:1, ds(SM_RST + 2, 2)], AF.Copy)
        sc.activation(c1T[:], smT[0:1, ds(SM_C12, 2)], AF.Copy)
        sc.activation(c2T[:], smT[0:1, ds(SM_C12 + 2, 2)], AF.Copy)

        # ===== keys (raw; scales applied to sims) =====
        tp_(smT[:, ds(SM_KT, 2)], ifk[:, 0:128], ident[0:2, 0:2],
            skip_group_check=True)
        tp_(smT[:, ds(SM_KT + 2, 2)], ifk[:, 128:256], ident[0:2, 0:2],
            skip_group_check=True)
        tp_(smT[0:64, ds(SM_KT + 4, 2)], ifk[:, C_WK:C_WK + 64],
            ident[0:2, 0:2], skip_group_check=True)
        keysR = T_([64, 8], "keysR", FPR)
        keysW = T_([64, 8], "keysW", FPR)
        v.tensor_scalar_mul(keysW[:], ones_full[0:64, 0:8], 0.0)
        key_engs = [v, sc, v, sc]
        for kk in range(4):
            src = smT[ds(64 * (kk % 2), 64), ds(SM_KT + 2 * (kk // 2), 2)]
            eng = key_engs[kk]
            if eng is sc:
                sc.activation(
                    keysR[:].rearrange("w (b r) -> w r b", r=4)[:, kk, :],
                    src, AF.Copy)
            else:
                eng.tensor_copy(
                    keysR[:].rearrange("w (b r) -> w r b", r=4)[:, kk, :],
                    src)
        for b in range(B):
            v.tensor_copy(keysW[:, ds(4 * b, 1)],
                          smT[0:64, ds(SM_KT + 4 + b, 1)])

        # ===== cw on old M =====
        simw = []
        for b in range(B):
            swb = PS([2, 512], "ctrl")
            mmr(swb[:], keysW[:, ds(4 * b, 2)], MT[:], start=True, stop=True)
            simw.append(swb)
        shx = PS([128, 512], "shx", bufs=1)  # cw/rc rows 0:4,64:68; rn8 r32
        c2cw = []
        for b in range(B):
            r0 = ds(64 * b, 1)
            cwdb = T_([1, 1], f"cwd{b}")
            v.scalar_tensor_tensor(shx[r0, 0:256],
                                   simw[b][0:1, ds(256 * b, 256)],
                                   rsWT[0:1, b:b + 1],
                                   rn_row2[0:1, ds(256 * b, 256)],
                                   op0=OP.mult, op1=OP.mult)
            sc.activation(shx[r0, 256:512], shx[r0, 0:256], AF.Exp,
                          accum_out=cwdb[:])
            v.reciprocal(cwdb[:], cwdb[:])
            c2cwb = T_([1, 256], f"c2cw{b}")
            v.tensor_scalar(c2cwb[:], shx[r0, 256:512], cwdb[:],
                            c2T[0:1, b:b + 1], op0=OP.mult, op1=OP.mult)
            c2cw.append(c2cwb)

        # ===== usage =====
        fgrow = []
        for b in range(B):
            fgp = PS([1, 4], "ctrl")
            mm(fgp[:], selcol[b][:], sig[:, 128:132], start=True, stop=True)
            fgs = T_([1, 4], f"fgrow{b}")
            v.tensor_copy(fgs[:], fgp[:])
            fgrow.append(fgs)
        for c in range(2):
            for b in range(B):
                mm(smT[:, ds(SM_FGB + 8 * c + 4 * b, 4)],
                   ones_full[0:1, 0:128], fgrow[b][:], start=True, stop=True,
                   skip_group_check=True)
        m1 = T_([128, 16], "m1")
        v.scalar_tensor_tensor(m1[:], smT[:, ds(SM_FGB, 16)], -1.0, rw16[:],
                               op0=OP.mult, op1=OP.mult)
        m2 = T_([128, 16], "m2")
        sc.activation(m2[:], m1[:], AF.Identity, bias=1.0)
        q8 = T_([128, 8], "q8")
        gp.tensor_tensor(q8[:],
                        m2[:].rearrange("p (g r) -> p g r", r=2)[:, :, 0],
                        m2[:].rearrange("p (g r) -> p g r", r=2)[:, :, 1],
                        op=OP.mult)
        ret4 = T_([128, 4], "ret4")
        v.tensor_tensor(ret4[:],
                        q8[:].rearrange("p (h u) -> p h u", u=2)[:, :, 0],
                        q8[:].rearrange("p (h u) -> p h u", u=2)[:, :, 1],
                        op=OP.mult)
        t1 = T_([128, 4], "t1")
        gp.tensor_tensor(t1[:], u_col[:], ww_col[:], op=OP.mult)
        t2 = T_([128, 4], "t2")
        gp.tensor_add(t2[:], u_col[:], ww_col[:])
        v.tensor_sub(t2[:], t2[:], t1[:])
        un_col = C_([128, 4], "u_col")
        v.tensor_tensor(un_col[:], t2[:], ret4[:], op=OP.mult)

        # ===== allocation =====
        lnu_col = T_([128, 4], "lnu_col")
        sc.activation(lnu_col[:], un_col[:], AF.Ln, bias=cE37[:, 0:1])
        ut_ps = PS([1, 512], "ctrl")
        for j in range(4):
            b, c = j // 2, j % 2
            tp_(ut_ps[0:1, ds(128 * j, 128)],
                un_col[:, ds(2 * c + b, 1)], ident[:],
                skip_group_check=True)
        u_row2 = T_([1, 512], "u_row2")
        sc.activation(u_row2[:], ut_ps[:], AF.Copy)
        # per-batch PSUM bank: broadcast u_b; ln(u) goes to SBUF
        lnubc_sb = T_([128, 512], "lnubc_sb")
        ubcln = []
        for b in range(B):
            ub = PS([128, 256], "ubcln", bufs=1)
            mm(ub[:], ones_full[0:1, 0:128],
               u_row2[0:1, ds(256 * b, 256)], start=True, stop=True)
            sc.activation(lnubc_sb[:, ds(256 * b, 256)], ub[:], AF.Ln,
                          bias=cE37[:, 0:1])
            ubcln.append(ub)
        A1 = T_([128, 4], "A1")
        eqc = T_([128, 4], "eqc")
        for c in range(2):
            for b in range(B):
                col = ds(2 * c + b, 1)
                scr = T_([128, 256], f"scr{c}{b}")
                v.scalar_tensor_tensor(scr[:], ubcln[b][:],
                                       un_col[:, col],
                                       lnubc_sb[:, ds(256 * b, 256)],
                                       op0=OP.is_lt, op1=OP.mult,
                                       accum_out=A1[:, col])
                scr2 = T_([128, 256], f"scr2{c}{b}")
                v.scalar_tensor_tensor(scr2[:], ubcln[b][:],
                                       un_col[:, col], jmask[c][:],
                                       op0=OP.is_equal, op1=OP.mult,
                                       accum_out=eqc[:, col])
        A = T_([128, 4], "A")
        v.tensor_tensor(A[:], eqc[:], lnu_col[:], op=OP.mult)
        v.tensor_add(A[:], A[:], A1[:])
        cpx = T_([128, 4], "cpx")
        sc.activation(cpx[:], A[:], AF.Exp)
        onemu = T_([128, 4], "onemu")
        v.tensor_scalar(onemu[:], un_col[:], -1.0, 1.0, op0=OP.mult,
                        op1=OP.add)
        a_col = T_([128, 4], "a_col")
        v.tensor_tensor(a_col[:], onemu[:], cpx[:], op=OP.mult)

        # ===== ww (row space, written into [1,512] row) =====
        at_ps = PS([1, 512], "ctrl")
        for j in range(4):
            b, c = j // 2, j % 2
            tp_(at_ps[0:1, ds(128 * j, 128)], a_col[:, ds(2 * c + b, 1)],
                ident[:], skip_group_check=True)
        wwn_row2 = C_([1, 512], "ww_row2", FPR)
        wws4 = T_([1, 4], "wws4")
        for b in range(B):
            for c in range(2):
                v.scalar_tensor_tensor(
                    wwn_row2[0:1, ds(256 * b + 128 * c, 128)],
                    at_ps[0:1, ds(128 * (2 * b + c), 128)],
                    c1T[0:1, b:b + 1],
                    c2cw[b][0:1, ds(128 * c, 128)],
                    op0=OP.mult, op1=OP.add,
                    accum_out=wws4[0:1, ds(2 * b + c, 1)])
        wws2 = T_([1, 2], "wws2")
        v.tensor_reduce(wws2[:], wws4[:].rearrange("o (b c) -> o b c", c=2),
                        axis=AX.X, op=OP.add)
        # stacked [2,256] / [2,512] forms via selector-scatter in PSUM
        ww2r_ps = PS([2, 256], "ctrl")
        for b in range(B):
            mmr(ww2r_ps[:], selrow[b][:], wwn_row2[0:1, ds(256 * b, 256)],
                start=(b == 0), stop=(b == 1))
        wwn_2r = C_([2, 256], "ww_2r", FPR)
        v.tensor_copy(wwn_2r[:], ww2r_ps[:])
        wwblk_ps = PS([2, 512], "ctrl")
        for b in range(B):
            mmr(wwblk_ps[:, ds(256 * b, 256)], selrow[b][:],
                wwn_row2[0:1, ds(256 * b, 256)], start=True, stop=True,
                skip_group_check=True)
        wwn_blk = C_([2, 512], "ww_blk", FPR)
        sc.activation(wwn_blk[:], wwblk_ps[:], AF.Copy)
        # ww_col via transposes of stacked halves (cols come out as (b))
        for c in range(2):
            tp_(smT[:, ds(SM_WC + 2 * c, 2)].bitcast(FPR),
                wwn_2r[:, ds(128 * c, 128)], identR[0:2, 0:2],
                skip_group_check=True)
        wwn_col = C_([128, 4], "ww_col")
        v.tensor_copy(wwn_col[:], smT[:, ds(SM_WC, 4)])

        # ===== L / LT updates (old p as rhs) =====
        a2 = []
        for c in range(2):
            a2c = PS([128, 512], "a2", bufs=1)
            mmr(a2c[:], wwn_2r[:, ds(128 * c, 128)], negblockmask[:],
                start=True, stop=False)
            mmr(a2c[:], negones_row[:], wwn_row2[:], start=False,
                stop=True)
            a2.append(a2c)
        Ln = []
        LTn = []
        for c in range(2):
            b_c = PS([128, 512], "aux")
            mmr(b_c[:], wwn_2r[:, ds(128 * c, 128)], p_blk[:], start=True,
                stop=True)
            b2_c = PS([128, 512], "aux")
            mmr(b2_c[:], p_2r[:, ds(128 * c, 128)], wwn_blk[:], start=True,
                stop=True)
            lnc = C_([128, 512], f"L{c}", FPR)
            v.scalar_tensor_tensor(lnc[:], a2[c][:], 1.0, L[c][:], op0=OP.add,
                                   op1=OP.mult)
            v.tensor_add(lnc[:], lnc[:], b_c[:])
            gp.affine_select(lnc[:], lnc[:], pattern=[[0, 2], [-1, 256]],
                             compare_op=OP.not_equal, fill=0.0, base=128 * c,
                             channel_multiplier=1)
            Ln.append(lnc)
            ltc = C_([128, 512], f"LT{c}", FPR)
            v.scalar_tensor_tensor(ltc[:], a2[c][:], 1.0, LT[c][:],
                                   op0=OP.add, op1=OP.mult)
            v.tensor_add(ltc[:], ltc[:], b2_c[:])
            gp.affine_select(ltc[:], ltc[:], pattern=[[0, 2], [-1, 256]],
                             compare_op=OP.not_equal, fill=0.0, base=128 * c,
                             channel_multiplier=1)
            LTn.append(ltc)

        # ===== p update (row space + stacked forms) =====
        pn_row2 = C_([1, 512], "p_row2", FPR)
        nws2 = T_([1, 2], "nws2")
        sc.activation(nws2[:], wws2[:], AF.Identity, bias=1.0, scale=-1.0)
        for b in range(B):
            v.scalar_tensor_tensor(pn_row2[0:1, ds(256 * b, 256)],
                                   p_row2[0:1, ds(256 * b, 256)],
                                   nws2[0:1, b:b + 1],
                                   wwn_row2[0:1, ds(256 * b, 256)],
                                   op0=OP.mult, op1=OP.add)
        p2r_ps = PS([2, 256], "ctrl")
        for b in range(B):
            mmr(p2r_ps[:], selrow[b][:], pn_row2[0:1, ds(256 * b, 256)],
                start=(b == 0), stop=(b == 1))
        pn_2r = C_([2, 256], "p_2r", FPR)
        v.tensor_copy(pn_2r[:], p2r_ps[:])
        pblk_ps = PS([2, 512], "ctrl")
        for b in range(B):
            mmr(pblk_ps[:, ds(256 * b, 256)], selrow[b][:],
                pn_row2[0:1, ds(256 * b, 256)], start=True, stop=True,
                skip_group_check=True)
        pn_blk = C_([2, 512], "p_blk", FPR)
        sc.activation(pn_blk[:], pblk_ps[:], AF.Copy)

        # ===== M update =====
        negev_2r = T_([2, 64], "negev_2r", FPR)
        v.tensor_scalar_mul(negev_2r[:], sig[:, 0:64], -1.0)
        wv_2r = T_([2, 64], "wv_2r", FPR)
        v.tensor_copy(wv_2r[:], if_ps[:, C_WV:C_WV + 64])
        q1 = PS([64, 512], "aux")
        mmr(q1[:], negev_2r[:], wwn_blk[:], start=True, stop=True)
        q2 = PS([64, 512], "aux")
        mmr(q2[:], wv_2r[:], wwn_blk[:], start=True, stop=True)
        MTn = C_([64, 512], "MT", FPR)
        v.scalar_tensor_tensor(MTn[:], q1[:], 1.0, MT[:], op0=OP.add,
                               op1=OP.mult)
        v.tensor_add(MTn[:], MTn[:], q2[:])
        # Ms via transposes of MTn
        mst = PS([128, 512], "aux")
        for c in range(2):
            for b in range(B):
                tp_(mst[:, ds(64 * (2 * c + b), 64)].bitcast(FPR),
                    MTn[0:64, ds(256 * b + 128 * c, 128)],
                    identR[0:64, 0:64], skip_group_check=True)
        Msn = []
        for c in range(2):
            msc = C_([128, 128], f"Ms{c}", FPR)
            eng = v if c == 0 else sc
            if eng is sc:
                sc.activation(msc[:], mst[:, ds(128 * c, 128)], AF.Copy)
            else:
                v.tensor_copy(msc[:], mst[:, ds(128 * c, 128)])
            Msn.append(msc)

        # ===== rnorm (new M) =====
        mt2 = T_([64, 512], "mt2", FPR)
        sc.activation(mt2[:], MTn[:], AF.Square)
        nq = PS([2, 512], "aux")
        mmr(nq[:], onesR[0:64, 0:2], mt2[:], start=True, stop=True)
        rnln = T_([1, 512], "rnln")
        sc.activation(rnln[:], nq[0:1, :], AF.Ln, bias=cE12[0:1, 0:1])
        rnn_row2 = C_([1, 512], "rn_row2")
        sc.activation(rnn_row2[:], rnln[:], AF.Exp, scale=-0.5)

        # ===== rc on new M =====
        simr = []
        for b in range(B):
            srb = PS([4, 512], "ctrl")
            mmr(srb[:], keysR[:, ds(4 * b, 4)], MTn[:], start=True, stop=True)
            simr.append(srb)
        for b in range(B):
            mm(shx[ds(32, 4), ds(256 * b, 256)], ones_full[0:1, 0:4],
               rnn_row2[0:1, ds(256 * b, 256)], start=True, stop=True,
               skip_group_check=True)
        rn8_sb = T_([4, 512], "rn8_sb")
        sc.activation(rn8_sb[:], shx[ds(32, 4), :], AF.Copy)
        for b in range(B):
            rr = ds(64 * b, 4)  # rc rows reuse cw rows (consumed)
            v.scalar_tensor_tensor(shx[rr, 0:256],
                                   simr[b][:, ds(256 * b, 256)],
                                   rsRT[:, b:b + 1],
                                   rn8_sb[:, ds(256 * b, 256)],
                                   op0=OP.mult, op1=OP.mult)
            sc.activation(shx[rr, 256:512], shx[rr, 0:256], AF.Exp,
                          accum_out=smT[ds(64 * b, 4), ds(SM_LNU, 1)])
            v.reciprocal(smT[ds(64 * b, 4), ds(SM_LNU, 1)],
                         smT[ds(64 * b, 4), ds(SM_LNU, 1)])

        # ===== fwd / bwd / rw blend =====
        bwd = []
        fwd = []
        for b in range(B):
            bwb = PS([4, 512], "aux")
            for c in range(2):
                mmr(bwb[:], rw16[:, ds(8 * c + 4 * b, 4)], Ln[c][:],
                    start=(c == 0), stop=(c == 1))
            bwd.append(bwb)
        for b in range(B):
            fwb = PS([4, 512], "aux")
            for c in range(2):
                mmr(fwb[:], rw16[:, ds(8 * c + 4 * b, 4)], LTn[c][:],
                    start=(c == 0), stop=(c == 1))
            fwd.append(fwb)
        rwb = []
        for b in range(B):
            blk = ds(256 * b, 256)
            rwbb = T_([4, 256], f"rwb{b}")
            rm1c = T_([4, 1], f"rm1c{b}")
            v.tensor_tensor(rm1c[:], rm_m[1][:, b:b + 1],
                            smT[ds(64 * b, 4), ds(SM_LNU, 1)], op=OP.mult)
            v.tensor_scalar_mul(rwbb[:], bwd[b][:, blk],
                                rm_m[0][:, b:b + 1])
            v.scalar_tensor_tensor(rwbb[:], shx[ds(64 * b, 4), 256:512],
                                   rm1c[:], rwbb[:], op0=OP.mult, op1=OP.add)
            v.scalar_tensor_tensor(rwbb[:], fwd[b][:, blk],
                                   rm_m[2][:, b:b + 1], rwbb[:],
                                   op0=OP.mult, op1=OP.add)
            rwb.append(rwbb)
        for c in range(2):
            for b in range(B):
                tp_(smT[:, ds(SM_RWT + 8 * c + 4 * b, 4)],
                    rwb[b][:, ds(128 * c, 128)], ident[0:4, 0:4],
                    skip_group_check=True)
        rwn16 = C_([128, 16], "rw16", FPR)
        v.tensor_copy(rwn16[:], smT[:, ds(SM_RWT, 16)])

        # ===== rv =====
        rv_sb = []
        for b in range(B):
            rvb = PS([4, 64], "ctrl")
            for c in range(2):
                mmr(rvb[:], rwn16[:, ds(8 * c + 4 * b, 4)],
                    Msn[c][:, ds(64 * b, 64)], start=(c == 0), stop=(c == 1))
            rvsb = T_([4, 64], f"rv_sb{b}")
            v.tensor_copy(rvsb[:], rvb[:])
            rv_sb.append(rvsb)
        for b in range(B):
            tp_(smT[0:64, ds(SM_RVT + 4 * b, 4)], rv_sb[b][:],
                ident[0:4, 0:4], skip_group_check=True)
        rvn128 = C_([128, 4], "rvT128", FPR)
        for b in range(B):
            quad = smT[0:64, ds(SM_RVT + 4 * b, 4)].rearrange(
                "w (j k) -> w k j", k=2)
            v.tensor_copy(rvn128[0:64, ds(2 * b, 2)], quad[:, 0, :])
            v.tensor_copy(rvn128[64:128, ds(2 * b, 2)], quad[:, 1, :])

        # ===== output =====
        po = PS([2, O], "ctrl")
        for k in range(4):
            mmr(po[:], hT[:, ds(2 * k, 2)], wo_sb[k][:], start=(k == 0),
                stop=False)
        for j in range(2):
            lhs = rvn128[:].rearrange("p (b j) -> p j b", j=2)[:, j, :]
            mmr(po[:], lhs, wm2[j][:], start=False, stop=(j == 1))
        out_sb = T_([2, O], "out_sb")
        sc.activation(out_sb[:], po[:], AF.Copy)
        dma(out=out_d.ap().rearrange("t b o -> (t b) o")[ds(2 * t_step, 2), :],
            in_=out_sb[:])

        if dbg is not None and last:
            dma(out=dbg["h"].ap(), in_=h_sb[:])
            dma(out=dbg["sig"].ap(), in_=sig[:])
            dma(out=dbg["cw"].ap()[0:1], in_=c2cw[0][:])
            dma(out=dbg["cw"].ap()[1:2], in_=c2cw[1][:])
            dma(out=dbg["ret"].ap(), in_=ret4[:])
            dma(out=dbg["u"].ap(), in_=un_col[:])
            dma(out=dbg["a"].ap(), in_=a_col[:])
            dma(out=dbg["ww"].ap(), in_=wwn_row2[:])
            dma(out=dbg["mt"].ap(), in_=MTn[:])
            dma(out=dbg["rn"].ap(), in_=rnn_row2[:])
            dma(out=dbg["rc"].ap()[0:4], in_=shx[0:4, 256:512])
            dma(out=dbg["rc"].ap()[4:8], in_=shx[64:68, 256:512])
            dma(out=dbg["rw"].ap()[0:4], in_=rwb[0][:])
            dma(out=dbg["rw"].ap()[4:8], in_=rwb[1][:])
            dma(out=dbg["rv"].ap()[0:4], in_=rv_sb[0][:])
            dma(out=dbg["rv"].ap()[4:8], in_=rv_sb[1][:])
            dma(out=dbg["L0"].ap(), in_=Ln[0][:])
            dma(out=dbg["LT0"].ap(), in_=LTn[0][:])
            dma(out=dbg["p"].ap(), in_=pn_row2[:])
            dma(out=dbg["lnu"].ap(), in_=lnu_col[:])
            dma(out=dbg["eqc"].ap(), in_=eqc[:])
            dma(out=dbg["A1"].ap(), in_=A1[:])

        MT, Ms, L, LT = MTn, Msn, Ln, LTn
        u_col, ww_col = un_col, wwn_col
        ww_2r, ww_row2, ww_blk = wwn_2r, wwn_row2, wwn_blk
        p_2r, p_blk, p_row2 = pn_2r, pn_blk, pn_row2
        rw16, rvT128, rn_row2 = rwn16, rvn128, rnn_row2


# ---------------------------------------------------------------------------
# Public entry point
# ---------------------------------------------------------------------------
_T, _BFULL, _NCORES = 64, 16, 8
_cache = {}


def _get_nc(T=_T, debug=False, fix=True):
    key = ("nc", T, debug, fix)
    if key not in _cache:
        nc = bass.Bass("TRN2")
        build(nc, T, debug=debug)
        if fix:
            fix_sync_waits(nc)
        _cache[key] = nc
    return _cache[key]


def _get_jit():
    """Build the sharded PJRT executable once and reuse it across calls
    (run_bass_kernel_spmd re-traces jax.jit on every call)."""
    if "jit" in _cache:
        return _cache["jit"]
    import jax
    import numpy as _np
    from jax.sharding import Mesh, PartitionSpec
    from jax.experimental.shard_map import shard_map
    from concourse import bass2jax as _b2j
    from concourse import mybir as _mybir
    _b2j.install_neuronx_cc_hook()
    nc = _get_nc()
    partition_name = (nc.partition_id_tensor.name
                      if nc.partition_id_tensor else None)
    in_names, out_names, out_avals, zero_shapes = [], [], [], []
    for alloc in nc.m.functions[0].allocations:
        if not isinstance(alloc, _mybir.MemoryLocationSet):
            continue
        name = alloc.memorylocations[0].name
        if alloc.kind == "ExternalInput":
            if name != partition_name:
                in_names.append(name)
        elif alloc.kind == "ExternalOutput":
            shape = tuple(alloc.tensor_shape)
            dtype = _mybir.dt.np(alloc.dtype)
            out_names.append(name)
            out_avals.append(jax.core.ShapedArray(shape, dtype))
            zero_shapes.append((shape, dtype))
    n_params = len(in_names)
    n_outs = len(out_avals)
    all_names = list(in_names) + out_names
    if partition_name is not None:
        all_names.append(partition_name)

    def _body(*args):
        operands = list(args)
        if partition_name is not None:
            operands.append(_b2j.partition_id_tensor())
        outs = _b2j._bass_exec_p.bind(
            *operands, out_avals=tuple(out_avals), in_names=tuple(all_names),
            out_names=tuple(out_names), lowering_input_output_aliases=(),
            sim_require_finite=True, sim_require_nnan=True, nc=nc)
        return tuple(outs)

    devices = jax.devices()[:_NCORES]
    mesh = Mesh(_np.asarray(devices), ("core",))
    in_specs = (PartitionSpec("core"),) * (n_params + n_outs)
    out_specs = (PartitionSpec("core"),) * n_outs
    donate = tuple(range(n_params, n_params + n_outs))
    try:
        smapped = shard_map(_body, mesh=mesh, in_specs=in_specs,
                            out_specs=out_specs, check_rep=False)
    except TypeError:
        smapped = shard_map(_body, mesh=mesh, in_specs=in_specs,
                            out_specs=out_specs, check_vma=False)
    fn = jax.jit(smapped, donate_argnums=donate, keep_unused=True)
    _cache["jit"] = (fn, in_names, out_names, out_avals, zero_shapes)
    return _cache["jit"]


def kernel(**inputs):
    x = np.ascontiguousarray(np.asarray(inputs["x"], dtype=np.float32))
    shared = {
        k: np.ascontiguousarray(np.asarray(inputs[k], dtype=np.float32))
        for k in ("W_hid", "b_hid", "W_iface", "W_out", "W_memout")
    }
    assert x.shape == (_T, _BFULL, I)
    in_maps = []
    for core in range(_NCORES):
        shard = np.ascontiguousarray(x[:, core * B:(core + 1) * B, :])
        m = {"x": shard}
        m.update(shared)
        in_maps.append(m)
    try:
        fn, in_names, out_names, out_avals, zero_shapes = _get_jit()
        concat_in = [
            np.concatenate([in_maps[c][name] for c in range(_NCORES)], axis=0)
            for name in in_names
        ]
        concat_zeros = [np.zeros((_NCORES * sh[0],) + tuple(sh[1:]), dt)
                        for sh, dt in zero_shapes]
        out_arrs = fn(*concat_in, *concat_zeros)
        oi = out_names.index("out")
        res = np.asarray(out_arrs[oi]).reshape(_NCORES, _T, B, O)
        out = np.empty((_T, _BFULL, O), dtype=np.float32)
        for core in range(_NCORES):
            out[:, core * B:(core + 1) * B, :] = res[core]
        return out
    except Exception:
        nc = _get_nc()
        res = run_bass_kernel_spmd(nc, in_maps,
                                   core_ids=list(range(_NCORES)))
        out = np.empty((_T, _BFULL, O), dtype=np.float32)
        for core in range(_NCORES):
            out[:, core * B:(core + 1) * B, :] = res.results[core]["out"]
        return out


# revision 17
# speedup vs baseline: 8.5819x; 2.2903x over previous
"""Optimized TRN2 Bass kernel for the DNC (NeuCom) recurrence — v2.

Key changes vs v1 baseline:
- Single activation table (natural_log_exp): sigmoid via exp + DVE reciprocal,
  inverse norms via exp(-0.5*ln(q+eps)), oneplus via ln(1+exp(x)).
- float32r matmuls for all large-free matmuls (4x fewer PE cycles/row).
- Block-diagonal fused matmuls: both batches in one instruction for sims,
  M update, L/LT updates, fwd/bwd.
- L^T maintained as a carry with elementwise updates (no per-step transposes).
- Allocation (usage sort) via masked log-sum instead of explicit permutation
  matmuls + scan: a_i = (1-u_i) * exp(sum_{j sorted before i} ln u_j).
  Exact ties (which persist among never-written slots) are handled by an
  equality tie-count term; compares run in ln-space so lt/eq stay consistent.
- Engine rebalance: copies on Activation, some elementwise on Pool.

Hardware constraint honored throughout: every SBUF operand of a non-DMA
instruction must start at partition 0/32/64/96 (PSUM operands are exempt),
so per-batch row data lives in separate base-0 tiles and [2,X] stacked tiles
are built via one-hot selector matmuls accumulated in PSUM.
"""
from contextlib import ExitStack

import numpy as np

import concourse.bass as bass
import concourse.mybir as mybir
import concourse.tile as tile
from concourse.bass import ds, ts
from concourse.bass_utils import run_bass_kernel_spmd

_ctr = [0]


def fix_sync_waits(nc):
    """walrus accepts at most ONE sync-wait per instruction; split extras."""
    for f in nc.m.functions:
        for bb in f.blocks:
            new_insts = []
            for inst in bb.instructions:
                si = inst.sync_info
                waits = list(si.on_wait) if si is not None else []
                if len(waits) > 1:
                    extra, keep = waits[:-1], waits[-1:]
                    while extra:
                        chunk, extra = extra[:1], extra[1:]
                        _ctr[0] += 1
                        nop = mybir.InstNoOp(
                            name=f"WFIX-{_ctr[0]}",
                            engine=inst.engine,
                            sync_info=mybir.SyncInfo(on_wait=chunk, on_update=[]),
                            text_hint="waitfix",
                        )
                        new_insts.append(nop)
                    si.on_wait = keep
                new_insts.append(inst)
            bb.instructions = new_insts
    return nc


FP = mybir.dt.float32
FPR = mybir.dt.float32r
AF = mybir.ActivationFunctionType
OP = mybir.AluOpType
AX = mybir.AxisListType

N, Wd, R, B = 256, 64, 4, 2
H, I, O, IF = 512, 512, 512, 471

C_RK, C_RB, C_WK, C_WB, C_EV, C_WV, C_FG, C_AG, C_WG, C_RM = (
    0, 256, 260, 324, 325, 389, 453, 457, 458, 459)

EQ_ON_POOL = True       # tie-count stt ops on Pool (else DVE)
LT_ADD_ON_POOL = True   # LT "+b2" adds on Pool (else DVE)


def r_(ap):
    return ap.bitcast(FPR)


def build(nc: bass.Bass, T: int, debug: bool = False):
    x_d = nc.dram_tensor("x", [T, B, I], FP, kind="ExternalInput")
    wh_d = nc.dram_tensor("W_hid", [I + R * Wd, H], FP, kind="ExternalInput")
    bh_d = nc.dram_tensor("b_hid", [H], FP, kind="ExternalInput")
    wi_d = nc.dram_tensor("W_iface", [H, IF], FP, kind="ExternalInput")
    wo_d = nc.dram_tensor("W_out", [H, O], FP, kind="ExternalInput")
    wm_d = nc.dram_tensor("W_memout", [R * Wd, O], FP, kind="ExternalInput")
    out_d = nc.dram_tensor("out", [T, B, O], FP, kind="ExternalOutput")
    dbg = None
    if debug:
        dbg = {k: nc.dram_tensor(f"dbg_{k}", s, FP, kind="ExternalOutput")
               for k, s in [("h", [2, H]), ("sig", [2, 134]),
                            ("cw", [2, 256]), ("ret", [128, 4]),
                            ("u", [128, 4]), ("a", [128, 4]),
                            ("ww", [1, 512]), ("mt", [64, 512]),
                            ("rn", [1, 512]), ("rc", [8, 256]),
                            ("rw", [8, 256]), ("rv", [8, 64]),
                            ("L0", [128, 512]), ("LT0", [128, 512]),
                            ("p", [1, 512]), ("lnu", [128, 4]),
                            ("eqc", [128, 4]), ("A1", [128, 4])]}
    with tile.TileContext(nc) as tc:
        with ExitStack() as ctx:
            _build(ctx, tc, nc, T, x_d, wh_d, bh_d, wi_d, wo_d, wm_d, out_d,
                   dbg)
    return nc


def _build(ctx, tc, nc, T, x_d, wh_d, bh_d, wi_d, wo_d, wm_d, out_d, dbg):
    per = ctx.enter_context(tc.tile_pool(name="persist", bufs=1))
    car = ctx.enter_context(tc.tile_pool(name="carry", bufs=2))
    tmp = ctx.enter_context(tc.tile_pool(name="tmp", bufs=2))
    ps = ctx.enter_context(tc.tile_pool(name="ps", bufs=2, space="PSUM"))

    dma = nc.sync.dma_start
    v = nc.vector
    sc = nc.scalar
    gp = nc.gpsimd
    te = nc.tensor
    mm = te.matmul

    def mmr(out, lhsT, rhs, **kw):
        mm(out, r_(lhsT), r_(rhs), **kw)

    def tp_(out, in_, idn, **kw):
        mm(out, in_, idn, is_transpose=True, **kw)

    def T_(shape, tag, dt=FP):
        return tmp.tile(shape, dt, tag=tag, name=tag)

    def C_(shape, tag, dt=FP):
        return car.tile(shape, dt, tag=tag, name=tag)

    def P_(shape, tag, dt=FP):
        return per.tile(shape, dt, tag=tag, name=tag)

    def PS(shape, tag, bufs=None):
        return ps.tile(shape, FP, tag=tag, name=tag, bufs=bufs)

    # ---------------- constants ----------------
    ones_full = P_([128, 512], "ones_full")
    v.memset(ones_full[:], 1.0)
    ident = P_([128, 128], "ident")
    v.tensor_copy(ident[:], ones_full[:, 0:128])
    gp.affine_select(ident[:], ident[:], pattern=[[-1, 128]],
                     compare_op=OP.is_equal, fill=0.0, base=0,
                     channel_multiplier=1)
    # blockmask[b, n] = 1 if n in batch-b block
    blockmask = P_([2, 512], "blockmask")
    v.tensor_copy(blockmask[:], ones_full[0:2, :])
    gp.affine_select(blockmask[:], blockmask[:], pattern=[[1, 512]],
                     compare_op=OP.is_ge, fill=0.0, base=0,
                     channel_multiplier=-256)
    gp.affine_select(blockmask[:], blockmask[:], pattern=[[-1, 512]],
                     compare_op=OP.is_ge, fill=0.0, base=255,
                     channel_multiplier=256)
    jmask = []
    for c in range(2):
        jm = P_([128, 256], f"jmask{c}")
        gp.affine_select(jm[:], ones_full[:, 0:256], pattern=[[-1, 256]],
                         compare_op=OP.is_ge, fill=0.0, base=128 * c - 1,
                         channel_multiplier=1)
        jmask.append(jm)
    negblockmask = P_([2, 512], "negblockmask", FPR)
    v.tensor_scalar_mul(negblockmask[:], blockmask[:], -1.0)
    negones_row = P_([1, 128], "negones_row", FPR)
    v.tensor_scalar_mul(negones_row[:], ones_full[0:1, 0:128], -1.0)
    onesR = P_([128, 512], "onesR", FPR)
    v.tensor_copy(onesR[:], ones_full[:])
    identR = P_([128, 128], "identR", FPR)
    v.tensor_copy(identR[:], ident[:])
    cE12 = P_([128, 1], "cE12")
    v.memset(cE12[:], 1e-12)
    cE37 = P_([128, 1], "cE37")
    v.memset(cE37[:], 1e-37)
    # one-hot selectors
    selrow = []  # [1,2] rows for scatter (lhsT)
    for b in range(B):
        sf = P_([1, 2], f"selrowF{b}")
        v.memset(sf[:], 0.0)
        v.memset(sf[0:1, b:b + 1], 1.0)
        s = P_([1, 2], f"selrow{b}", FPR)
        v.tensor_copy(s[:], sf[:])
        selrow.append(s)
    selcol0 = P_([2, 1], "selcol0")
    v.memset(selcol0[:], 0.0)
    v.memset(selcol0[0:1, 0:1], 1.0)
    selcol1 = P_([2, 1], "selcol1")
    v.tensor_sub(selcol1[:], ones_full[0:2, 0:1], selcol0[:])
    selcol = [selcol0, selcol1]

    # ---------------- weights ----------------
    def load_w(dram, n_tiles, cols, name, row0=0, rows=128):
        out = []
        for k in range(n_tiles):
            t = P_([rows, cols], f"{name}{k}", FPR)
            nc.gpsimd.dma_start(out=t[:],
                                in_=dram.ap()[ds(row0 + k * rows, rows), :])
            out.append(t)
        return out

    wh_sb = load_w(wh_d, 4, H, "wh")
    wrv2 = load_w(wh_d, 2, H, "wrv2", row0=512, rows=128)
    # W_iface padded to even free size (f32r matmul ISA constraint)
    wi_sb = []
    for k in range(4):
        t = P_([128, IF + 1], f"wi{k}", FPR)
        v.tensor_scalar_mul(t[:], ones_full[:, 0:IF + 1], 0.0)
        nc.gpsimd.dma_start(out=t[:, 0:IF],
                            in_=wi_d.ap()[ds(k * 128, 128), :])
        wi_sb.append(t)
    wo_sb = load_w(wo_d, 4, O, "wo")
    wm2 = load_w(wm_d, 2, O, "wm2", rows=128)
    bh_sb = P_([1, H], "bh")
    dma(out=bh_sb[:], in_=bh_d.ap()[None, :])

    # ---------------- Xp precompute ----------------
    TB = T * B
    assert TB <= 128
    xnat = P_([128, I], "xnat")
    dma(out=xnat[:TB, :], in_=x_d.ap().rearrange("t b i -> (t b) i"))
    xt_sb = []
    for k in range(4):
        t = P_([128, 128], f"xt{k}", FPR)
        xtp = PS([128, 512], "ctrl")
        tp_(xtp[:, 0:TB], xnat[:TB, ts(k, 128)], ident[:TB, :TB])
        v.tensor_copy(t[:, :TB], xtp[:, 0:TB])
        xt_sb.append(t)
    xp_sb = P_([128, H], "xp", FPR)
    xp_ps = PS([128, H], "ctrl")
    for k in range(4):
        mmr(xp_ps[:TB, :], xt_sb[k][:, :TB], wh_sb[k][:], start=(k == 0),
            stop=False)
    mm(xp_ps[:TB, :], ones_full[0:1, :TB], bh_sb[:], start=False, stop=True)
    v.tensor_copy(xp_sb[:TB, :], xp_ps[:TB, :])

    # ---------------- carries (initial) ----------------
    MT = C_([64, 512], "MT", FPR)
    v.tensor_scalar_mul(MT[:], ones_full[0:64, :], 1e-6)
    Ms = []
    for c in range(2):
        m = C_([128, 128], f"Ms{c}", FPR)
        v.tensor_scalar_mul(m[:], ones_full[:, 0:128], 1e-6)
        Ms.append(m)
    L = []
    LT = []
    for c in range(2):
        l = C_([128, 512], f"L{c}", FPR)
        v.tensor_scalar_mul(l[:], ones_full[:], 0.0)
        L.append(l)
        lt = C_([128, 512], f"LT{c}", FPR)
        v.tensor_scalar_mul(lt[:], ones_full[:], 0.0)
        LT.append(lt)
    u_col = C_([128, 4], "u_col")
    v.memset(u_col[:], 0.0)
    ww_col = C_([128, 4], "ww_col")
    v.memset(ww_col[:], 0.0)
    ww_2r = C_([2, 256], "ww_2r", FPR)
    v.tensor_scalar_mul(ww_2r[:], ones_full[0:2, 0:256], 0.0)
    ww_row2 = C_([1, 512], "ww_row2", FPR)
    v.tensor_scalar_mul(ww_row2[:], ones_full[0:1, :], 0.0)
    ww_blk = C_([2, 512], "ww_blk", FPR)
    v.tensor_scalar_mul(ww_blk[:], ones_full[0:2, :], 0.0)
    p_2r = C_([2, 256], "p_2r", FPR)
    v.tensor_scalar_mul(p_2r[:], ones_full[0:2, 0:256], 0.0)
    p_blk = C_([2, 512], "p_blk", FPR)
    v.tensor_scalar_mul(p_blk[:], ones_full[0:2, :], 0.0)
    p_row2 = C_([1, 512], "p_row2", FPR)
    v.tensor_scalar_mul(p_row2[:], ones_full[0:1, :], 0.0)
    rw16 = C_([128, 16], "rw16", FPR)
    v.tensor_scalar_mul(rw16[:], ones_full[:, 0:16], 0.0)
    rvT128 = C_([128, 4], "rvT128", FPR)
    v.tensor_scalar_mul(rvT128[:], ones_full[:, 0:4], 0.0)
    rn_row2 = C_([1, 512], "rn_row2")
    v.memset(rn_row2[:], float((Wd * 1e-12 + 1e-12) ** -0.5))

    # smT column map (scratch PSUM bank, tag "sm"):
    SM_HTP, SM_RMG, SM_C12, SM_RST, SM_KT, SM_FGB = 0, 8, 14, 18, 22, 32
    SM_LNU, SM_AT, SM_WC, SM_RWT, SM_RVT = 48, 176, 304, 308, 324

    # ---------------- steps ----------------
    for t_step in range(T):
        last = (t_step == T - 1)
        smT = PS([128, 512], "sm", bufs=1)

        # ===== controller h =====
        h_ps = PS([2, H], "ctrl")
        for j in range(2):
            lhs = rvT128[:].rearrange("p (b j) -> p j b", j=2)[:, j, :]
            mmr(h_ps[:], lhs, wrv2[j][:], start=(j == 0), stop=False)
        mmr(h_ps[:], identR[:TB, ds(2 * t_step, 2)], xp_sb[:TB, :],
            start=False, stop=True)
        h_sb = T_([2, H], "h_sb")
        sc.activation(h_sb[:], h_ps[:], AF.Relu)
        for k in range(4):
            tp_(smT[:, ds(SM_HTP + 2 * k, 2)], h_sb[:, ts(k, 128)],
                ident[0:2, 0:2], skip_group_check=True)
        hT = T_([128, 8], "hT", FPR)
        v.tensor_copy(hT[:], smT[:, ds(SM_HTP, 8)])

        # ===== iface =====
        if_ps = PS([2, IF + 1], "ctrl")
        for k in range(4):
            mmr(if_ps[:], hT[:, ds(2 * k, 2)], wi_sb[k][:], start=(k == 0),
                stop=(k == 3))

        # -- iface activations (full 2-row ops only) --
        esig = T_([2, 134], "esig")
        sc.activation(esig[:], if_ps[:, C_EV:C_RM], AF.Exp, scale=-1.0)
        v.tensor_scalar_add(esig[:], esig[:], 1.0)
        sig = T_([2, 134], "sig")
        v.reciprocal(sig[:], esig[:])
        # sig: [,0:64]=ev  [,128:132]=fg  [,132:133]=ag  [,133:134]=wg

        rme = T_([2, 12], "rme")
        sc.activation(rme[:], if_ps[:, C_RM:C_RM + 12], AF.Exp)
        rmden = T_([2, 4], "rmden")
        v.tensor_reduce(rmden[:], rme[:].rearrange("b (r m) -> b r m", m=3),
                        axis=AX.X, op=OP.add)
        v.reciprocal(rmden[:], rmden[:])
        rmG = T_([2, 12], "rmG")
        v.tensor_tensor(
            out=rmG[:].rearrange("b (m r) -> b m r", r=4),
            in0=rme[:].rearrange("b (r m) -> b m r", m=3),
            in1=rmden[:].rearrange("b (u r) -> b u r", u=1).broadcast_to(
                [2, 3, 4]),
            op=OP.mult)
        for m3 in range(3):
            tp_(smT[0:4, ds(SM_RMG + 2 * m3, 2)], rmG[:, ds(4 * m3, 4)],
                ident[0:2, 0:2], skip_group_check=True)
        rm_m = []
        for m3 in range(3):
            rmt = T_([4, 2], f"rm_m{m3}")
            if m3 == 1:
                sc.activation(rmt[:], smT[0:4, ds(SM_RMG + 2 * m3, 2)],
                              AF.Copy)
            else:
                v.tensor_copy(rmt[:], smT[0:4, ds(SM_RMG + 2 * m3, 2)])
            rm_m.append(rmt)

        # gates -> transposed rows [1,2]
        c1 = T_([2, 1], "c1")
        v.tensor_tensor(c1[:], sig[:, 132:133], sig[:, 133:134], op=OP.mult)
        c2 = T_([2, 1], "c2")
        v.tensor_scalar(c2[:], sig[:, 132:133], -1.0, 1.0, op0=OP.mult,
                        op1=OP.add)
        v.tensor_mul(c2[:], c2[:], sig[:, 133:134])
        tp_(smT[0:1, ds(SM_C12, 2)], c1[:, 0:1], ident[0:2, 0:2],
            skip_group_check=True)
        tp_(smT[0:1, ds(SM_C12 + 2, 2)], c2[:, 0:1], ident[0:2, 0:2],
            skip_group_check=True)
        c1T = T_([1, 2], "c1T")
        c2T = T_([1, 2], "c2T")

        # oneplus(rb|wb) = 1 + ln(1+exp(x)); key norms; rs = (1+sp)/||k||
        bw5 = T_([2, 5], "bw5")
        sc.activation(bw5[:, 0:4], if_ps[:, C_RB:C_RB + 4], AF.Copy)
        sc.activation(bw5[:, 4:5], if_ps[:, C_WB:C_WB + 1], AF.Copy)
        sc.activation(bw5[:], bw5[:], AF.Exp)
        sc.activation(bw5[:], bw5[:], AF.Ln, bias=1.0)
        ifk = T_([2, 325], "ifk")
        v.tensor_copy(ifk[:], if_ps[:, 0:C_EV])
        ksq = T_([2, 325], "ksq")
        v.tensor_tensor(ksq[:], ifk[:], ifk[:], op=OP.mult)
        kn2 = T_([2, 5], "kn2")
        v.tensor_reduce(kn2[:, 0:4],
                        ksq[:, 0:256].rearrange("b (k w) -> b k w", w=64),
                        axis=AX.X, op=OP.add)
        v.tensor_reduce(kn2[:, 4:5], ksq[:, C_WK:C_WK + 64], axis=AX.X,
                        op=OP.add)
        sc.activation(kn2[:], kn2[:], AF.Ln, bias=cE12[0:2, 0:1])
        invkn = T_([2, 5], "invkn")
        sc.activation(invkn[:], kn2[:], AF.Exp, scale=-0.5)
        rs = T_([2, 5], "rs")
        v.scalar_tensor_tensor(rs[:], bw5[:], 1.0, invkn[:], op0=OP.add,
                               op1=OP.mult)
        # transpose read scales [2,4]->[4,2] and write scale [2,1]->[1,2]
        tp_(smT[0:4, ds(SM_RST, 2)], rs[:, 0:4], ident[0:2, 0:2],
            skip_group_check=True)
        tp_(smT[0:1, ds(SM_RST + 2, 2)], rs[:, 4:5], ident[0:2, 0:2],
            skip_group_check=True)
        rsRT = T_([4, 2], "rsRT")
        sc.activation(rsRT[:], smT[0:4, ds(SM_RST, 2)], AF.Copy)
        rsWT = T_([1, 2], "rsWT")
        sc.activation(rsWT[:], smT[0:1, ds(SM_RST + 2, 2)], AF.Copy)
        sc.activation(c1T[:], smT[0:1, ds(SM_C12, 2)], AF.Copy)
        sc.activation(c2T[:], smT[0:1, ds(SM_C12 + 2, 2)], AF.Copy)

        # ===== keys (raw; scales applied to sims) =====
        tp_(smT[:, ds(SM_KT, 2)], ifk[:, 0:128], ident[0:2, 0:2],
            skip_group_check=True)
        tp_(smT[:, ds(SM_KT + 2, 2)], ifk[:, 128:256], ident[0:2, 0:2],
            skip_group_check=True)
        tp_(smT[0:64, ds(SM_KT + 4, 2)], ifk[:, C_WK:C_WK + 64],
            ident[0:2, 0:2], skip_group_check=True)
        keysR = T_([64, 8], "keysR", FPR)
        keysW = T_([64, 8], "keysW", FPR)
        v.tensor_scalar_mul(keysW[:], ones_full[0:64, 0:8], 0.0)
        key_engs = [v, sc, v, sc]
        for kk in range(4):
            src = smT[ds(64 * (kk % 2), 64), ds(SM_KT + 2 * (kk // 2), 2)]
            eng = key_engs[kk]
            if eng is sc:
                sc.activation(
                    keysR[:].rearrange("w (b r) -> w r b", r=4)[:, kk, :],
                    src, AF.Copy)
            else:
                eng.tensor_copy(
                    keysR[:].rearrange("w (b r) -> w r b", r=4)[:, kk, :],
                    src)
        for b in range(B):
            v.tensor_copy(keysW[:, ds(4 * b, 1)],
                          smT[0:64, ds(SM_KT + 4 + b, 1)])

        # ===== cw on old M =====
        simw = []
        for b in range(B):
            swb = PS([2, 512], "ctrl")
            mmr(swb[:], keysW[:, ds(4 * b, 2)], MT[:], start=True, stop=True)
            simw.append(swb)
        shx = PS([128, 512], "shx", bufs=1)  # cw/rc rows 0:4,64:68; rn8 r32
        c2cw = []
        for b in range(B):
            r0 = ds(64 * b, 1)
            cwdb = T_([1, 1], f"cwd{b}")
            v.scalar_tensor_tensor(shx[r0, 0:256],
                                   simw[b][0:1, ds(256 * b, 256)],
                                   rsWT[0:1, b:b + 1],
                                   rn_row2[0:1, ds(256 * b, 256)],
                                   op0=OP.mult, op1=OP.mult)
            sc.activation(shx[r0, 256:512], shx[r0, 0:256], AF.Exp,
                          accum_out=cwdb[:])
            v.reciprocal(cwdb[:], cwdb[:])
            c2cwb = T_([1, 256], f"c2cw{b}")
            v.tensor_scalar(c2cwb[:], shx[r0, 256:512], cwdb[:],
                            c2T[0:1, b:b + 1], op0=OP.mult, op1=OP.mult)
            c2cw.append(c2cwb)

        # ===== usage =====
        fgrow = []
        for b in range(B):
            fgp = PS([1, 4], "ctrl")
            mm(fgp[:], selcol[b][:], sig[:, 128:132], start=True, stop=True)
            fgs = T_([1, 4], f"fgrow{b}")
            v.tensor_copy(fgs[:], fgp[:])
            fgrow.append(fgs)
        for c in range(2):
            for b in range(B):
                mm(smT[:, ds(SM_FGB + 8 * c + 4 * b, 4)],
                   ones_full[0:1, 0:128], fgrow[b][:], start=True, stop=True,
                   skip_group_check=True)
        m1 = T_([128, 16], "m1")
        v.scalar_tensor_tensor(m1[:], smT[:, ds(SM_FGB, 16)], -1.0, rw16[:],
                               op0=OP.mult, op1=OP.mult)
        m2 = T_([128, 16], "m2")
        sc.activation(m2[:], m1[:], AF.Identity, bias=1.0)
        q8 = T_([128, 8], "q8")
        gp.tensor_tensor(q8[:],
                        m2[:].rearrange("p (g r) -> p g r", r=2)[:, :, 0],
                        m2[:].rearrange("p (g r) -> p g r", r=2)[:, :, 1],
                        op=OP.mult)
        ret4 = T_([128, 4], "ret4")
        v.tensor_tensor(ret4[:],
                        q8[:].rearrange("p (h u) -> p h u", u=2)[:, :, 0],
                        q8[:].rearrange("p (h u) -> p h u", u=2)[:, :, 1],
                        op=OP.mult)
        t1 = T_([128, 4], "t1")
        gp.tensor_tensor(t1[:], u_col[:], ww_col[:], op=OP.mult)
        t2 = T_([128, 4], "t2")
        gp.tensor_add(t2[:], u_col[:], ww_col[:])
        v.tensor_sub(t2[:], t2[:], t1[:])
        un_col = C_([128, 4], "u_col")
        v.tensor_tensor(un_col[:], t2[:], ret4[:], op=OP.mult)

        # ===== allocation =====
        lnu_col = T_([128, 4], "lnu_col")
        sc.activation(lnu_col[:], un_col[:], AF.Ln, bias=cE37[:, 0:1])
        ut_ps = PS([1, 512], "ctrl")
        for j in range(4):
            b, c = j // 2, j % 2
            tp_(ut_ps[0:1, ds(128 * j, 128)],
                un_col[:, ds(2 * c + b, 1)], ident[:],
                skip_group_check=True)
        u_row2 = T_([1, 512], "u_row2")
        sc.activation(u_row2[:], ut_ps[:], AF.Copy)
        # per-batch PSUM bank: broadcast u_b; ln(u) goes to SBUF
        lnubc_sb = T_([128, 512], "lnubc_sb")
        ubcln = []
        for b in range(B):
            ub = PS([128, 256], "ubcln", bufs=1)
            mm(ub[:], ones_full[0:1, 0:128],
               u_row2[0:1, ds(256 * b, 256)], start=True, stop=True)
            sc.activation(lnubc_sb[:, ds(256 * b, 256)], ub[:], AF.Ln,
                          bias=cE37[:, 0:1])
            ubcln.append(ub)
        A1 = T_([128, 4], "A1")
        eqc = T_([128, 4], "eqc")
        for c in range(2):
            for b in range(B):
                col = ds(2 * c + b, 1)
                scr = T_([128, 256], f"scr{c}{b}")
                v.scalar_tensor_tensor(scr[:], ubcln[b][:],
                                       un_col[:, col],
                                       lnubc_sb[:, ds(256 * b, 256)],
                                       op0=OP.is_lt, op1=OP.mult,
                                       accum_out=A1[:, col])
                scr2 = T_([128, 256], f"scr2{c}{b}")
                v.scalar_tensor_tensor(scr2[:], ubcln[b][:],
                                       un_col[:, col], jmask[c][:],
                                       op0=OP.is_equal, op1=OP.mult,
                                       accum_out=eqc[:, col])
        A = T_([128, 4], "A")
        v.tensor_tensor(A[:], eqc[:], lnu_col[:], op=OP.mult)
        v.tensor_add(A[:], A[:], A1[:])
        cpx = T_([128, 4], "cpx")
        sc.activation(cpx[:], A[:], AF.Exp)
        onemu = T_([128, 4], "onemu")
        v.tensor_scalar(onemu[:], un_col[:], -1.0, 1.0, op0=OP.mult,
                        op1=OP.add)
        a_col = T_([128, 4], "a_col")
        v.tensor_tensor(a_col[:], onemu[:], cpx[:], op=OP.mult)

        # ===== ww (row space, written into [1,512] row) =====
        at_ps = PS([1, 512], "ctrl")
        for j in range(4):
            b, c = j // 2, j % 2
            tp_(at_ps[0:1, ds(128 * j, 128)], a_col[:, ds(2 * c + b, 1)],
                ident[:], skip_group_check=True)
        wwn_row2 = C_([1, 512], "ww_row2", FPR)
        wws4 = T_([1, 4], "wws4")
        for b in range(B):
            for c in range(2):
                v.scalar_tensor_tensor(
                    wwn_row2[0:1, ds(256 * b + 128 * c, 128)],
                    at_ps[0:1, ds(128 * (2 * b + c), 128)],
                    c1T[0:1, b:b + 1],
                    c2cw[b][0:1, ds(128 * c, 128)],
                    op0=OP.mult, op1=OP.add,
                    accum_out=wws4[0:1, ds(2 * b + c, 1)])
        wws2 = T_([1, 2], "wws2")
        v.tensor_reduce(wws2[:], wws4[:].rearrange("o (b c) -> o b c", c=2),
                        axis=AX.X, op=OP.add)
        # stacked [2,256] / [2,512] forms via selector-scatter in PSUM
        ww2r_ps = PS([2, 256], "ctrl")
        for b in range(B):
            mmr(ww2r_ps[:], selrow[b][:], wwn_row2[0:1, ds(256 * b, 256)],
                start=(b == 0), stop=(b == 1))
        wwn_2r = C_([2, 256], "ww_2r", FPR)
        v.tensor_copy(wwn_2r[:], ww2r_ps[:])
        wwblk_ps = PS([2, 512], "ctrl")
        for b in range(B):
            mmr(wwblk_ps[:, ds(256 * b, 256)], selrow[b][:],
                wwn_row2[0:1, ds(256 * b, 256)], start=True, stop=True,
                skip_group_check=True)
        wwn_blk = C_([2, 512], "ww_blk", FPR)
        sc.activation(wwn_blk[:], wwblk_ps[:], AF.Copy)
        # ww_col via transposes of stacked halves (cols come out as (b))
        for c in range(2):
            tp_(smT[:, ds(SM_WC + 2 * c, 2)].bitcast(FPR),
                wwn_2r[:, ds(128 * c, 128)], identR[0:2, 0:2],
                skip_group_check=True)
        wwn_col = C_([128, 4], "ww_col")
        v.tensor_copy(wwn_col[:], smT[:, ds(SM_WC, 4)])

        # ===== L / LT updates (old p as rhs) =====
        a2 = []
        for c in range(2):
            a2c = PS([128, 512], "a2", bufs=1)
            mmr(a2c[:], wwn_2r[:, ds(128 * c, 128)], negblockmask[:],
                start=True, stop=False)
            mmr(a2c[:], negones_row[:], wwn_row2[:], start=False,
                stop=True)
            a2.append(a2c)
        Ln = []
        LTn = []
        for c in range(2):
            b_c = PS([128, 512], "aux")
            mmr(b_c[:], wwn_2r[:, ds(128 * c, 128)], p_blk[:], start=True,
                stop=True)
            b2_c = PS([128, 512], "aux")
            mmr(b2_c[:], p_2r[:, ds(128 * c, 128)], wwn_blk[:], start=True,
                stop=True)
            lnc = C_([128, 512], f"L{c}", FPR)
            v.scalar_tensor_tensor(lnc[:], a2[c][:], 1.0, L[c][:], op0=OP.add,
                                   op1=OP.mult)
            v.tensor_add(lnc[:], lnc[:], b_c[:])
            gp.affine_select(lnc[:], lnc[:], pattern=[[0, 2], [-1, 256]],
                             compare_op=OP.not_equal, fill=0.0, base=128 * c,
                             channel_multiplier=1)
            Ln.append(lnc)
            ltc = C_([128, 512], f"LT{c}", FPR)
            v.scalar_tensor_tensor(ltc[:], a2[c][:], 1.0, LT[c][:],
                                   op0=OP.add, op1=OP.mult)
            v.tensor_add(ltc[:], ltc[:], b2_c[:])
            gp.affine_select(ltc[:], ltc[:], pattern=[[0, 2], [-1, 256]],
                             compare_op=OP.not_equal, fill=0.0, base=128 * c,
                             channel_multiplier=1)
            LTn.append(ltc)

        # ===== p update (row space + stacked forms) =====
        pn_row2 = C_([1, 512], "p_row2", FPR)
        nws2 = T_([1, 2], "nws2")
        sc.activation(nws2[:], wws2[:], AF.Identity, bias=1.0, scale=-1.0)
        for b in range(B):
            v.scalar_tensor_tensor(pn_row2[0:1, ds(256 * b, 256)],
                                   p_row2[0:1, ds(256 * b, 256)],
                                   nws2[0:1, b:b + 1],
                                   wwn_row2[0:1, ds(256 * b, 256)],
                                   op0=OP.mult, op1=OP.add)
        p2r_ps = PS([2, 256], "ctrl")
        for b in range(B):
            mmr(p2r_ps[:], selrow[b][:], pn_row2[0:1, ds(256 * b, 256)],
                start=(b == 0), stop=(b == 1))
        pn_2r = C_([2, 256], "p_2r", FPR)
        v.tensor_copy(pn_2r[:], p2r_ps[:])
        pblk_ps = PS([2, 512], "ctrl")
        for b in range(B):
            mmr(pblk_ps[:, ds(256 * b, 256)], selrow[b][:],
                pn_row2[0:1, ds(256 * b, 256)], start=True, stop=True,
                skip_group_check=True)
        pn_blk = C_([2, 512], "p_blk", FPR)
        sc.activation(pn_blk[:], pblk_ps[:], AF.Copy)

        # ===== M update =====
        negev_2r = T_([2, 64], "negev_2r", FPR)
        v.tensor_scalar_mul(negev_2r[:], sig[:, 0:64], -1.0)
        wv_2r = T_([2, 64], "wv_2r", FPR)
        v.tensor_copy(wv_2r[:], if_ps[:, C_WV:C_WV + 64])
        q1 = PS([64, 512], "aux")
        mmr(q1[:], negev_2r[:], wwn_blk[:], start=True, stop=True)
        q2 = PS([64, 512], "aux")
        mmr(q2[:], wv_2r[:], wwn_blk[:], start=True, stop=True)
        MTn = C_([64, 512], "MT", FPR)
        v.scalar_tensor_tensor(MTn[:], q1[:], 1.0, MT[:], op0=OP.add,
                               op1=OP.mult)
        v.tensor_add(MTn[:], MTn[:], q2[:])
        # Ms via transposes of MTn
        mst = PS([128, 512], "aux")
        for c in range(2):
            for b in range(B):
                tp_(mst[:, ds(64 * (2 * c + b), 64)].bitcast(FPR),
                    MTn[0:64, ds(256 * b + 128 * c, 128)],
                    identR[0:64, 0:64], skip_group_check=True)
        Msn = []
        for c in range(2):
            msc = C_([128, 128], f"Ms{c}", FPR)
            eng = v if c == 0 else sc
            if eng is sc:
                sc.activation(msc[:], mst[:, ds(128 * c, 128)], AF.Copy)
            else:
                v.tensor_copy(msc[:], mst[:, ds(128 * c, 128)])
            Msn.append(msc)

        # ===== rnorm (new M) =====
        mt2 = T_([64, 512], "mt2", FPR)
        sc.activation(mt2[:], MTn[:], AF.Square)
        nq = PS([2, 512], "aux")
        mmr(nq[:], onesR[0:64, 0:2], mt2[:], start=True, stop=True)
        rnln = T_([1, 512], "rnln")
        sc.activation(rnln[:], nq[0:1, :], AF.Ln, bias=cE12[0:1, 0:1])
        rnn_row2 = C_([1, 512], "rn_row2")
        sc.activation(rnn_row2[:], rnln[:], AF.Exp, scale=-0.5)

        # ===== rc on new M =====
        simr = []
        for b in range(B):
            srb = PS([4, 512], "ctrl")
            mmr(srb[:], keysR[:, ds(4 * b, 4)], MTn[:], start=True, stop=True)
            simr.append(srb)
        for b in range(B):
            mm(shx[ds(32, 4), ds(256 * b, 256)], ones_full[0:1, 0:4],
               rnn_row2[0:1, ds(256 * b, 256)], start=True, stop=True,
               skip_group_check=True)
        rn8_sb = T_([4, 512], "rn8_sb")
        sc.activation(rn8_sb[:], shx[ds(32, 4), :], AF.Copy)
        for b in range(B):
            rr = ds(64 * b, 4)  # rc rows reuse cw rows (consumed)
            v.scalar_tensor_tensor(shx[rr, 0:256],
                                   simr[b][:, ds(256 * b, 256)],
                                   rsRT[:, b:b + 1],
                                   rn8_sb[:, ds(256 * b, 256)],
                                   op0=OP.mult, op1=OP.mult)
            sc.activation(shx[rr, 256:512], shx[rr, 0:256], AF.Exp,
                          accum_out=smT[ds(64 * b, 4), ds(SM_LNU, 1)])
            v.reciprocal(smT[ds(64 * b, 4), ds(SM_LNU, 1)],
                         smT[ds(64 * b, 4), ds(SM_LNU, 1)])

        # ===== fwd / bwd / rw blend =====
        bwd = []
        fwd = []
        for b in range(B):
            bwb = PS([4, 512], "aux")
            for c in range(2):
                mmr(bwb[:], rw16[:, ds(8 * c + 4 * b, 4)], Ln[c][:],
                    start=(c == 0), stop=(c == 1))
            bwd.append(bwb)
        for b in range(B):
            fwb = PS([4, 512], "aux")
            for c in range(2):
                mmr(fwb[:], rw16[:, ds(8 * c + 4 * b, 4)], LTn[c][:],
                    start=(c == 0), stop=(c == 1))
            fwd.append(fwb)
        rwb = []
        for b in range(B):
            blk = ds(256 * b, 256)
            rwbb = T_([4, 256], f"rwb{b}")
            rm1c = T_([4, 1], f"rm1c{b}")
            v.tensor_tensor(rm1c[:], rm_m[1][:, b:b + 1],
                            smT[ds(64 * b, 4), ds(SM_LNU, 1)], op=OP.mult)
            v.tensor_scalar_mul(rwbb[:], bwd[b][:, blk],
                                rm_m[0][:, b:b + 1])
            v.scalar_tensor_tensor(rwbb[:], shx[ds(64 * b, 4), 256:512],
                                   rm1c[:], rwbb[:], op0=OP.mult, op1=OP.add)
            v.scalar_tensor_tensor(rwbb[:], fwd[b][:, blk],
                                   rm_m[2][:, b:b + 1], rwbb[:],
                                   op0=OP.mult, op1=OP.add)
            rwb.append(rwbb)
        for c in range(2):
            for b in range(B):
                tp_(smT[:, ds(SM_RWT + 8 * c + 4 * b, 4)],
                    rwb[b][:, ds(128 * c, 128)], ident[0:4, 0:4],
                    skip_group_check=True)
        rwn16 = C_([128, 16], "rw16", FPR)
        v.tensor_copy(rwn16[:], smT[:, ds(SM_RWT, 16)])

        # ===== rv =====
        rv_sb = []
        for b in range(B):
            rvb = PS([4, 64], "ctrl")
            for c in range(2):
                mmr(rvb[:], rwn16[:, ds(8 * c + 4 * b, 4)],
                    Msn[c][:, ds(64 * b, 64)], start=(c == 0), stop=(c == 1))
            rvsb = T_([4, 64], f"rv_sb{b}")
            v.tensor_copy(rvsb[:], rvb[:])
            rv_sb.append(rvsb)
        for b in range(B):
            tp_(smT[0:64, ds(SM_RVT + 4 * b, 4)], rv_sb[b][:],
                ident[0:4, 0:4], skip_group_check=True)
        rvn128 = C_([128, 4], "rvT128", FPR)
        for b in range(B):
            quad = smT[0:64, ds(SM_RVT + 4 * b, 4)].rearrange(
                "w (j k) -> w k j", k=2)
            v.tensor_copy(rvn128[0:64, ds(2 * b, 2)], quad[:, 0, :])
            v.tensor_copy(rvn128[64:128, ds(2 * b, 2)], quad[:, 1, :])

        # ===== output =====
        po = PS([2, O], "ctrl")
        for k in range(4):
            mmr(po[:], hT[:, ds(2 * k, 2)], wo_sb[k][:], start=(k == 0),
                stop=False)
        for j in range(2):
            lhs = rvn128[:].rearrange("p (b j) -> p j b", j=2)[:, j, :]
            mmr(po[:], lhs, wm2[j][:], start=False, stop=(j == 1))
        out_sb = T_([2, O], "out_sb")
        sc.activation(out_sb[:], po[:], AF.Copy)
        dma(out=out_d.ap().rearrange("t b o -> (t b) o")[ds(2 * t_step, 2), :],
            in_=out_sb[:])

        if dbg is not None and last:
            dma(out=dbg["h"].ap(), in_=h_sb[:])
            dma(out=dbg["sig"].ap(), in_=sig[:])
            dma(out=dbg["cw"].ap()[0:1], in_=c2cw[0][:])
            dma(out=dbg["cw"].ap()[1:2], in_=c2cw[1][:])
            dma(out=dbg["ret"].ap(), in_=ret4[:])
            dma(out=dbg["u"].ap(), in_=un_col[:])
            dma(out=dbg["a"].ap(), in_=a_col[:])
            dma(out=dbg["ww"].ap(), in_=wwn_row2[:])
            dma(out=dbg["mt"].ap(), in_=MTn[:])
            dma(out=dbg["rn"].ap(), in_=rnn_row2[:])
            dma(out=dbg["rc"].ap()[0:4], in_=shx[0:4, 256:512])
            dma(out=dbg["rc"].ap()[4:8], in_=shx[64:68, 256:512])
            dma(out=dbg["rw"].ap()[0:4], in_=rwb[0][:])
            dma(out=dbg["rw"].ap()[4:8], in_=rwb[1][:])
            dma(out=dbg["rv"].ap()[0:4], in_=rv_sb[0][:])
            dma(out=dbg["rv"].ap()[4:8], in_=rv_sb[1][:])
            dma(out=dbg["L0"].ap(), in_=Ln[0][:])
            dma(out=dbg["LT0"].ap(), in_=LTn[0][:])
            dma(out=dbg["p"].ap(), in_=pn_row2[:])
            dma(out=dbg["lnu"].ap(), in_=lnu_col[:])
            dma(out=dbg["eqc"].ap(), in_=eqc[:])
            dma(out=dbg["A1"].ap(), in_=A1[:])

        MT, Ms, L, LT = MTn, Msn, Ln, LTn
        u_col, ww_col = un_col, wwn_col
        ww_2r, ww_row2, ww_blk = wwn_2r, wwn_row2, wwn_blk
        p_2r, p_blk, p_row2 = pn_2r, pn_blk, pn_row2
        rw16, rvT128, rn_row2 = rwn16, rvn128, rnn_row2


# ---------------------------------------------------------------------------
# Public entry point
# ---------------------------------------------------------------------------
_T, _BFULL, _NCORES = 64, 16, 8
_cache = {}


def _get_nc(T=_T, debug=False, fix=True):
    key = ("nc", T, debug, fix)
    if key not in _cache:
        nc = bass.Bass("TRN2")
        build(nc, T, debug=debug)
        if fix:
            fix_sync_waits(nc)
        _cache[key] = nc
    return _cache[key]


def _get_jit():
    """Build the sharded PJRT executable once and reuse it across calls
    (run_bass_kernel_spmd re-traces jax.jit on every call)."""
    if "jit" in _cache:
        return _cache["jit"]
    import jax
    import numpy as _np
    from jax.sharding import Mesh, PartitionSpec
    from jax.experimental.shard_map import shard_map
    from concourse import bass2jax as _b2j
    from concourse import mybir as _mybir
    _b2j.install_neuronx_cc_hook()
    nc = _get_nc()
    partition_name = (nc.partition_id_tensor.name
                      if nc.partition_id_tensor else None)
    in_names, out_names, out_avals, zero_shapes = [], [], [], []
    for alloc in nc.m.functions[0].allocations:
        if not isinstance(alloc, _mybir.MemoryLocationSet):
            continue
        name = alloc.memorylocations[0].name
        if alloc.kind == "ExternalInput":
            if name != partition_name:
                in_names.append(name)
        elif alloc.kind == "ExternalOutput":
            shape = tuple(alloc.tensor_shape)
            dtype = _mybir.dt.np(alloc.dtype)
            out_names.append(name)
            out_avals.append(jax.core.ShapedArray(shape, dtype))
            zero_shapes.append((shape, dtype))
    n_params = len(in_names)
    n_outs = len(out_avals)
    all_names = list(in_names) + out_names
    if partition_name is not None:
        all_names.append(partition_name)

    def _body(*args):
        operands = list(args)
        if partition_name is not None:
            operands.append(_b2j.partition_id_tensor())
        outs = _b2j._bass_exec_p.bind(
            *operands, out_avals=tuple(out_avals), in_names=tuple(all_names),
            out_names=tuple(out_names), lowering_input_output_aliases=(),
            sim_require_finite=True, sim_require_nnan=True, nc=nc)
        return tuple(outs)

    devices = jax.devices()[:_NCORES]
    mesh = Mesh(_np.asarray(devices), ("core",))
    in_specs = (PartitionSpec("core"),) * (n_params + n_outs)
    out_specs = (PartitionSpec("core"),) * n_outs
    donate = tuple(range(n_params, n_params + n_outs))
    try:
        smapped = shard_map(_body, mesh=mesh, in_specs=in_specs,
                            out_specs=out_specs, check_rep=False)
    except TypeError:
        smapped = shard_map(_body, mesh=mesh, in_specs=in_specs,
                            out_specs=out_specs, check_vma=False)
    fn = jax.jit(smapped, donate_argnums=donate, keep_unused=True)
    _cache["jit"] = (fn, in_names, out_names, out_avals, zero_shapes)
    return _cache["jit"]


def kernel(**inputs):
    x = np.ascontiguousarray(np.asarray(inputs["x"], dtype=np.float32))
    shared = {
        k: np.ascontiguousarray(np.asarray(inputs[k], dtype=np.float32))
        for k in ("W_hid", "b_hid", "W_iface", "W_out", "W_memout")
    }
    assert x.shape == (_T, _BFULL, I)
    in_maps = []
    for core in range(_NCORES):
        shard = np.ascontiguousarray(x[:, core * B:(core + 1) * B, :])
        m = {"x": shard}
        m.update(shared)
        in_maps.append(m)
    try:
        fn, in_names, out_names, out_avals, zero_shapes = _get_jit()
        import jax
        # Weights are replicated per core and rarely change between calls:
        # keep their device placement cached, revalidated by exact equality.
        wcache = _cache.setdefault("wdev", {})
        concat_in = []
        for name in in_names:
            host = np.concatenate(
                [in_maps[c][name] for c in range(_NCORES)], axis=0)
            if name == "x":
                concat_in.append(host)
                continue
            ent = wcache.get(name)
            if ent is not None and ent[0].shape == host.shape and \
                    np.array_equal(ent[0], host):
                concat_in.append(ent[1])
            else:
                dev = jax.device_put(host)
                dev.block_until_ready()
                wcache[name] = (host.copy(), dev)
                concat_in.append(dev)
        concat_zeros = [np.zeros((_NCORES * sh[0],) + tuple(sh[1:]), dt)
                        for sh, dt in zero_shapes]
        out_arrs = fn(*concat_in, *concat_zeros)
        oi = out_names.index("out")
        res = np.asarray(out_arrs[oi]).reshape(_NCORES, _T, B, O)
        out = np.empty((_T, _BFULL, O), dtype=np.float32)
        for core in range(_NCORES):
            out[:, core * B:(core + 1) * B, :] = res[core]
        return out
    except Exception:
        nc = _get_nc()
        res = run_bass_kernel_spmd(nc, in_maps,
                                   core_ids=list(range(_NCORES)))
        out = np.empty((_T, _BFULL, O), dtype=np.float32)
        for core in range(_NCORES):
            out[:, core * B:(core + 1) * B, :] = res.results[core]["out"]
        return out


# revision 18
# speedup vs baseline: 10.5030x; 1.2238x over previous
"""Optimized TRN2 Bass kernel for the DNC (NeuCom) recurrence — v2.

Key changes vs v1 baseline:
- Single activation table (natural_log_exp): sigmoid via exp + DVE reciprocal,
  inverse norms via exp(-0.5*ln(q+eps)), oneplus via ln(1+exp(x)).
- float32r matmuls for all large-free matmuls (4x fewer PE cycles/row).
- Block-diagonal fused matmuls: both batches in one instruction for sims,
  M update, L/LT updates, fwd/bwd.
- L^T maintained as a carry with elementwise updates (no per-step transposes).
- Allocation (usage sort) via masked log-sum instead of explicit permutation
  matmuls + scan: a_i = (1-u_i) * exp(sum_{j sorted before i} ln u_j).
  Exact ties (which persist among never-written slots) are handled by an
  equality tie-count term; compares run in ln-space so lt/eq stay consistent.
- Engine rebalance: copies on Activation, some elementwise on Pool.

Hardware constraint honored throughout: every SBUF operand of a non-DMA
instruction must start at partition 0/32/64/96 (PSUM operands are exempt),
so per-batch row data lives in separate base-0 tiles and [2,X] stacked tiles
are built via one-hot selector matmuls accumulated in PSUM.
"""
from contextlib import ExitStack

import numpy as np

import concourse.bass as bass
import concourse.mybir as mybir
import concourse.tile as tile
from concourse.bass import ds, ts
from concourse.bass_utils import run_bass_kernel_spmd

_ctr = [0]


def fix_sync_waits(nc):
    """walrus accepts at most ONE sync-wait per instruction; split extras."""
    for f in nc.m.functions:
        for bb in f.blocks:
            new_insts = []
            for inst in bb.instructions:
                si = inst.sync_info
                waits = list(si.on_wait) if si is not None else []
                if len(waits) > 1:
                    extra, keep = waits[:-1], waits[-1:]
                    while extra:
                        chunk, extra = extra[:1], extra[1:]
                        _ctr[0] += 1
                        nop = mybir.InstNoOp(
                            name=f"WFIX-{_ctr[0]}",
                            engine=inst.engine,
                            sync_info=mybir.SyncInfo(on_wait=chunk, on_update=[]),
                            text_hint="waitfix",
                        )
                        new_insts.append(nop)
                    si.on_wait = keep
                new_insts.append(inst)
            bb.instructions = new_insts
    return nc


FP = mybir.dt.float32
FPR = mybir.dt.float32r
AF = mybir.ActivationFunctionType
OP = mybir.AluOpType
AX = mybir.AxisListType

N, Wd, R, B = 256, 64, 4, 2
H, I, O, IF = 512, 512, 512, 471

C_RK, C_RB, C_WK, C_WB, C_EV, C_WV, C_FG, C_AG, C_WG, C_RM = (
    0, 256, 260, 324, 325, 389, 453, 457, 458, 459)

EQ_ON_POOL = True       # tie-count stt ops on Pool (else DVE)
LT_ADD_ON_POOL = True   # LT "+b2" adds on Pool (else DVE)


def r_(ap):
    return ap.bitcast(FPR)


def build(nc: bass.Bass, T: int, debug: bool = False):
    x_d = nc.dram_tensor("x", [T, B, I], FP, kind="ExternalInput")
    wh_d = nc.dram_tensor("W_hid", [I + R * Wd, H], FP, kind="ExternalInput")
    bh_d = nc.dram_tensor("b_hid", [H], FP, kind="ExternalInput")
    wi_d = nc.dram_tensor("W_iface", [H, IF], FP, kind="ExternalInput")
    wo_d = nc.dram_tensor("W_out", [H, O], FP, kind="ExternalInput")
    wm_d = nc.dram_tensor("W_memout", [R * Wd, O], FP, kind="ExternalInput")
    out_d = nc.dram_tensor("out", [T, B, O], FP, kind="ExternalOutput")
    dbg = None
    if debug:
        dbg = {k: nc.dram_tensor(f"dbg_{k}", s, FP, kind="ExternalOutput")
               for k, s in [("h", [2, H]), ("sig", [2, 134]),
                            ("cw", [2, 256]), ("ret", [128, 4]),
                            ("u", [128, 4]), ("a", [128, 4]),
                            ("ww", [1, 512]), ("mt", [64, 512]),
                            ("rn", [1, 512]), ("rc", [8, 256]),
                            ("rw", [8, 256]), ("rv", [8, 64]),
                            ("L0", [128, 512]), ("LT0", [128, 512]),
                            ("p", [1, 512]), ("lnu", [128, 4]),
                            ("eqc", [128, 4]), ("A1", [128, 4])]}
    with tile.TileContext(nc) as tc:
        with ExitStack() as ctx:
            _build(ctx, tc, nc, T, x_d, wh_d, bh_d, wi_d, wo_d, wm_d, out_d,
                   dbg)
    return nc


def _build(ctx, tc, nc, T, x_d, wh_d, bh_d, wi_d, wo_d, wm_d, out_d, dbg):
    per = ctx.enter_context(tc.tile_pool(name="persist", bufs=1))
    car = ctx.enter_context(tc.tile_pool(name="carry", bufs=2))
    tmp = ctx.enter_context(tc.tile_pool(name="tmp", bufs=2))
    ps = ctx.enter_context(tc.tile_pool(name="ps", bufs=2, space="PSUM"))

    dma = nc.sync.dma_start
    v = nc.vector
    sc = nc.scalar
    gp = nc.gpsimd
    te = nc.tensor
    mm = te.matmul

    def mmr(out, lhsT, rhs, **kw):
        mm(out, r_(lhsT), r_(rhs), **kw)

    def tp_(out, in_, idn, **kw):
        mm(out, in_, idn, is_transpose=True, **kw)

    def T_(shape, tag, dt=FP):
        return tmp.tile(shape, dt, tag=tag, name=tag)

    def C_(shape, tag, dt=FP):
        return car.tile(shape, dt, tag=tag, name=tag)

    def P_(shape, tag, dt=FP):
        return per.tile(shape, dt, tag=tag, name=tag)

    def PS(shape, tag, bufs=None):
        return ps.tile(shape, FP, tag=tag, name=tag, bufs=bufs)

    # ---------------- constants ----------------
    ones_full = P_([128, 512], "ones_full")
    v.memset(ones_full[:], 1.0)
    ident = P_([128, 128], "ident")
    v.tensor_copy(ident[:], ones_full[:, 0:128])
    gp.affine_select(ident[:], ident[:], pattern=[[-1, 128]],
                     compare_op=OP.is_equal, fill=0.0, base=0,
                     channel_multiplier=1)
    # blockmask[b, n] = 1 if n in batch-b block
    blockmask = P_([2, 512], "blockmask")
    v.tensor_copy(blockmask[:], ones_full[0:2, :])
    gp.affine_select(blockmask[:], blockmask[:], pattern=[[1, 512]],
                     compare_op=OP.is_ge, fill=0.0, base=0,
                     channel_multiplier=-256)
    gp.affine_select(blockmask[:], blockmask[:], pattern=[[-1, 512]],
                     compare_op=OP.is_ge, fill=0.0, base=255,
                     channel_multiplier=256)
    jmask = []
    for c in range(2):
        jm = P_([128, 256], f"jmask{c}")
        gp.affine_select(jm[:], ones_full[:, 0:256], pattern=[[-1, 256]],
                         compare_op=OP.is_ge, fill=0.0, base=128 * c - 1,
                         channel_multiplier=1)
        jmask.append(jm)
    negblockmask = P_([2, 512], "negblockmask", FPR)
    v.tensor_scalar_mul(negblockmask[:], blockmask[:], -1.0)
    negones_row = P_([1, 128], "negones_row", FPR)
    v.tensor_scalar_mul(negones_row[:], ones_full[0:1, 0:128], -1.0)
    onesR = P_([128, 512], "onesR", FPR)
    v.tensor_copy(onesR[:], ones_full[:])
    identR = P_([128, 128], "identR", FPR)
    v.tensor_copy(identR[:], ident[:])
    cE12 = P_([128, 1], "cE12")
    v.memset(cE12[:], 1e-12)
    cE37 = P_([128, 1], "cE37")
    v.memset(cE37[:], 1e-37)
    # one-hot selectors
    selrow = []  # [1,2] rows for scatter (lhsT)
    for b in range(B):
        sf = P_([1, 2], f"selrowF{b}")
        v.memset(sf[:], 0.0)
        v.memset(sf[0:1, b:b + 1], 1.0)
        s = P_([1, 2], f"selrow{b}", FPR)
        v.tensor_copy(s[:], sf[:])
        selrow.append(s)
    selcol0 = P_([2, 1], "selcol0")
    v.memset(selcol0[:], 0.0)
    v.memset(selcol0[0:1, 0:1], 1.0)
    selcol1 = P_([2, 1], "selcol1")
    v.tensor_sub(selcol1[:], ones_full[0:2, 0:1], selcol0[:])
    selcol = [selcol0, selcol1]

    # ---------------- weights ----------------
    def load_w(dram, n_tiles, cols, name, row0=0, rows=128):
        out = []
        for k in range(n_tiles):
            t = P_([rows, cols], f"{name}{k}", FPR)
            nc.gpsimd.dma_start(out=t[:],
                                in_=dram.ap()[ds(row0 + k * rows, rows), :])
            out.append(t)
        return out

    wh_sb = load_w(wh_d, 4, H, "wh")
    wrv2 = load_w(wh_d, 2, H, "wrv2", row0=512, rows=128)
    # W_iface padded to even free size (f32r matmul ISA constraint)
    wi_sb = []
    for k in range(4):
        t = P_([128, IF + 1], f"wi{k}", FPR)
        v.tensor_scalar_mul(t[:], ones_full[:, 0:IF + 1], 0.0)
        nc.gpsimd.dma_start(out=t[:, 0:IF],
                            in_=wi_d.ap()[ds(k * 128, 128), :])
        wi_sb.append(t)
    wo_sb = load_w(wo_d, 4, O, "wo")
    wm2 = load_w(wm_d, 2, O, "wm2", rows=128)
    bh_sb = P_([1, H], "bh")
    dma(out=bh_sb[:], in_=bh_d.ap()[None, :])

    # ---------------- Xp precompute ----------------
    TB = T * B
    assert TB <= 128
    xnat = P_([128, I], "xnat")
    dma(out=xnat[:TB, :], in_=x_d.ap().rearrange("t b i -> (t b) i"))
    xt_sb = []
    for k in range(4):
        t = P_([128, 128], f"xt{k}", FPR)
        xtp = PS([128, 512], "ctrl")
        tp_(xtp[:, 0:TB], xnat[:TB, ts(k, 128)], ident[:TB, :TB])
        v.tensor_copy(t[:, :TB], xtp[:, 0:TB])
        xt_sb.append(t)
    xp_sb = P_([128, H], "xp", FPR)
    xp_ps = PS([128, H], "ctrl")
    for k in range(4):
        mmr(xp_ps[:TB, :], xt_sb[k][:, :TB], wh_sb[k][:], start=(k == 0),
            stop=False)
    mm(xp_ps[:TB, :], ones_full[0:1, :TB], bh_sb[:], start=False, stop=True)
    v.tensor_copy(xp_sb[:TB, :], xp_ps[:TB, :])

    # ---------------- carries (initial) ----------------
    MT = C_([64, 512], "MT", FPR)
    v.tensor_scalar_mul(MT[:], ones_full[0:64, :], 1e-6)
    Ms = []
    for c in range(2):
        m = C_([128, 128], f"Ms{c}", FPR)
        v.tensor_scalar_mul(m[:], ones_full[:, 0:128], 1e-6)
        Ms.append(m)
    L = []
    LT = []
    for c in range(2):
        l = C_([128, 512], f"L{c}", FPR)
        v.tensor_scalar_mul(l[:], ones_full[:], 0.0)
        L.append(l)
        lt = C_([128, 512], f"LT{c}", FPR)
        v.tensor_scalar_mul(lt[:], ones_full[:], 0.0)
        LT.append(lt)
    u_col = C_([128, 4], "u_col")
    v.memset(u_col[:], 0.0)
    ww_col = C_([128, 4], "ww_col")
    v.memset(ww_col[:], 0.0)
    ww_2r = C_([2, 256], "ww_2r", FPR)
    v.tensor_scalar_mul(ww_2r[:], ones_full[0:2, 0:256], 0.0)
    ww_row2 = C_([1, 512], "ww_row2", FPR)
    v.tensor_scalar_mul(ww_row2[:], ones_full[0:1, :], 0.0)
    ww_blk = C_([2, 512], "ww_blk", FPR)
    v.tensor_scalar_mul(ww_blk[:], ones_full[0:2, :], 0.0)
    p_2r = C_([2, 256], "p_2r", FPR)
    v.tensor_scalar_mul(p_2r[:], ones_full[0:2, 0:256], 0.0)
    p_blk = C_([2, 512], "p_blk", FPR)
    v.tensor_scalar_mul(p_blk[:], ones_full[0:2, :], 0.0)
    p_row2 = C_([1, 512], "p_row2", FPR)
    v.tensor_scalar_mul(p_row2[:], ones_full[0:1, :], 0.0)
    rw16 = C_([128, 16], "rw16", FPR)
    v.tensor_scalar_mul(rw16[:], ones_full[:, 0:16], 0.0)
    rvT128 = C_([128, 4], "rvT128", FPR)
    v.tensor_scalar_mul(rvT128[:], ones_full[:, 0:4], 0.0)
    rn_row2 = C_([1, 512], "rn_row2")
    v.memset(rn_row2[:], float((Wd * 1e-12 + 1e-12) ** -0.5))

    # smT column map (scratch PSUM bank, tag "sm"):
    SM_HTP, SM_RMG, SM_C12, SM_RST, SM_KT, SM_FGB = 0, 8, 14, 18, 22, 32
    SM_LNU, SM_AT, SM_WC, SM_RWT, SM_RVT = 48, 176, 304, 308, 324

    # ---------------- steps ----------------
    for t_step in range(T):
        last = (t_step == T - 1)
        smT = PS([128, 512], "sm", bufs=1)

        # ===== controller h =====
        h_ps = PS([2, H], "ctrl")
        for j in range(2):
            lhs = rvT128[:].rearrange("p (b j) -> p j b", j=2)[:, j, :]
            mmr(h_ps[:], lhs, wrv2[j][:], start=(j == 0), stop=False)
        mmr(h_ps[:], identR[:TB, ds(2 * t_step, 2)], xp_sb[:TB, :],
            start=False, stop=True)
        h_sb = T_([2, H], "h_sb")
        sc.activation(h_sb[:], h_ps[:], AF.Relu)
        for k in range(4):
            tp_(smT[:, ds(SM_HTP + 2 * k, 2)], h_sb[:, ts(k, 128)],
                ident[0:2, 0:2], skip_group_check=True)
        hT = T_([128, 8], "hT", FPR)
        v.tensor_copy(hT[:], smT[:, ds(SM_HTP, 8)])

        # ===== iface =====
        if_ps = PS([2, IF + 1], "ctrl")
        for k in range(4):
            mmr(if_ps[:], hT[:, ds(2 * k, 2)], wi_sb[k][:], start=(k == 0),
                stop=(k == 3))

        # -- iface activations (full 2-row ops only) --
        esig = T_([2, 134], "esig")
        sc.activation(esig[:], if_ps[:, C_EV:C_RM], AF.Exp, scale=-1.0)
        v.tensor_scalar_add(esig[:], esig[:], 1.0)
        sig = T_([2, 134], "sig")
        v.reciprocal(sig[:], esig[:])
        # sig: [,0:64]=ev  [,128:132]=fg  [,132:133]=ag  [,133:134]=wg

        rme = T_([2, 12], "rme")
        sc.activation(rme[:], if_ps[:, C_RM:C_RM + 12], AF.Exp)
        rmden = T_([2, 4], "rmden")
        v.tensor_reduce(rmden[:], rme[:].rearrange("b (r m) -> b r m", m=3),
                        axis=AX.X, op=OP.add)
        v.reciprocal(rmden[:], rmden[:])
        rmG = T_([2, 12], "rmG")
        v.tensor_tensor(
            out=rmG[:].rearrange("b (m r) -> b m r", r=4),
            in0=rme[:].rearrange("b (r m) -> b m r", m=3),
            in1=rmden[:].rearrange("b (u r) -> b u r", u=1).broadcast_to(
                [2, 3, 4]),
            op=OP.mult)
        for m3 in range(3):
            tp_(smT[0:4, ds(SM_RMG + 2 * m3, 2)], rmG[:, ds(4 * m3, 4)],
                ident[0:2, 0:2], skip_group_check=True)
        rm_m = []
        for m3 in range(3):
            rmt = T_([4, 2], f"rm_m{m3}")
            if m3 == 1:
                sc.activation(rmt[:], smT[0:4, ds(SM_RMG + 2 * m3, 2)],
                              AF.Copy)
            else:
                v.tensor_copy(rmt[:], smT[0:4, ds(SM_RMG + 2 * m3, 2)])
            rm_m.append(rmt)

        # gates -> transposed rows [1,2]
        c1 = T_([2, 1], "c1")
        v.tensor_tensor(c1[:], sig[:, 132:133], sig[:, 133:134], op=OP.mult)
        c2 = T_([2, 1], "c2")
        v.tensor_scalar(c2[:], sig[:, 132:133], -1.0, 1.0, op0=OP.mult,
                        op1=OP.add)
        v.tensor_mul(c2[:], c2[:], sig[:, 133:134])
        tp_(smT[0:1, ds(SM_C12, 2)], c1[:, 0:1], ident[0:2, 0:2],
            skip_group_check=True)
        tp_(smT[0:1, ds(SM_C12 + 2, 2)], c2[:, 0:1], ident[0:2, 0:2],
            skip_group_check=True)
        c1T = T_([1, 2], "c1T")
        c2T = T_([1, 2], "c2T")

        # oneplus(rb|wb) = 1 + ln(1+exp(x)); key norms; rs = (1+sp)/||k||
        bw5 = T_([2, 5], "bw5")
        sc.activation(bw5[:, 0:4], if_ps[:, C_RB:C_RB + 4], AF.Copy)
        sc.activation(bw5[:, 4:5], if_ps[:, C_WB:C_WB + 1], AF.Copy)
        sc.activation(bw5[:], bw5[:], AF.Exp)
        sc.activation(bw5[:], bw5[:], AF.Ln, bias=1.0)
        ifk = T_([2, 325], "ifk")
        v.tensor_copy(ifk[:], if_ps[:, 0:C_EV])
        ksq = T_([2, 325], "ksq")
        v.tensor_tensor(ksq[:], ifk[:], ifk[:], op=OP.mult)
        kn2 = T_([2, 5], "kn2")
        v.tensor_reduce(kn2[:, 0:4],
                        ksq[:, 0:256].rearrange("b (k w) -> b k w", w=64),
                        axis=AX.X, op=OP.add)
        v.tensor_reduce(kn2[:, 4:5], ksq[:, C_WK:C_WK + 64], axis=AX.X,
                        op=OP.add)
        sc.activation(kn2[:], kn2[:], AF.Ln, bias=cE12[0:2, 0:1])
        invkn = T_([2, 5], "invkn")
        sc.activation(invkn[:], kn2[:], AF.Exp, scale=-0.5)
        rs = T_([2, 5], "rs")
        v.scalar_tensor_tensor(rs[:], bw5[:], 1.0, invkn[:], op0=OP.add,
                               op1=OP.mult)
        # transpose read scales [2,4]->[4,2] and write scale [2,1]->[1,2]
        tp_(smT[0:4, ds(SM_RST, 2)], rs[:, 0:4], ident[0:2, 0:2],
            skip_group_check=True)
        tp_(smT[0:1, ds(SM_RST + 2, 2)], rs[:, 4:5], ident[0:2, 0:2],
            skip_group_check=True)
        rsRT = T_([4, 2], "rsRT")
        sc.activation(rsRT[:], smT[0:4, ds(SM_RST, 2)], AF.Copy)
        rsWT = T_([1, 2], "rsWT")
        sc.activation(rsWT[:], smT[0:1, ds(SM_RST + 2, 2)], AF.Copy)
        sc.activation(c1T[:], smT[0:1, ds(SM_C12, 2)], AF.Copy)
        sc.activation(c2T[:], smT[0:1, ds(SM_C12 + 2, 2)], AF.Copy)

        # ===== keys (raw; scales applied to sims) =====
        tp_(smT[:, ds(SM_KT, 2)], ifk[:, 0:128], ident[0:2, 0:2],
            skip_group_check=True)
        tp_(smT[:, ds(SM_KT + 2, 2)], ifk[:, 128:256], ident[0:2, 0:2],
            skip_group_check=True)
        tp_(smT[0:64, ds(SM_KT + 4, 2)], ifk[:, C_WK:C_WK + 64],
            ident[0:2, 0:2], skip_group_check=True)
        keysR = T_([64, 8], "keysR", FPR)
        keysW = T_([64, 8], "keysW", FPR)
        v.tensor_scalar_mul(keysW[:], ones_full[0:64, 0:8], 0.0)
        key_engs = [v, sc, v, sc]
        for kk in range(4):
            src = smT[ds(64 * (kk % 2), 64), ds(SM_KT + 2 * (kk // 2), 2)]
            eng = key_engs[kk]
            if eng is sc:
                sc.activation(
                    keysR[:].rearrange("w (b r) -> w r b", r=4)[:, kk, :],
                    src, AF.Copy)
            else:
                eng.tensor_copy(
                    keysR[:].rearrange("w (b r) -> w r b", r=4)[:, kk, :],
                    src)
        for b in range(B):
            v.tensor_copy(keysW[:, ds(4 * b, 1)],
                          smT[0:64, ds(SM_KT + 4 + b, 1)])

        # ===== cw on old M =====
        simw = []
        for b in range(B):
            swb = PS([2, 512], "ctrl")
            mmr(swb[:], keysW[:, ds(4 * b, 2)], MT[:], start=True, stop=True)
            simw.append(swb)
        shx = PS([128, 512], "shx", bufs=1)  # cw/rc rows 0:4,64:68; rn8 r32
        c2cw = []
        for b in range(B):
            r0 = ds(64 * b, 1)
            cwdb = T_([1, 1], f"cwd{b}")
            v.scalar_tensor_tensor(shx[r0, 0:256],
                                   simw[b][0:1, ds(256 * b, 256)],
                                   rsWT[0:1, b:b + 1],
                                   rn_row2[0:1, ds(256 * b, 256)],
                                   op0=OP.mult, op1=OP.mult)
            sc.activation(shx[r0, 256:512], shx[r0, 0:256], AF.Exp,
                          accum_out=cwdb[:])
            v.reciprocal(cwdb[:], cwdb[:])
            c2cwb = T_([1, 256], f"c2cw{b}")
            v.tensor_scalar(c2cwb[:], shx[r0, 256:512], cwdb[:],
                            c2T[0:1, b:b + 1], op0=OP.mult, op1=OP.mult)
            c2cw.append(c2cwb)

        # ===== usage =====
        fgrow = []
        for b in range(B):
            fgp = PS([1, 4], "ctrl")
            mm(fgp[:], selcol[b][:], sig[:, 128:132], start=True, stop=True)
            fgs = T_([1, 4], f"fgrow{b}")
            v.tensor_copy(fgs[:], fgp[:])
            fgrow.append(fgs)
        for c in range(2):
            for b in range(B):
                mm(smT[:, ds(SM_FGB + 8 * c + 4 * b, 4)],
                   ones_full[0:1, 0:128], fgrow[b][:], start=True, stop=True,
                   skip_group_check=True)
        m1 = T_([128, 16], "m1")
        v.scalar_tensor_tensor(m1[:], smT[:, ds(SM_FGB, 16)], -1.0, rw16[:],
                               op0=OP.mult, op1=OP.mult)
        m2 = T_([128, 16], "m2")
        sc.activation(m2[:], m1[:], AF.Identity, bias=1.0)
        q8 = T_([128, 8], "q8")
        gp.tensor_tensor(q8[:],
                        m2[:].rearrange("p (g r) -> p g r", r=2)[:, :, 0],
                        m2[:].rearrange("p (g r) -> p g r", r=2)[:, :, 1],
                        op=OP.mult)
        ret4 = T_([128, 4], "ret4")
        v.tensor_tensor(ret4[:],
                        q8[:].rearrange("p (h u) -> p h u", u=2)[:, :, 0],
                        q8[:].rearrange("p (h u) -> p h u", u=2)[:, :, 1],
                        op=OP.mult)
        t1 = T_([128, 4], "t1")
        gp.tensor_tensor(t1[:], u_col[:], ww_col[:], op=OP.mult)
        t2 = T_([128, 4], "t2")
        gp.tensor_add(t2[:], u_col[:], ww_col[:])
        v.tensor_sub(t2[:], t2[:], t1[:])
        un_col = C_([128, 4], "u_col")
        v.tensor_tensor(un_col[:], t2[:], ret4[:], op=OP.mult)

        # ===== allocation =====
        lnu_col = T_([128, 4], "lnu_col")
        sc.activation(lnu_col[:], un_col[:], AF.Ln, bias=cE37[:, 0:1])
        ut_ps = PS([1, 512], "ctrl")
        for j in range(4):
            b, c = j // 2, j % 2
            tp_(ut_ps[0:1, ds(128 * j, 128)],
                un_col[:, ds(2 * c + b, 1)], ident[:],
                skip_group_check=True)
        u_row2 = T_([1, 512], "u_row2")
        sc.activation(u_row2[:], ut_ps[:], AF.Copy)
        # per-batch PSUM bank: broadcast u_b; ln(u) goes to SBUF
        lnubc_sb = T_([128, 512], "lnubc_sb")
        ubcln = []
        for b in range(B):
            ub = PS([128, 256], "ubcln", bufs=1)
            mm(ub[:], ones_full[0:1, 0:128],
               u_row2[0:1, ds(256 * b, 256)], start=True, stop=True)
            sc.activation(lnubc_sb[:, ds(256 * b, 256)], ub[:], AF.Ln,
                          bias=cE37[:, 0:1])
            ubcln.append(ub)
        A1 = T_([128, 4], "A1")
        eqc = T_([128, 4], "eqc")
        for c in range(2):
            for b in range(B):
                col = ds(2 * c + b, 1)
                scr = T_([128, 256], f"scr{c}{b}")
                v.scalar_tensor_tensor(scr[:], ubcln[b][:],
                                       un_col[:, col],
                                       lnubc_sb[:, ds(256 * b, 256)],
                                       op0=OP.is_lt, op1=OP.mult,
                                       accum_out=A1[:, col])
                scr2 = T_([128, 256], f"scr2{c}{b}")
                v.scalar_tensor_tensor(scr2[:], ubcln[b][:],
                                       un_col[:, col], jmask[c][:],
                                       op0=OP.is_equal, op1=OP.mult,
                                       accum_out=eqc[:, col])
        A = T_([128, 4], "A")
        v.tensor_tensor(A[:], eqc[:], lnu_col[:], op=OP.mult)
        v.tensor_add(A[:], A[:], A1[:])
        cpx = T_([128, 4], "cpx")
        sc.activation(cpx[:], A[:], AF.Exp)
        onemu = T_([128, 4], "onemu")
        v.tensor_scalar(onemu[:], un_col[:], -1.0, 1.0, op0=OP.mult,
                        op1=OP.add)
        a_col = T_([128, 4], "a_col")
        v.tensor_tensor(a_col[:], onemu[:], cpx[:], op=OP.mult)

        # ===== ww (row space, written into [1,512] row) =====
        at_ps = PS([1, 512], "ctrl")
        for j in range(4):
            b, c = j // 2, j % 2
            tp_(at_ps[0:1, ds(128 * j, 128)], a_col[:, ds(2 * c + b, 1)],
                ident[:], skip_group_check=True)
        wwn_row2 = C_([1, 512], "ww_row2", FPR)
        wws4 = T_([1, 4], "wws4")
        for b in range(B):
            for c in range(2):
                v.scalar_tensor_tensor(
                    wwn_row2[0:1, ds(256 * b + 128 * c, 128)],
                    at_ps[0:1, ds(128 * (2 * b + c), 128)],
                    c1T[0:1, b:b + 1],
                    c2cw[b][0:1, ds(128 * c, 128)],
                    op0=OP.mult, op1=OP.add,
                    accum_out=wws4[0:1, ds(2 * b + c, 1)])
        wws2 = T_([1, 2], "wws2")
        v.tensor_reduce(wws2[:], wws4[:].rearrange("o (b c) -> o b c", c=2),
                        axis=AX.X, op=OP.add)
        # stacked [2,256] / [2,512] forms via selector-scatter in PSUM
        ww2r_ps = PS([2, 256], "ctrl")
        for b in range(B):
            mmr(ww2r_ps[:], selrow[b][:], wwn_row2[0:1, ds(256 * b, 256)],
                start=(b == 0), stop=(b == 1))
        wwn_2r = C_([2, 256], "ww_2r", FPR)
        v.tensor_copy(wwn_2r[:], ww2r_ps[:])
        wwblk_ps = PS([2, 512], "ctrl")
        for b in range(B):
            mmr(wwblk_ps[:, ds(256 * b, 256)], selrow[b][:],
                wwn_row2[0:1, ds(256 * b, 256)], start=True, stop=True,
                skip_group_check=True)
        wwn_blk = C_([2, 512], "ww_blk", FPR)
        sc.activation(wwn_blk[:], wwblk_ps[:], AF.Copy)
        # ww_col via transposes of stacked halves (cols come out as (b))
        for c in range(2):
            tp_(smT[:, ds(SM_WC + 2 * c, 2)].bitcast(FPR),
                wwn_2r[:, ds(128 * c, 128)], identR[0:2, 0:2],
                skip_group_check=True)
        wwn_col = C_([128, 4], "ww_col")
        v.tensor_copy(wwn_col[:], smT[:, ds(SM_WC, 4)])

        # ===== L / LT updates (old p as rhs) =====
        a2 = []
        for c in range(2):
            a2c = PS([128, 512], "a2", bufs=1)
            mmr(a2c[:], wwn_2r[:, ds(128 * c, 128)], negblockmask[:],
                start=True, stop=False)
            mmr(a2c[:], negones_row[:], wwn_row2[:], start=False,
                stop=True)
            a2.append(a2c)
        Ln = []
        LTn = []
        for c in range(2):
            b_c = PS([128, 512], "aux")
            mmr(b_c[:], wwn_2r[:, ds(128 * c, 128)], p_blk[:], start=True,
                stop=True)
            b2_c = PS([128, 512], "aux")
            mmr(b2_c[:], p_2r[:, ds(128 * c, 128)], wwn_blk[:], start=True,
                stop=True)
            lnc = C_([128, 512], f"L{c}", FPR)
            v.scalar_tensor_tensor(lnc[:], a2[c][:], 1.0, L[c][:], op0=OP.add,
                                   op1=OP.mult)
            v.tensor_add(lnc[:], lnc[:], b_c[:])
            gp.affine_select(lnc[:], lnc[:], pattern=[[0, 2], [-1, 256]],
                             compare_op=OP.not_equal, fill=0.0, base=128 * c,
                             channel_multiplier=1)
            Ln.append(lnc)
            ltc = C_([128, 512], f"LT{c}", FPR)
            v.scalar_tensor_tensor(ltc[:], a2[c][:], 1.0, LT[c][:],
                                   op0=OP.add, op1=OP.mult)
            v.tensor_add(ltc[:], ltc[:], b2_c[:])
            gp.affine_select(ltc[:], ltc[:], pattern=[[0, 2], [-1, 256]],
                             compare_op=OP.not_equal, fill=0.0, base=128 * c,
                             channel_multiplier=1)
            LTn.append(ltc)

        # ===== p update (row space + stacked forms) =====
        pn_row2 = C_([1, 512], "p_row2", FPR)
        nws2 = T_([1, 2], "nws2")
        sc.activation(nws2[:], wws2[:], AF.Identity, bias=1.0, scale=-1.0)
        for b in range(B):
            v.scalar_tensor_tensor(pn_row2[0:1, ds(256 * b, 256)],
                                   p_row2[0:1, ds(256 * b, 256)],
                                   nws2[0:1, b:b + 1],
                                   wwn_row2[0:1, ds(256 * b, 256)],
                                   op0=OP.mult, op1=OP.add)
        p2r_ps = PS([2, 256], "ctrl")
        for b in range(B):
            mmr(p2r_ps[:], selrow[b][:], pn_row2[0:1, ds(256 * b, 256)],
                start=(b == 0), stop=(b == 1))
        pn_2r = C_([2, 256], "p_2r", FPR)
        v.tensor_copy(pn_2r[:], p2r_ps[:])
        pblk_ps = PS([2, 512], "ctrl")
        for b in range(B):
            mmr(pblk_ps[:, ds(256 * b, 256)], selrow[b][:],
                pn_row2[0:1, ds(256 * b, 256)], start=True, stop=True,
                skip_group_check=True)
        pn_blk = C_([2, 512], "p_blk", FPR)
        sc.activation(pn_blk[:], pblk_ps[:], AF.Copy)

        # ===== M update =====
        negev_2r = T_([2, 64], "negev_2r", FPR)
        v.tensor_scalar_mul(negev_2r[:], sig[:, 0:64], -1.0)
        wv_2r = T_([2, 64], "wv_2r", FPR)
        v.tensor_copy(wv_2r[:], if_ps[:, C_WV:C_WV + 64])
        q1 = PS([64, 512], "aux")
        mmr(q1[:], negev_2r[:], wwn_blk[:], start=True, stop=True)
        q2 = PS([64, 512], "aux")
        mmr(q2[:], wv_2r[:], wwn_blk[:], start=True, stop=True)
        MTn = C_([64, 512], "MT", FPR)
        v.scalar_tensor_tensor(MTn[:], q1[:], 1.0, MT[:], op0=OP.add,
                               op1=OP.mult)
        v.tensor_add(MTn[:], MTn[:], q2[:])
        # Ms via transposes of MTn
        mst = PS([128, 512], "aux")
        for c in range(2):
            for b in range(B):
                tp_(mst[:, ds(64 * (2 * c + b), 64)].bitcast(FPR),
                    MTn[0:64, ds(256 * b + 128 * c, 128)],
                    identR[0:64, 0:64], skip_group_check=True)
        Msn = []
        for c in range(2):
            msc = C_([128, 128], f"Ms{c}", FPR)
            eng = v if c == 0 else sc
            if eng is sc:
                sc.activation(msc[:], mst[:, ds(128 * c, 128)], AF.Copy)
            else:
                v.tensor_copy(msc[:], mst[:, ds(128 * c, 128)])
            Msn.append(msc)

        # ===== rnorm (new M) =====
        mt2 = T_([64, 512], "mt2", FPR)
        sc.activation(mt2[:], MTn[:], AF.Square)
        nq = PS([2, 512], "aux")
        mmr(nq[:], onesR[0:64, 0:2], mt2[:], start=True, stop=True)
        rnln = T_([1, 512], "rnln")
        sc.activation(rnln[:], nq[0:1, :], AF.Ln, bias=cE12[0:1, 0:1])
        rnn_row2 = C_([1, 512], "rn_row2")
        sc.activation(rnn_row2[:], rnln[:], AF.Exp, scale=-0.5)

        # ===== rc on new M =====
        simr = []
        for b in range(B):
            srb = PS([4, 512], "ctrl")
            mmr(srb[:], keysR[:, ds(4 * b, 4)], MTn[:], start=True, stop=True)
            simr.append(srb)
        for b in range(B):
            mm(shx[ds(32, 4), ds(256 * b, 256)], ones_full[0:1, 0:4],
               rnn_row2[0:1, ds(256 * b, 256)], start=True, stop=True,
               skip_group_check=True)
        rn8_sb = T_([4, 512], "rn8_sb")
        sc.activation(rn8_sb[:], shx[ds(32, 4), :], AF.Copy)
        for b in range(B):
            rr = ds(64 * b, 4)  # rc rows reuse cw rows (consumed)
            v.scalar_tensor_tensor(shx[rr, 0:256],
                                   simr[b][:, ds(256 * b, 256)],
                                   rsRT[:, b:b + 1],
                                   rn8_sb[:, ds(256 * b, 256)],
                                   op0=OP.mult, op1=OP.mult)
            sc.activation(shx[rr, 256:512], shx[rr, 0:256], AF.Exp,
                          accum_out=smT[ds(64 * b, 4), ds(SM_LNU, 1)])
            v.reciprocal(smT[ds(64 * b, 4), ds(SM_LNU, 1)],
                         smT[ds(64 * b, 4), ds(SM_LNU, 1)])

        # ===== fwd / bwd / rw blend =====
        bwd = []
        fwd = []
        for b in range(B):
            bwb = PS([4, 512], "aux")
            for c in range(2):
                mmr(bwb[:], rw16[:, ds(8 * c + 4 * b, 4)], Ln[c][:],
                    start=(c == 0), stop=(c == 1))
            bwd.append(bwb)
        for b in range(B):
            fwb = PS([4, 512], "aux")
            for c in range(2):
                mmr(fwb[:], rw16[:, ds(8 * c + 4 * b, 4)], LTn[c][:],
                    start=(c == 0), stop=(c == 1))
            fwd.append(fwb)
        rwb = []
        for b in range(B):
            blk = ds(256 * b, 256)
            rwbb = T_([4, 256], f"rwb{b}")
            rm1c = T_([4, 1], f"rm1c{b}")
            v.tensor_tensor(rm1c[:], rm_m[1][:, b:b + 1],
                            smT[ds(64 * b, 4), ds(SM_LNU, 1)], op=OP.mult)
            v.tensor_scalar_mul(rwbb[:], bwd[b][:, blk],
                                rm_m[0][:, b:b + 1])
            v.scalar_tensor_tensor(rwbb[:], shx[ds(64 * b, 4), 256:512],
                                   rm1c[:], rwbb[:], op0=OP.mult, op1=OP.add)
            v.scalar_tensor_tensor(rwbb[:], fwd[b][:, blk],
                                   rm_m[2][:, b:b + 1], rwbb[:],
                                   op0=OP.mult, op1=OP.add)
            rwb.append(rwbb)
        for c in range(2):
            for b in range(B):
                tp_(smT[:, ds(SM_RWT + 8 * c + 4 * b, 4)],
                    rwb[b][:, ds(128 * c, 128)], ident[0:4, 0:4],
                    skip_group_check=True)
        rwn16 = C_([128, 16], "rw16", FPR)
        v.tensor_copy(rwn16[:], smT[:, ds(SM_RWT, 16)])

        # ===== rv =====
        rv_sb = []
        for b in range(B):
            rvb = PS([4, 64], "ctrl")
            for c in range(2):
                mmr(rvb[:], rwn16[:, ds(8 * c + 4 * b, 4)],
                    Msn[c][:, ds(64 * b, 64)], start=(c == 0), stop=(c == 1))
            rvsb = T_([4, 64], f"rv_sb{b}")
            v.tensor_copy(rvsb[:], rvb[:])
            rv_sb.append(rvsb)
        for b in range(B):
            tp_(smT[0:64, ds(SM_RVT + 4 * b, 4)], rv_sb[b][:],
                ident[0:4, 0:4], skip_group_check=True)
        rvn128 = C_([128, 4], "rvT128", FPR)
        for b in range(B):
            quad = smT[0:64, ds(SM_RVT + 4 * b, 4)].rearrange(
                "w (j k) -> w k j", k=2)
            v.tensor_copy(rvn128[0:64, ds(2 * b, 2)], quad[:, 0, :])
            v.tensor_copy(rvn128[64:128, ds(2 * b, 2)], quad[:, 1, :])

        # ===== output =====
        po = PS([2, O], "ctrl")
        for k in range(4):
            mmr(po[:], hT[:, ds(2 * k, 2)], wo_sb[k][:], start=(k == 0),
                stop=False)
        for j in range(2):
            lhs = rvn128[:].rearrange("p (b j) -> p j b", j=2)[:, j, :]
            mmr(po[:], lhs, wm2[j][:], start=False, stop=(j == 1))
        out_sb = T_([2, O], "out_sb")
        sc.activation(out_sb[:], po[:], AF.Copy)
        dma(out=out_d.ap().rearrange("t b o -> (t b) o")[ds(2 * t_step, 2), :],
            in_=out_sb[:])

        if dbg is not None and last:
            dma(out=dbg["h"].ap(), in_=h_sb[:])
            dma(out=dbg["sig"].ap(), in_=sig[:])
            dma(out=dbg["cw"].ap()[0:1], in_=c2cw[0][:])
            dma(out=dbg["cw"].ap()[1:2], in_=c2cw[1][:])
            dma(out=dbg["ret"].ap(), in_=ret4[:])
            dma(out=dbg["u"].ap(), in_=un_col[:])
            dma(out=dbg["a"].ap(), in_=a_col[:])
            dma(out=dbg["ww"].ap(), in_=wwn_row2[:])
            dma(out=dbg["mt"].ap(), in_=MTn[:])
            dma(out=dbg["rn"].ap(), in_=rnn_row2[:])
            dma(out=dbg["rc"].ap()[0:4], in_=shx[0:4, 256:512])
            dma(out=dbg["rc"].ap()[4:8], in_=shx[64:68, 256:512])
            dma(out=dbg["rw"].ap()[0:4], in_=rwb[0][:])
            dma(out=dbg["rw"].ap()[4:8], in_=rwb[1][:])
            dma(out=dbg["rv"].ap()[0:4], in_=rv_sb[0][:])
            dma(out=dbg["rv"].ap()[4:8], in_=rv_sb[1][:])
            dma(out=dbg["L0"].ap(), in_=Ln[0][:])
            dma(out=dbg["LT0"].ap(), in_=LTn[0][:])
            dma(out=dbg["p"].ap(), in_=pn_row2[:])
            dma(out=dbg["lnu"].ap(), in_=lnu_col[:])
            dma(out=dbg["eqc"].ap(), in_=eqc[:])
            dma(out=dbg["A1"].ap(), in_=A1[:])

        MT, Ms, L, LT = MTn, Msn, Ln, LTn
        u_col, ww_col = un_col, wwn_col
        ww_2r, ww_row2, ww_blk = wwn_2r, wwn_row2, wwn_blk
        p_2r, p_blk, p_row2 = pn_2r, pn_blk, pn_row2
        rw16, rvT128, rn_row2 = rwn16, rvn128, rnn_row2


# ---------------------------------------------------------------------------
# Public entry point
# ---------------------------------------------------------------------------
_T, _BFULL, _NCORES = 64, 16, 8
_cache = {}


def _get_nc(T=_T, debug=False, fix=True):
    key = ("nc", T, debug, fix)
    if key not in _cache:
        nc = bass.Bass("TRN2")
        build(nc, T, debug=debug)
        if fix:
            fix_sync_waits(nc)
        _cache[key] = nc
    return _cache[key]


def _get_jit():
    """Build the sharded PJRT executable once and reuse it across calls
    (run_bass_kernel_spmd re-traces jax.jit on every call)."""
    if "jit" in _cache:
        return _cache["jit"]
    import jax
    import numpy as _np
    from jax.sharding import Mesh, PartitionSpec
    from jax.experimental.shard_map import shard_map
    from concourse import bass2jax as _b2j
    from concourse import mybir as _mybir
    _b2j.install_neuronx_cc_hook()
    nc = _get_nc()
    partition_name = (nc.partition_id_tensor.name
                      if nc.partition_id_tensor else None)
    in_names, out_names, out_avals, zero_shapes = [], [], [], []
    for alloc in nc.m.functions[0].allocations:
        if not isinstance(alloc, _mybir.MemoryLocationSet):
            continue
        name = alloc.memorylocations[0].name
        if alloc.kind == "ExternalInput":
            if name != partition_name:
                in_names.append(name)
        elif alloc.kind == "ExternalOutput":
            shape = tuple(alloc.tensor_shape)
            dtype = _mybir.dt.np(alloc.dtype)
            out_names.append(name)
            out_avals.append(jax.core.ShapedArray(shape, dtype))
            zero_shapes.append((shape, dtype))
    n_params = len(in_names)
    n_outs = len(out_avals)
    all_names = list(in_names) + out_names
    if partition_name is not None:
        all_names.append(partition_name)

    def _body(*args):
        operands = list(args)
        if partition_name is not None:
            operands.append(_b2j.partition_id_tensor())
        outs = _b2j._bass_exec_p.bind(
            *operands, out_avals=tuple(out_avals), in_names=tuple(all_names),
            out_names=tuple(out_names), lowering_input_output_aliases=(),
            sim_require_finite=True, sim_require_nnan=True, nc=nc)
        return tuple(outs)

    devices = jax.devices()[:_NCORES]
    mesh = Mesh(_np.asarray(devices), ("core",))
    in_specs = (PartitionSpec("core"),) * (n_params + n_outs)
    out_specs = (PartitionSpec("core"),) * n_outs
    donate = tuple(range(n_params, n_params + n_outs))
    try:
        smapped = shard_map(_body, mesh=mesh, in_specs=in_specs,
                            out_specs=out_specs, check_rep=False)
    except TypeError:
        smapped = shard_map(_body, mesh=mesh, in_specs=in_specs,
                            out_specs=out_specs, check_vma=False)
    fn = jax.jit(smapped, donate_argnums=donate, keep_unused=True)
    _cache["mesh"] = mesh
    _cache["jit"] = (fn, in_names, out_names, out_avals, zero_shapes)
    return _cache["jit"]


def kernel(**inputs):
    x = np.ascontiguousarray(np.asarray(inputs["x"], dtype=np.float32))
    shared = {
        k: np.ascontiguousarray(np.asarray(inputs[k], dtype=np.float32))
        for k in ("W_hid", "b_hid", "W_iface", "W_out", "W_memout")
    }
    assert x.shape == (_T, _BFULL, I)
    in_maps = []
    for core in range(_NCORES):
        shard = np.ascontiguousarray(x[:, core * B:(core + 1) * B, :])
        m = {"x": shard}
        m.update(shared)
        in_maps.append(m)
    try:
        fn, in_names, out_names, out_avals, zero_shapes = _get_jit()
        import jax
        # Weights are replicated per core and rarely change between calls:
        # keep their device placement cached, revalidated by exact equality.
        wcache = _cache.setdefault("wdev", {})
        concat_in = []
        for name in in_names:
            host = np.concatenate(
                [in_maps[c][name] for c in range(_NCORES)], axis=0)
            if name == "x":
                concat_in.append(host)
                continue
            ent = wcache.get(name)
            if ent is not None and ent[0].shape == host.shape and \
                    np.array_equal(ent[0], host):
                concat_in.append(ent[1])
            else:
                from jax.sharding import NamedSharding, PartitionSpec as _P
                shd = NamedSharding(_cache["mesh"], _P("core"))
                dev = jax.device_put(host, shd)
                dev.block_until_ready()
                wcache[name] = (host.copy(), dev)
                concat_in.append(dev)
        concat_zeros = [np.zeros((_NCORES * sh[0],) + tuple(sh[1:]), dt)
                        for sh, dt in zero_shapes]
        out_arrs = fn(*concat_in, *concat_zeros)
        oi = out_names.index("out")
        res = np.asarray(out_arrs[oi]).reshape(_NCORES, _T, B, O)
        out = np.empty((_T, _BFULL, O), dtype=np.float32)
        for core in range(_NCORES):
            out[:, core * B:(core + 1) * B, :] = res[core]
        return out
    except Exception:
        nc = _get_nc()
        res = run_bass_kernel_spmd(nc, in_maps,
                                   core_ids=list(range(_NCORES)))
        out = np.empty((_T, _BFULL, O), dtype=np.float32)
        for core in range(_NCORES):
            out[:, core * B:(core + 1) * B, :] = res.results[core]["out"]
        return out


# revision 19
# speedup vs baseline: 12.9302x; 1.2311x over previous
"""Optimized TRN2 Bass kernel for the DNC (NeuCom) recurrence — v2.

Key changes vs v1 baseline:
- Single activation table (natural_log_exp): sigmoid via exp + DVE reciprocal,
  inverse norms via exp(-0.5*ln(q+eps)), oneplus via ln(1+exp(x)).
- float32r matmuls for all large-free matmuls (4x fewer PE cycles/row).
- Block-diagonal fused matmuls: both batches in one instruction for sims,
  M update, L/LT updates, fwd/bwd.
- L^T maintained as a carry with elementwise updates (no per-step transposes).
- Allocation (usage sort) via masked log-sum instead of explicit permutation
  matmuls + scan: a_i = (1-u_i) * exp(sum_{j sorted before i} ln u_j).
  Exact ties (which persist among never-written slots) are handled by an
  equality tie-count term; compares run in ln-space so lt/eq stay consistent.
- Engine rebalance: copies on Activation, some elementwise on Pool.

Hardware constraint honored throughout: every SBUF operand of a non-DMA
instruction must start at partition 0/32/64/96 (PSUM operands are exempt),
so per-batch row data lives in separate base-0 tiles and [2,X] stacked tiles
are built via one-hot selector matmuls accumulated in PSUM.
"""
from contextlib import ExitStack

import numpy as np

import concourse.bass as bass
import concourse.mybir as mybir
import concourse.tile as tile
from concourse.bass import ds, ts
from concourse.bass_utils import run_bass_kernel_spmd

_ctr = [0]


def fix_sync_waits(nc):
    """walrus accepts at most ONE sync-wait per instruction; split extras."""
    for f in nc.m.functions:
        for bb in f.blocks:
            new_insts = []
            for inst in bb.instructions:
                si = inst.sync_info
                waits = list(si.on_wait) if si is not None else []
                if len(waits) > 1:
                    extra, keep = waits[:-1], waits[-1:]
                    while extra:
                        chunk, extra = extra[:1], extra[1:]
                        _ctr[0] += 1
                        nop = mybir.InstNoOp(
                            name=f"WFIX-{_ctr[0]}",
                            engine=inst.engine,
                            sync_info=mybir.SyncInfo(on_wait=chunk, on_update=[]),
                            text_hint="waitfix",
                        )
                        new_insts.append(nop)
                    si.on_wait = keep
                new_insts.append(inst)
            bb.instructions = new_insts
    return nc


FP = mybir.dt.float32
FPR = mybir.dt.float32r
AF = mybir.ActivationFunctionType
OP = mybir.AluOpType
AX = mybir.AxisListType

N, Wd, R, B = 256, 64, 4, 2
H, I, O, IF = 512, 512, 512, 471

C_RK, C_RB, C_WK, C_WB, C_EV, C_WV, C_FG, C_AG, C_WG, C_RM = (
    0, 256, 260, 324, 325, 389, 453, 457, 458, 459)

EQ_ON_POOL = True       # tie-count stt ops on Pool (else DVE)
LT_ADD_ON_POOL = True   # LT "+b2" adds on Pool (else DVE)


def r_(ap):
    return ap.bitcast(FPR)


def build(nc: bass.Bass, T: int, debug: bool = False):
    x_d = nc.dram_tensor("x", [T, B, I], FP, kind="ExternalInput")
    wh_d = nc.dram_tensor("W_hid", [I + R * Wd, H], FP, kind="ExternalInput")
    bh_d = nc.dram_tensor("b_hid", [H], FP, kind="ExternalInput")
    wi_d = nc.dram_tensor("W_iface", [H, IF], FP, kind="ExternalInput")
    wo_d = nc.dram_tensor("W_out", [H, O], FP, kind="ExternalInput")
    wm_d = nc.dram_tensor("W_memout", [R * Wd, O], FP, kind="ExternalInput")
    out_d = nc.dram_tensor("out", [T, B, O], FP, kind="ExternalOutput")
    dbg = None
    if debug:
        dbg = {k: nc.dram_tensor(f"dbg_{k}", s, FP, kind="ExternalOutput")
               for k, s in [("h", [2, H]), ("sig", [2, 134]),
                            ("cw", [2, 256]), ("ret", [128, 4]),
                            ("u", [128, 4]), ("a", [128, 4]),
                            ("ww", [1, 512]), ("mt", [64, 512]),
                            ("rn", [1, 512]), ("rc", [8, 256]),
                            ("rw", [8, 256]), ("rv", [8, 64]),
                            ("L0", [128, 512]), ("LT0", [128, 512]),
                            ("p", [1, 512]), ("lnu", [128, 4]),
                            ("eqc", [128, 4]), ("A1", [128, 4])]}
    with tile.TileContext(nc) as tc:
        with ExitStack() as ctx:
            _build(ctx, tc, nc, T, x_d, wh_d, bh_d, wi_d, wo_d, wm_d, out_d,
                   dbg)
    return nc


def _build(ctx, tc, nc, T, x_d, wh_d, bh_d, wi_d, wo_d, wm_d, out_d, dbg):
    per = ctx.enter_context(tc.tile_pool(name="persist", bufs=1))
    car = ctx.enter_context(tc.tile_pool(name="carry", bufs=2))
    tmp = ctx.enter_context(tc.tile_pool(name="tmp", bufs=2))
    ps = ctx.enter_context(tc.tile_pool(name="ps", bufs=2, space="PSUM"))

    dma = nc.sync.dma_start
    v = nc.vector
    sc = nc.scalar
    gp = nc.gpsimd
    te = nc.tensor
    mm = te.matmul

    def mmr(out, lhsT, rhs, **kw):
        mm(out, r_(lhsT), r_(rhs), **kw)

    def tp_(out, in_, idn, **kw):
        mm(out, in_, idn, is_transpose=True, **kw)

    def T_(shape, tag, dt=FP):
        return tmp.tile(shape, dt, tag=tag, name=tag)

    def C_(shape, tag, dt=FP):
        return car.tile(shape, dt, tag=tag, name=tag)

    def P_(shape, tag, dt=FP):
        return per.tile(shape, dt, tag=tag, name=tag)

    def PS(shape, tag, bufs=None):
        return ps.tile(shape, FP, tag=tag, name=tag, bufs=bufs)

    # ---------------- constants ----------------
    ones_full = P_([128, 512], "ones_full")
    v.memset(ones_full[:], 1.0)
    ident = P_([128, 128], "ident")
    v.tensor_copy(ident[:], ones_full[:, 0:128])
    gp.affine_select(ident[:], ident[:], pattern=[[-1, 128]],
                     compare_op=OP.is_equal, fill=0.0, base=0,
                     channel_multiplier=1)
    # blockmask[b, n] = 1 if n in batch-b block
    blockmask = P_([2, 512], "blockmask")
    v.tensor_copy(blockmask[:], ones_full[0:2, :])
    gp.affine_select(blockmask[:], blockmask[:], pattern=[[1, 512]],
                     compare_op=OP.is_ge, fill=0.0, base=0,
                     channel_multiplier=-256)
    gp.affine_select(blockmask[:], blockmask[:], pattern=[[-1, 512]],
                     compare_op=OP.is_ge, fill=0.0, base=255,
                     channel_multiplier=256)
    jmask = []
    for c in range(2):
        jm = P_([128, 256], f"jmask{c}")
        gp.affine_select(jm[:], ones_full[:, 0:256], pattern=[[-1, 256]],
                         compare_op=OP.is_ge, fill=0.0, base=128 * c - 1,
                         channel_multiplier=1)
        jmask.append(jm)
    negblockmask = P_([2, 512], "negblockmask", FPR)
    v.tensor_scalar_mul(negblockmask[:], blockmask[:], -1.0)
    negones_row = P_([1, 128], "negones_row", FPR)
    v.tensor_scalar_mul(negones_row[:], ones_full[0:1, 0:128], -1.0)
    onesR = P_([128, 512], "onesR", FPR)
    v.tensor_copy(onesR[:], ones_full[:])
    identR = P_([128, 128], "identR", FPR)
    v.tensor_copy(identR[:], ident[:])
    cE12 = P_([128, 1], "cE12")
    v.memset(cE12[:], 1e-12)
    cE37 = P_([128, 1], "cE37")
    v.memset(cE37[:], 1e-37)
    # one-hot selectors
    selrow = []  # [1,2] rows for scatter (lhsT)
    for b in range(B):
        sf = P_([1, 2], f"selrowF{b}")
        v.memset(sf[:], 0.0)
        v.memset(sf[0:1, b:b + 1], 1.0)
        s = P_([1, 2], f"selrow{b}", FPR)
        v.tensor_copy(s[:], sf[:])
        selrow.append(s)
    selcol0 = P_([2, 1], "selcol0")
    v.memset(selcol0[:], 0.0)
    v.memset(selcol0[0:1, 0:1], 1.0)
    selcol1 = P_([2, 1], "selcol1")
    v.tensor_sub(selcol1[:], ones_full[0:2, 0:1], selcol0[:])
    selcol = [selcol0, selcol1]

    # ---------------- weights ----------------
    def load_w(dram, n_tiles, cols, name, row0=0, rows=128):
        out = []
        for k in range(n_tiles):
            t = P_([rows, cols], f"{name}{k}", FPR)
            nc.gpsimd.dma_start(out=t[:],
                                in_=dram.ap()[ds(row0 + k * rows, rows), :])
            out.append(t)
        return out

    wh_sb = load_w(wh_d, 4, H, "wh")
    wrv2 = load_w(wh_d, 2, H, "wrv2", row0=512, rows=128)
    # W_iface padded to even free size (f32r matmul ISA constraint)
    wi_sb = []
    for k in range(4):
        t = P_([128, IF + 1], f"wi{k}", FPR)
        v.tensor_scalar_mul(t[:], ones_full[:, 0:IF + 1], 0.0)
        nc.gpsimd.dma_start(out=t[:, 0:IF],
                            in_=wi_d.ap()[ds(k * 128, 128), :])
        wi_sb.append(t)
    wo_sb = load_w(wo_d, 4, O, "wo")
    wm2 = load_w(wm_d, 2, O, "wm2", rows=128)
    bh_sb = P_([1, H], "bh")
    dma(out=bh_sb[:], in_=bh_d.ap()[None, :])

    # ---------------- Xp precompute ----------------
    TB = T * B
    assert TB <= 128
    xnat = P_([128, I], "xnat")
    dma(out=xnat[:TB, :], in_=x_d.ap().rearrange("t b i -> (t b) i"))
    xt_sb = []
    for k in range(4):
        t = P_([128, 128], f"xt{k}", FPR)
        xtp = PS([128, 512], "ctrl")
        tp_(xtp[:, 0:TB], xnat[:TB, ts(k, 128)], ident[:TB, :TB])
        v.tensor_copy(t[:, :TB], xtp[:, 0:TB])
        xt_sb.append(t)
    xp_sb = P_([128, H], "xp", FPR)
    xp_ps = PS([128, H], "ctrl")
    for k in range(4):
        mmr(xp_ps[:TB, :], xt_sb[k][:, :TB], wh_sb[k][:], start=(k == 0),
            stop=False)
    mm(xp_ps[:TB, :], ones_full[0:1, :TB], bh_sb[:], start=False, stop=True)
    v.tensor_copy(xp_sb[:TB, :], xp_ps[:TB, :])

    # ---------------- carries (initial) ----------------
    MT = C_([64, 512], "MT", FPR)
    v.tensor_scalar_mul(MT[:], ones_full[0:64, :], 1e-6)
    Ms = []
    for c in range(2):
        m = C_([128, 128], f"Ms{c}", FPR)
        v.tensor_scalar_mul(m[:], ones_full[:, 0:128], 1e-6)
        Ms.append(m)
    L = []
    LT = []
    for c in range(2):
        l = C_([128, 512], f"L{c}", FPR)
        v.tensor_scalar_mul(l[:], ones_full[:], 0.0)
        L.append(l)
        lt = C_([128, 512], f"LT{c}", FPR)
        v.tensor_scalar_mul(lt[:], ones_full[:], 0.0)
        LT.append(lt)
    u_col = C_([128, 4], "u_col")
    v.memset(u_col[:], 0.0)
    ww_col = C_([128, 4], "ww_col")
    v.memset(ww_col[:], 0.0)
    ww_2r = C_([2, 256], "ww_2r", FPR)
    v.tensor_scalar_mul(ww_2r[:], ones_full[0:2, 0:256], 0.0)
    ww_row2 = C_([1, 512], "ww_row2", FPR)
    v.tensor_scalar_mul(ww_row2[:], ones_full[0:1, :], 0.0)
    ww_blk = C_([2, 512], "ww_blk", FPR)
    v.tensor_scalar_mul(ww_blk[:], ones_full[0:2, :], 0.0)
    p_2r = C_([2, 256], "p_2r", FPR)
    v.tensor_scalar_mul(p_2r[:], ones_full[0:2, 0:256], 0.0)
    p_blk = C_([2, 512], "p_blk", FPR)
    v.tensor_scalar_mul(p_blk[:], ones_full[0:2, :], 0.0)
    p_row2 = C_([1, 512], "p_row2", FPR)
    v.tensor_scalar_mul(p_row2[:], ones_full[0:1, :], 0.0)
    rw16 = C_([128, 16], "rw16", FPR)
    v.tensor_scalar_mul(rw16[:], ones_full[:, 0:16], 0.0)
    rvT128 = C_([128, 4], "rvT128", FPR)
    v.tensor_scalar_mul(rvT128[:], ones_full[:, 0:4], 0.0)
    rn_row2 = C_([1, 512], "rn_row2")
    v.memset(rn_row2[:], float((Wd * 1e-12 + 1e-12) ** -0.5))

    # smT column map (scratch PSUM bank, tag "sm"):
    SM_HTP, SM_RMG, SM_C12, SM_RST, SM_KT, SM_FGB = 0, 8, 14, 18, 22, 32
    SM_LNU, SM_AT, SM_WC, SM_RWT, SM_RVT = 48, 176, 304, 308, 324

    # ---------------- steps ----------------
    for t_step in range(T):
        last = (t_step == T - 1)
        smT = PS([128, 512], "sm", bufs=1)

        # ===== controller h =====
        h_ps = PS([2, H], "ctrl")
        for j in range(2):
            lhs = rvT128[:].rearrange("p (b j) -> p j b", j=2)[:, j, :]
            mmr(h_ps[:], lhs, wrv2[j][:], start=(j == 0), stop=False)
        mmr(h_ps[:], identR[:TB, ds(2 * t_step, 2)], xp_sb[:TB, :],
            start=False, stop=True)
        h_sb = T_([2, H], "h_sb")
        sc.activation(h_sb[:], h_ps[:], AF.Relu)
        for k in range(4):
            tp_(smT[:, ds(SM_HTP + 2 * k, 2)], h_sb[:, ts(k, 128)],
                ident[0:2, 0:2], skip_group_check=True)
        hT = T_([128, 8], "hT", FPR)
        v.tensor_copy(hT[:], smT[:, ds(SM_HTP, 8)])

        # ===== iface =====
        if_ps = PS([2, IF + 1], "ctrl")
        for k in range(4):
            mmr(if_ps[:], hT[:, ds(2 * k, 2)], wi_sb[k][:], start=(k == 0),
                stop=(k == 3))

        # -- iface activations (full 2-row ops only) --
        esig = T_([2, 134], "esig")
        sc.activation(esig[:], if_ps[:, C_EV:C_RM], AF.Exp, scale=-1.0)
        v.tensor_scalar_add(esig[:], esig[:], 1.0)
        sig = T_([2, 134], "sig")
        v.reciprocal(sig[:], esig[:])
        # sig: [,0:64]=ev  [,128:132]=fg  [,132:133]=ag  [,133:134]=wg

        rme = T_([2, 12], "rme")
        sc.activation(rme[:], if_ps[:, C_RM:C_RM + 12], AF.Exp)
        rmden = T_([2, 4], "rmden")
        v.tensor_reduce(rmden[:], rme[:].rearrange("b (r m) -> b r m", m=3),
                        axis=AX.X, op=OP.add)
        v.reciprocal(rmden[:], rmden[:])
        rmG = T_([2, 12], "rmG")
        v.tensor_tensor(
            out=rmG[:].rearrange("b (m r) -> b m r", r=4),
            in0=rme[:].rearrange("b (r m) -> b m r", m=3),
            in1=rmden[:].rearrange("b (u r) -> b u r", u=1).broadcast_to(
                [2, 3, 4]),
            op=OP.mult)
        for m3 in range(3):
            tp_(smT[0:4, ds(SM_RMG + 2 * m3, 2)], rmG[:, ds(4 * m3, 4)],
                ident[0:2, 0:2], skip_group_check=True)
        rm_m = []
        for m3 in range(3):
            rmt = T_([4, 2], f"rm_m{m3}")
            if m3 == 1:
                sc.activation(rmt[:], smT[0:4, ds(SM_RMG + 2 * m3, 2)],
                              AF.Copy)
            else:
                v.tensor_copy(rmt[:], smT[0:4, ds(SM_RMG + 2 * m3, 2)])
            rm_m.append(rmt)

        # gates -> transposed rows [1,2]
        c1 = T_([2, 1], "c1")
        v.tensor_tensor(c1[:], sig[:, 132:133], sig[:, 133:134], op=OP.mult)
        c2 = T_([2, 1], "c2")
        v.tensor_scalar(c2[:], sig[:, 132:133], -1.0, 1.0, op0=OP.mult,
                        op1=OP.add)
        v.tensor_mul(c2[:], c2[:], sig[:, 133:134])
        tp_(smT[0:1, ds(SM_C12, 2)], c1[:, 0:1], ident[0:2, 0:2],
            skip_group_check=True)
        tp_(smT[0:1, ds(SM_C12 + 2, 2)], c2[:, 0:1], ident[0:2, 0:2],
            skip_group_check=True)
        c1T = T_([1, 2], "c1T")
        c2T = T_([1, 2], "c2T")

        # oneplus(rb|wb) = 1 + ln(1+exp(x)); key norms; rs = (1+sp)/||k||
        bw5 = T_([2, 5], "bw5")
        sc.activation(bw5[:, 0:4], if_ps[:, C_RB:C_RB + 4], AF.Copy)
        sc.activation(bw5[:, 4:5], if_ps[:, C_WB:C_WB + 1], AF.Copy)
        sc.activation(bw5[:], bw5[:], AF.Exp)
        sc.activation(bw5[:], bw5[:], AF.Ln, bias=1.0)
        ifk = T_([2, 325], "ifk")
        v.tensor_copy(ifk[:], if_ps[:, 0:C_EV])
        ksq = T_([2, 325], "ksq")
        v.tensor_tensor(ksq[:], ifk[:], ifk[:], op=OP.mult)
        kn2 = T_([2, 5], "kn2")
        v.tensor_reduce(kn2[:, 0:4],
                        ksq[:, 0:256].rearrange("b (k w) -> b k w", w=64),
                        axis=AX.X, op=OP.add)
        v.tensor_reduce(kn2[:, 4:5], ksq[:, C_WK:C_WK + 64], axis=AX.X,
                        op=OP.add)
        sc.activation(kn2[:], kn2[:], AF.Ln, bias=cE12[0:2, 0:1])
        invkn = T_([2, 5], "invkn")
        sc.activation(invkn[:], kn2[:], AF.Exp, scale=-0.5)
        rs = T_([2, 5], "rs")
        v.scalar_tensor_tensor(rs[:], bw5[:], 1.0, invkn[:], op0=OP.add,
                               op1=OP.mult)
        # transpose read scales [2,4]->[4,2] and write scale [2,1]->[1,2]
        tp_(smT[0:4, ds(SM_RST, 2)], rs[:, 0:4], ident[0:2, 0:2],
            skip_group_check=True)
        tp_(smT[0:1, ds(SM_RST + 2, 2)], rs[:, 4:5], ident[0:2, 0:2],
            skip_group_check=True)
        rsRT = T_([4, 2], "rsRT")
        sc.activation(rsRT[:], smT[0:4, ds(SM_RST, 2)], AF.Copy)
        rsWT = T_([1, 2], "rsWT")
        sc.activation(rsWT[:], smT[0:1, ds(SM_RST + 2, 2)], AF.Copy)
        sc.activation(c1T[:], smT[0:1, ds(SM_C12, 2)], AF.Copy)
        sc.activation(c2T[:], smT[0:1, ds(SM_C12 + 2, 2)], AF.Copy)

        # ===== keys (raw; scales applied to sims) =====
        tp_(smT[:, ds(SM_KT, 2)], ifk[:, 0:128], ident[0:2, 0:2],
            skip_group_check=True)
        tp_(smT[:, ds(SM_KT + 2, 2)], ifk[:, 128:256], ident[0:2, 0:2],
            skip_group_check=True)
        tp_(smT[0:64, ds(SM_KT + 4, 2)], ifk[:, C_WK:C_WK + 64],
            ident[0:2, 0:2], skip_group_check=True)
        keysR = T_([64, 8], "keysR", FPR)
        keysW = T_([64, 8], "keysW", FPR)
        v.tensor_scalar_mul(keysW[:], ones_full[0:64, 0:8], 0.0)
        key_engs = [v, sc, v, sc]
        for kk in range(4):
            src = smT[ds(64 * (kk % 2), 64), ds(SM_KT + 2 * (kk // 2), 2)]
            eng = key_engs[kk]
            if eng is sc:
                sc.activation(
                    keysR[:].rearrange("w (b r) -> w r b", r=4)[:, kk, :],
                    src, AF.Copy)
            else:
                eng.tensor_copy(
                    keysR[:].rearrange("w (b r) -> w r b", r=4)[:, kk, :],
                    src)
        for b in range(B):
            v.tensor_copy(keysW[:, ds(4 * b, 1)],
                          smT[0:64, ds(SM_KT + 4 + b, 1)])

        # ===== cw on old M =====
        simw = []
        for b in range(B):
            swb = PS([2, 512], "ctrl")
            mmr(swb[:], keysW[:, ds(4 * b, 2)], MT[:], start=True, stop=True)
            simw.append(swb)
        shx = PS([128, 512], "shx", bufs=1)  # cw/rc rows 0:4,64:68; rn8 r32
        c2cw = []
        for b in range(B):
            r0 = ds(64 * b, 1)
            cwdb = T_([1, 1], f"cwd{b}")
            v.scalar_tensor_tensor(shx[r0, 0:256],
                                   simw[b][0:1, ds(256 * b, 256)],
                                   rsWT[0:1, b:b + 1],
                                   rn_row2[0:1, ds(256 * b, 256)],
                                   op0=OP.mult, op1=OP.mult)
            sc.activation(shx[r0, 256:512], shx[r0, 0:256], AF.Exp,
                          accum_out=cwdb[:])
            v.reciprocal(cwdb[:], cwdb[:])
            c2cwb = T_([1, 256], f"c2cw{b}")
            v.tensor_scalar(c2cwb[:], shx[r0, 256:512], cwdb[:],
                            c2T[0:1, b:b + 1], op0=OP.mult, op1=OP.mult)
            c2cw.append(c2cwb)

        # ===== usage =====
        fgrow = []
        for b in range(B):
            fgp = PS([1, 4], "ctrl")
            mm(fgp[:], selcol[b][:], sig[:, 128:132], start=True, stop=True)
            fgs = T_([1, 4], f"fgrow{b}")
            v.tensor_copy(fgs[:], fgp[:])
            fgrow.append(fgs)
        for c in range(2):
            for b in range(B):
                mm(smT[:, ds(SM_FGB + 8 * c + 4 * b, 4)],
                   ones_full[0:1, 0:128], fgrow[b][:], start=True, stop=True,
                   skip_group_check=True)
        m1 = T_([128, 16], "m1")
        v.scalar_tensor_tensor(m1[:], smT[:, ds(SM_FGB, 16)], -1.0, rw16[:],
                               op0=OP.mult, op1=OP.mult)
        m2 = T_([128, 16], "m2")
        sc.activation(m2[:], m1[:], AF.Identity, bias=1.0)
        q8 = T_([128, 8], "q8")
        gp.tensor_tensor(q8[:],
                        m2[:].rearrange("p (g r) -> p g r", r=2)[:, :, 0],
                        m2[:].rearrange("p (g r) -> p g r", r=2)[:, :, 1],
                        op=OP.mult)
        ret4 = T_([128, 4], "ret4")
        v.tensor_tensor(ret4[:],
                        q8[:].rearrange("p (h u) -> p h u", u=2)[:, :, 0],
                        q8[:].rearrange("p (h u) -> p h u", u=2)[:, :, 1],
                        op=OP.mult)
        t1 = T_([128, 4], "t1")
        gp.tensor_tensor(t1[:], u_col[:], ww_col[:], op=OP.mult)
        t2 = T_([128, 4], "t2")
        gp.tensor_add(t2[:], u_col[:], ww_col[:])
        v.tensor_sub(t2[:], t2[:], t1[:])
        un_col = C_([128, 4], "u_col")
        v.tensor_tensor(un_col[:], t2[:], ret4[:], op=OP.mult)

        # ===== allocation =====
        lnu_col = T_([128, 4], "lnu_col")
        sc.activation(lnu_col[:], un_col[:], AF.Ln, bias=cE37[:, 0:1])
        ut_ps = PS([1, 512], "ctrl")
        for j in range(4):
            b, c = j // 2, j % 2
            tp_(ut_ps[0:1, ds(128 * j, 128)],
                un_col[:, ds(2 * c + b, 1)], ident[:],
                skip_group_check=True)
        u_row2 = T_([1, 512], "u_row2")
        sc.activation(u_row2[:], ut_ps[:], AF.Copy)
        # per-batch PSUM bank: broadcast u_b; ln(u) goes to SBUF
        lnubc_sb = T_([128, 512], "lnubc_sb")
        ubcln = []
        for b in range(B):
            ub = PS([128, 256], "ubcln", bufs=1)
            mm(ub[:], ones_full[0:1, 0:128],
               u_row2[0:1, ds(256 * b, 256)], start=True, stop=True)
            sc.activation(lnubc_sb[:, ds(256 * b, 256)], ub[:], AF.Ln,
                          bias=cE37[:, 0:1])
            ubcln.append(ub)
        A1 = T_([128, 4], "A1")
        eqc = T_([128, 4], "eqc")
        for c in range(2):
            for b in range(B):
                col = ds(2 * c + b, 1)
                scr = T_([128, 256], f"scr{c}{b}")
                v.scalar_tensor_tensor(scr[:], ubcln[b][:],
                                       un_col[:, col],
                                       lnubc_sb[:, ds(256 * b, 256)],
                                       op0=OP.is_lt, op1=OP.mult,
                                       accum_out=A1[:, col])
                scr2 = T_([128, 256], f"scr2{c}{b}")
                v.scalar_tensor_tensor(scr2[:], ubcln[b][:],
                                       un_col[:, col], jmask[c][:],
                                       op0=OP.is_equal, op1=OP.mult,
                                       accum_out=eqc[:, col])
        A = T_([128, 4], "A")
        v.tensor_tensor(A[:], eqc[:], lnu_col[:], op=OP.mult)
        v.tensor_add(A[:], A[:], A1[:])
        cpx = T_([128, 4], "cpx")
        sc.activation(cpx[:], A[:], AF.Exp)
        onemu = T_([128, 4], "onemu")
        v.tensor_scalar(onemu[:], un_col[:], -1.0, 1.0, op0=OP.mult,
                        op1=OP.add)
        a_col = T_([128, 4], "a_col")
        v.tensor_tensor(a_col[:], onemu[:], cpx[:], op=OP.mult)

        # ===== ww (row space, written into [1,512] row) =====
        at_ps = PS([1, 512], "ctrl")
        for j in range(4):
            b, c = j // 2, j % 2
            tp_(at_ps[0:1, ds(128 * j, 128)], a_col[:, ds(2 * c + b, 1)],
                ident[:], skip_group_check=True)
        wwn_row2 = C_([1, 512], "ww_row2", FPR)
        wws4 = T_([1, 4], "wws4")
        for b in range(B):
            for c in range(2):
                v.scalar_tensor_tensor(
                    wwn_row2[0:1, ds(256 * b + 128 * c, 128)],
                    at_ps[0:1, ds(128 * (2 * b + c), 128)],
                    c1T[0:1, b:b + 1],
                    c2cw[b][0:1, ds(128 * c, 128)],
                    op0=OP.mult, op1=OP.add,
                    accum_out=wws4[0:1, ds(2 * b + c, 1)])
        wws2 = T_([1, 2], "wws2")
        v.tensor_reduce(wws2[:], wws4[:].rearrange("o (b c) -> o b c", c=2),
                        axis=AX.X, op=OP.add)
        # stacked [2,256] / [2,512] forms via selector-scatter in PSUM
        ww2r_ps = PS([2, 256], "ctrl")
        for b in range(B):
            mmr(ww2r_ps[:], selrow[b][:], wwn_row2[0:1, ds(256 * b, 256)],
                start=(b == 0), stop=(b == 1))
        wwn_2r = C_([2, 256], "ww_2r", FPR)
        v.tensor_copy(wwn_2r[:], ww2r_ps[:])
        wwblk_ps = PS([2, 512], "ctrl")
        for b in range(B):
            mmr(wwblk_ps[:, ds(256 * b, 256)], selrow[b][:],
                wwn_row2[0:1, ds(256 * b, 256)], start=True, stop=True,
                skip_group_check=True)
        wwn_blk = C_([2, 512], "ww_blk", FPR)
        sc.activation(wwn_blk[:], wwblk_ps[:], AF.Copy)
        # ww_col via transposes of stacked halves (cols come out as (b))
        for c in range(2):
            tp_(smT[:, ds(SM_WC + 2 * c, 2)].bitcast(FPR),
                wwn_2r[:, ds(128 * c, 128)], identR[0:2, 0:2],
                skip_group_check=True)
        wwn_col = C_([128, 4], "ww_col")
        v.tensor_copy(wwn_col[:], smT[:, ds(SM_WC, 4)])

        # ===== L / LT updates (old p as rhs) =====
        a2 = []
        for c in range(2):
            a2c = PS([128, 512], "a2", bufs=1)
            mmr(a2c[:], wwn_2r[:, ds(128 * c, 128)], negblockmask[:],
                start=True, stop=False)
            mmr(a2c[:], negones_row[:], wwn_row2[:], start=False,
                stop=True)
            a2.append(a2c)
        Ln = []
        LTn = []
        for c in range(2):
            b_c = PS([128, 512], "aux")
            mmr(b_c[:], wwn_2r[:, ds(128 * c, 128)], p_blk[:], start=True,
                stop=True)
            b2_c = PS([128, 512], "aux")
            mmr(b2_c[:], p_2r[:, ds(128 * c, 128)], wwn_blk[:], start=True,
                stop=True)
            lnc = C_([128, 512], f"L{c}", FPR)
            v.scalar_tensor_tensor(lnc[:], a2[c][:], 1.0, L[c][:], op0=OP.add,
                                   op1=OP.mult)
            v.tensor_add(lnc[:], lnc[:], b_c[:])
            gp.affine_select(lnc[:], lnc[:], pattern=[[0, 2], [-1, 256]],
                             compare_op=OP.not_equal, fill=0.0, base=128 * c,
                             channel_multiplier=1)
            Ln.append(lnc)
            ltc = C_([128, 512], f"LT{c}", FPR)
            v.scalar_tensor_tensor(ltc[:], a2[c][:], 1.0, LT[c][:],
                                   op0=OP.add, op1=OP.mult)
            v.tensor_add(ltc[:], ltc[:], b2_c[:])
            gp.affine_select(ltc[:], ltc[:], pattern=[[0, 2], [-1, 256]],
                             compare_op=OP.not_equal, fill=0.0, base=128 * c,
                             channel_multiplier=1)
            LTn.append(ltc)

        # ===== p update (row space + stacked forms) =====
        pn_row2 = C_([1, 512], "p_row2", FPR)
        nws2 = T_([1, 2], "nws2")
        sc.activation(nws2[:], wws2[:], AF.Identity, bias=1.0, scale=-1.0)
        for b in range(B):
            v.scalar_tensor_tensor(pn_row2[0:1, ds(256 * b, 256)],
                                   p_row2[0:1, ds(256 * b, 256)],
                                   nws2[0:1, b:b + 1],
                                   wwn_row2[0:1, ds(256 * b, 256)],
                                   op0=OP.mult, op1=OP.add)
        p2r_ps = PS([2, 256], "ctrl")
        for b in range(B):
            mmr(p2r_ps[:], selrow[b][:], pn_row2[0:1, ds(256 * b, 256)],
                start=(b == 0), stop=(b == 1))
        pn_2r = C_([2, 256], "p_2r", FPR)
        v.tensor_copy(pn_2r[:], p2r_ps[:])
        pblk_ps = PS([2, 512], "ctrl")
        for b in range(B):
            mmr(pblk_ps[:, ds(256 * b, 256)], selrow[b][:],
                pn_row2[0:1, ds(256 * b, 256)], start=True, stop=True,
                skip_group_check=True)
        pn_blk = C_([2, 512], "p_blk", FPR)
        sc.activation(pn_blk[:], pblk_ps[:], AF.Copy)

        # ===== M update =====
        negev_2r = T_([2, 64], "negev_2r", FPR)
        v.tensor_scalar_mul(negev_2r[:], sig[:, 0:64], -1.0)
        wv_2r = T_([2, 64], "wv_2r", FPR)
        v.tensor_copy(wv_2r[:], if_ps[:, C_WV:C_WV + 64])
        q1 = PS([64, 512], "aux")
        mmr(q1[:], negev_2r[:], wwn_blk[:], start=True, stop=True)
        q2 = PS([64, 512], "aux")
        mmr(q2[:], wv_2r[:], wwn_blk[:], start=True, stop=True)
        MTn = C_([64, 512], "MT", FPR)
        v.scalar_tensor_tensor(MTn[:], q1[:], 1.0, MT[:], op0=OP.add,
                               op1=OP.mult)
        v.tensor_add(MTn[:], MTn[:], q2[:])
        # Ms via transposes of MTn
        mst = PS([128, 512], "aux")
        for c in range(2):
            for b in range(B):
                tp_(mst[:, ds(64 * (2 * c + b), 64)].bitcast(FPR),
                    MTn[0:64, ds(256 * b + 128 * c, 128)],
                    identR[0:64, 0:64], skip_group_check=True)
        Msn = []
        for c in range(2):
            msc = C_([128, 128], f"Ms{c}", FPR)
            eng = v if c == 0 else sc
            if eng is sc:
                sc.activation(msc[:], mst[:, ds(128 * c, 128)], AF.Copy)
            else:
                v.tensor_copy(msc[:], mst[:, ds(128 * c, 128)])
            Msn.append(msc)

        # ===== rnorm (new M) =====
        mt2 = T_([64, 512], "mt2", FPR)
        sc.activation(mt2[:], MTn[:], AF.Square)
        nq = PS([2, 512], "aux")
        mmr(nq[:], onesR[0:64, 0:2], mt2[:], start=True, stop=True)
        rnln = T_([1, 512], "rnln")
        sc.activation(rnln[:], nq[0:1, :], AF.Ln, bias=cE12[0:1, 0:1])
        rnn_row2 = C_([1, 512], "rn_row2")
        sc.activation(rnn_row2[:], rnln[:], AF.Exp, scale=-0.5)

        # ===== rc on new M =====
        simr = []
        for b in range(B):
            srb = PS([4, 512], "ctrl")
            mmr(srb[:], keysR[:, ds(4 * b, 4)], MTn[:], start=True, stop=True)
            simr.append(srb)
        for b in range(B):
            mm(shx[ds(32, 4), ds(256 * b, 256)], ones_full[0:1, 0:4],
               rnn_row2[0:1, ds(256 * b, 256)], start=True, stop=True,
               skip_group_check=True)
        rn8_sb = T_([4, 512], "rn8_sb")
        sc.activation(rn8_sb[:], shx[ds(32, 4), :], AF.Copy)
        for b in range(B):
            rr = ds(64 * b, 4)  # rc rows reuse cw rows (consumed)
            v.scalar_tensor_tensor(shx[rr, 0:256],
                                   simr[b][:, ds(256 * b, 256)],
                                   rsRT[:, b:b + 1],
                                   rn8_sb[:, ds(256 * b, 256)],
                                   op0=OP.mult, op1=OP.mult)
            sc.activation(shx[rr, 256:512], shx[rr, 0:256], AF.Exp,
                          accum_out=smT[ds(64 * b, 4), ds(SM_LNU, 1)])
            v.reciprocal(smT[ds(64 * b, 4), ds(SM_LNU, 1)],
                         smT[ds(64 * b, 4), ds(SM_LNU, 1)])

        # ===== fwd / bwd / rw blend =====
        bwd = []
        fwd = []
        for b in range(B):
            bwb = PS([4, 512], "aux")
            for c in range(2):
                mmr(bwb[:], rw16[:, ds(8 * c + 4 * b, 4)], Ln[c][:],
                    start=(c == 0), stop=(c == 1))
            bwd.append(bwb)
        for b in range(B):
            fwb = PS([4, 512], "aux")
            for c in range(2):
                mmr(fwb[:], rw16[:, ds(8 * c + 4 * b, 4)], LTn[c][:],
                    start=(c == 0), stop=(c == 1))
            fwd.append(fwb)
        rwb = []
        for b in range(B):
            blk = ds(256 * b, 256)
            rwbb = T_([4, 256], f"rwb{b}")
            rm1c = T_([4, 1], f"rm1c{b}")
            v.tensor_tensor(rm1c[:], rm_m[1][:, b:b + 1],
                            smT[ds(64 * b, 4), ds(SM_LNU, 1)], op=OP.mult)
            v.tensor_scalar_mul(rwbb[:], bwd[b][:, blk],
                                rm_m[0][:, b:b + 1])
            v.scalar_tensor_tensor(rwbb[:], shx[ds(64 * b, 4), 256:512],
                                   rm1c[:], rwbb[:], op0=OP.mult, op1=OP.add)
            v.scalar_tensor_tensor(rwbb[:], fwd[b][:, blk],
                                   rm_m[2][:, b:b + 1], rwbb[:],
                                   op0=OP.mult, op1=OP.add)
            rwb.append(rwbb)
        for c in range(2):
            for b in range(B):
                tp_(smT[:, ds(SM_RWT + 8 * c + 4 * b, 4)],
                    rwb[b][:, ds(128 * c, 128)], ident[0:4, 0:4],
                    skip_group_check=True)
        rwn16 = C_([128, 16], "rw16", FPR)
        v.tensor_copy(rwn16[:], smT[:, ds(SM_RWT, 16)])

        # ===== rv =====
        rv_sb = []
        for b in range(B):
            rvb = PS([4, 64], "ctrl")
            for c in range(2):
                mmr(rvb[:], rwn16[:, ds(8 * c + 4 * b, 4)],
                    Msn[c][:, ds(64 * b, 64)], start=(c == 0), stop=(c == 1))
            rvsb = T_([4, 64], f"rv_sb{b}")
            v.tensor_copy(rvsb[:], rvb[:])
            rv_sb.append(rvsb)
        for b in range(B):
            tp_(smT[0:64, ds(SM_RVT + 4 * b, 4)], rv_sb[b][:],
                ident[0:4, 0:4], skip_group_check=True)
        rvn128 = C_([128, 4], "rvT128", FPR)
        for b in range(B):
            quad = smT[0:64, ds(SM_RVT + 4 * b, 4)].rearrange(
                "w (j k) -> w k j", k=2)
            v.tensor_copy(rvn128[0:64, ds(2 * b, 2)], quad[:, 0, :])
            v.tensor_copy(rvn128[64:128, ds(2 * b, 2)], quad[:, 1, :])

        # ===== output =====
        po = PS([2, O], "ctrl")
        for k in range(4):
            mmr(po[:], hT[:, ds(2 * k, 2)], wo_sb[k][:], start=(k == 0),
                stop=False)
        for j in range(2):
            lhs = rvn128[:].rearrange("p (b j) -> p j b", j=2)[:, j, :]
            mmr(po[:], lhs, wm2[j][:], start=False, stop=(j == 1))
        out_sb = T_([2, O], "out_sb")
        sc.activation(out_sb[:], po[:], AF.Copy)
        dma(out=out_d.ap().rearrange("t b o -> (t b) o")[ds(2 * t_step, 2), :],
            in_=out_sb[:])

        if dbg is not None and last:
            dma(out=dbg["h"].ap(), in_=h_sb[:])
            dma(out=dbg["sig"].ap(), in_=sig[:])
            dma(out=dbg["cw"].ap()[0:1], in_=c2cw[0][:])
            dma(out=dbg["cw"].ap()[1:2], in_=c2cw[1][:])
            dma(out=dbg["ret"].ap(), in_=ret4[:])
            dma(out=dbg["u"].ap(), in_=un_col[:])
            dma(out=dbg["a"].ap(), in_=a_col[:])
            dma(out=dbg["ww"].ap(), in_=wwn_row2[:])
            dma(out=dbg["mt"].ap(), in_=MTn[:])
            dma(out=dbg["rn"].ap(), in_=rnn_row2[:])
            dma(out=dbg["rc"].ap()[0:4], in_=shx[0:4, 256:512])
            dma(out=dbg["rc"].ap()[4:8], in_=shx[64:68, 256:512])
            dma(out=dbg["rw"].ap()[0:4], in_=rwb[0][:])
            dma(out=dbg["rw"].ap()[4:8], in_=rwb[1][:])
            dma(out=dbg["rv"].ap()[0:4], in_=rv_sb[0][:])
            dma(out=dbg["rv"].ap()[4:8], in_=rv_sb[1][:])
            dma(out=dbg["L0"].ap(), in_=Ln[0][:])
            dma(out=dbg["LT0"].ap(), in_=LTn[0][:])
            dma(out=dbg["p"].ap(), in_=pn_row2[:])
            dma(out=dbg["lnu"].ap(), in_=lnu_col[:])
            dma(out=dbg["eqc"].ap(), in_=eqc[:])
            dma(out=dbg["A1"].ap(), in_=A1[:])

        MT, Ms, L, LT = MTn, Msn, Ln, LTn
        u_col, ww_col = un_col, wwn_col
        ww_2r, ww_row2, ww_blk = wwn_2r, wwn_row2, wwn_blk
        p_2r, p_blk, p_row2 = pn_2r, pn_blk, pn_row2
        rw16, rvT128, rn_row2 = rwn16, rvn128, rnn_row2


# ---------------------------------------------------------------------------
# Public entry point
# ---------------------------------------------------------------------------
_T, _BFULL, _NCORES = 64, 16, 8
_cache = {}


def _get_nc(T=_T, debug=False, fix=True):
    key = ("nc", T, debug, fix)
    if key not in _cache:
        nc = bass.Bass("TRN2")
        build(nc, T, debug=debug)
        if fix:
            fix_sync_waits(nc)
        _cache[key] = nc
    return _cache[key]


def _get_jit():
    """Build the sharded PJRT executable once and reuse it across calls
    (run_bass_kernel_spmd re-traces jax.jit on every call)."""
    if "jit" in _cache:
        return _cache["jit"]
    import jax
    import numpy as _np
    from jax.sharding import Mesh, PartitionSpec
    from jax.experimental.shard_map import shard_map
    from concourse import bass2jax as _b2j
    from concourse import mybir as _mybir
    _b2j.install_neuronx_cc_hook()
    nc = _get_nc()
    partition_name = (nc.partition_id_tensor.name
                      if nc.partition_id_tensor else None)
    in_names, out_names, out_avals, zero_shapes = [], [], [], []
    for alloc in nc.m.functions[0].allocations:
        if not isinstance(alloc, _mybir.MemoryLocationSet):
            continue
        name = alloc.memorylocations[0].name
        if alloc.kind == "ExternalInput":
            if name != partition_name:
                in_names.append(name)
        elif alloc.kind == "ExternalOutput":
            shape = tuple(alloc.tensor_shape)
            dtype = _mybir.dt.np(alloc.dtype)
            out_names.append(name)
            out_avals.append(jax.core.ShapedArray(shape, dtype))
            zero_shapes.append((shape, dtype))
    n_params = len(in_names)
    n_outs = len(out_avals)
    all_names = list(in_names) + out_names
    if partition_name is not None:
        all_names.append(partition_name)

    def _body(*args):
        operands = list(args)
        if partition_name is not None:
            operands.append(_b2j.partition_id_tensor())
        outs = _b2j._bass_exec_p.bind(
            *operands, out_avals=tuple(out_avals), in_names=tuple(all_names),
            out_names=tuple(out_names), lowering_input_output_aliases=(),
            sim_require_finite=True, sim_require_nnan=True, nc=nc)
        return tuple(outs)

    devices = jax.devices()[:_NCORES]
    mesh = Mesh(_np.asarray(devices), ("core",))
    in_specs = (PartitionSpec("core"),) * (n_params + n_outs)
    out_specs = (PartitionSpec("core"),) * n_outs
    donate = tuple(range(n_params, n_params + n_outs))
    try:
        smapped = shard_map(_body, mesh=mesh, in_specs=in_specs,
                            out_specs=out_specs, check_rep=False)
    except TypeError:
        smapped = shard_map(_body, mesh=mesh, in_specs=in_specs,
                            out_specs=out_specs, check_vma=False)
    fn = jax.jit(smapped, donate_argnums=donate, keep_unused=True)
    _cache["mesh"] = mesh
    _cache["jit"] = (fn, in_names, out_names, out_avals, zero_shapes)
    return _cache["jit"]


def kernel(**inputs):
    x = np.ascontiguousarray(np.asarray(inputs["x"], dtype=np.float32))
    shared = {
        k: np.ascontiguousarray(np.asarray(inputs[k], dtype=np.float32))
        for k in ("W_hid", "b_hid", "W_iface", "W_out", "W_memout")
    }
    assert x.shape == (_T, _BFULL, I)
    in_maps = []
    for core in range(_NCORES):
        shard = np.ascontiguousarray(x[:, core * B:(core + 1) * B, :])
        m = {"x": shard}
        m.update(shared)
        in_maps.append(m)
    try:
        fn, in_names, out_names, out_avals, zero_shapes = _get_jit()
        import jax
        # Weights are replicated per core and rarely change between calls:
        # keep their device placement cached, revalidated by exact equality.
        wcache = _cache.setdefault("wdev", {})
        concat_in = []
        for name in in_names:
            if name == "x":
                concat_in.append(np.concatenate(
                    [in_maps[c][name] for c in range(_NCORES)], axis=0))
                continue
            # weights are identical across cores: validate against the
            # single-copy input, replicate only on cache miss
            single = in_maps[0][name]
            ent = wcache.get(name)
            if ent is not None and ent[0].shape == single.shape and \
                    np.array_equal(ent[0], single):
                concat_in.append(ent[1])
            else:
                from jax.sharding import NamedSharding, PartitionSpec as _P
                host = np.concatenate([single] * _NCORES, axis=0)
                shd = NamedSharding(_cache["mesh"], _P("core"))
                dev = jax.device_put(host, shd)
                dev.block_until_ready()
                wcache[name] = (single.copy(), dev)
                concat_in.append(dev)
        concat_zeros = [np.zeros((_NCORES * sh[0],) + tuple(sh[1:]), dt)
                        for sh, dt in zero_shapes]
        out_arrs = fn(*concat_in, *concat_zeros)
        oi = out_names.index("out")
        res = np.asarray(out_arrs[oi]).reshape(_NCORES, _T, B, O)
        out = np.empty((_T, _BFULL, O), dtype=np.float32)
        for core in range(_NCORES):
            out[:, core * B:(core + 1) * B, :] = res[core]
        return out
    except Exception:
        nc = _get_nc()
        res = run_bass_kernel_spmd(nc, in_maps,
                                   core_ids=list(range(_NCORES)))
        out = np.empty((_T, _BFULL, O), dtype=np.float32)
        for core in range(_NCORES):
            out[:, core * B:(core + 1) * B, :] = res.results[core]["out"]
        return out


# revision 21
# speedup vs baseline: 13.8346x; 1.0700x over previous
"""Optimized TRN2 Bass kernel for the DNC (NeuCom) recurrence — v2.

Key changes vs v1 baseline:
- Single activation table (natural_log_exp): sigmoid via exp + DVE reciprocal,
  inverse norms via exp(-0.5*ln(q+eps)), oneplus via ln(1+exp(x)).
- float32r matmuls for all large-free matmuls (4x fewer PE cycles/row).
- Block-diagonal fused matmuls: both batches in one instruction for sims,
  M update, L/LT updates, fwd/bwd.
- L^T maintained as a carry with elementwise updates (no per-step transposes).
- Allocation (usage sort) via masked log-sum instead of explicit permutation
  matmuls + scan: a_i = (1-u_i) * exp(sum_{j sorted before i} ln u_j).
  Exact ties (which persist among never-written slots) are handled by an
  equality tie-count term; compares run in ln-space so lt/eq stay consistent.
- Engine rebalance: copies on Activation, some elementwise on Pool.

Hardware constraint honored throughout: every SBUF operand of a non-DMA
instruction must start at partition 0/32/64/96 (PSUM operands are exempt),
so per-batch row data lives in separate base-0 tiles and [2,X] stacked tiles
are built via one-hot selector matmuls accumulated in PSUM.
"""
from contextlib import ExitStack

import numpy as np

import concourse.bass as bass
import concourse.mybir as mybir
import concourse.tile as tile
from concourse.bass import ds, ts
from concourse.bass_utils import run_bass_kernel_spmd

_ctr = [0]


def fix_sync_waits(nc):
    """walrus accepts at most ONE sync-wait per instruction; split extras."""
    for f in nc.m.functions:
        for bb in f.blocks:
            new_insts = []
            for inst in bb.instructions:
                si = inst.sync_info
                waits = list(si.on_wait) if si is not None else []
                if len(waits) > 1:
                    extra, keep = waits[:-1], waits[-1:]
                    while extra:
                        chunk, extra = extra[:1], extra[1:]
                        _ctr[0] += 1
                        nop = mybir.InstNoOp(
                            name=f"WFIX-{_ctr[0]}",
                            engine=inst.engine,
                            sync_info=mybir.SyncInfo(on_wait=chunk, on_update=[]),
                            text_hint="waitfix",
                        )
                        new_insts.append(nop)
                    si.on_wait = keep
                new_insts.append(inst)
            bb.instructions = new_insts
    return nc


FP = mybir.dt.float32
FPR = mybir.dt.float32r
AF = mybir.ActivationFunctionType
OP = mybir.AluOpType
AX = mybir.AxisListType

N, Wd, R, B = 256, 64, 4, 2
H, I, O, IF = 512, 512, 512, 471

C_RK, C_RB, C_WK, C_WB, C_EV, C_WV, C_FG, C_AG, C_WG, C_RM = (
    0, 256, 260, 324, 325, 389, 453, 457, 458, 459)

EQ_ON_POOL = True       # tie-count stt ops on Pool (else DVE)
LT_ADD_ON_POOL = True   # LT "+b2" adds on Pool (else DVE)


def r_(ap):
    return ap.bitcast(FPR)


def build(nc: bass.Bass, T: int, debug: bool = False):
    x_d = nc.dram_tensor("x", [T, B, I], FP, kind="ExternalInput")
    wh_d = nc.dram_tensor("W_hid", [I + R * Wd, H], FP, kind="ExternalInput")
    bh_d = nc.dram_tensor("b_hid", [H], FP, kind="ExternalInput")
    wi_d = nc.dram_tensor("W_iface", [H, IF], FP, kind="ExternalInput")
    wo_d = nc.dram_tensor("W_out", [H, O], FP, kind="ExternalInput")
    wm_d = nc.dram_tensor("W_memout", [R * Wd, O], FP, kind="ExternalInput")
    out_d = nc.dram_tensor("out", [T, B, O], FP, kind="ExternalOutput")
    dbg = None
    if debug:
        dbg = {k: nc.dram_tensor(f"dbg_{k}", s, FP, kind="ExternalOutput")
               for k, s in [("h", [2, H]), ("sig", [2, 134]),
                            ("cw", [2, 256]), ("ret", [128, 4]),
                            ("u", [128, 4]), ("a", [128, 4]),
                            ("ww", [1, 512]), ("mt", [64, 512]),
                            ("rn", [1, 512]), ("rc", [8, 256]),
                            ("rw", [8, 256]), ("rv", [8, 64]),
                            ("L0", [128, 512]), ("LT0", [128, 512]),
                            ("p", [1, 512]), ("lnu", [128, 4]),
                            ("eqc", [128, 4]), ("A1", [128, 4])]}
    with tile.TileContext(nc) as tc:
        with ExitStack() as ctx:
            _build(ctx, tc, nc, T, x_d, wh_d, bh_d, wi_d, wo_d, wm_d, out_d,
                   dbg)
    return nc


def _build(ctx, tc, nc, T, x_d, wh_d, bh_d, wi_d, wo_d, wm_d, out_d, dbg):
    per = ctx.enter_context(tc.tile_pool(name="persist", bufs=1))
    car = ctx.enter_context(tc.tile_pool(name="carry", bufs=2))
    tmp = ctx.enter_context(tc.tile_pool(name="tmp", bufs=2))
    ps = ctx.enter_context(tc.tile_pool(name="ps", bufs=2, space="PSUM"))

    dma = nc.sync.dma_start
    v = nc.vector
    sc = nc.scalar
    gp = nc.gpsimd
    te = nc.tensor
    mm = te.matmul

    def mmr(out, lhsT, rhs, **kw):
        mm(out, r_(lhsT), r_(rhs), **kw)

    def tp_(out, in_, idn, **kw):
        mm(out, in_, idn, is_transpose=True, **kw)

    def T_(shape, tag, dt=FP):
        return tmp.tile(shape, dt, tag=tag, name=tag)

    def C_(shape, tag, dt=FP):
        return car.tile(shape, dt, tag=tag, name=tag)

    def P_(shape, tag, dt=FP):
        return per.tile(shape, dt, tag=tag, name=tag)

    def PS(shape, tag, bufs=None):
        return ps.tile(shape, FP, tag=tag, name=tag, bufs=bufs)

    # ---------------- constants ----------------
    ones_full = P_([128, 512], "ones_full")
    v.memset(ones_full[:], 1.0)
    ident = P_([128, 128], "ident")
    v.tensor_copy(ident[:], ones_full[:, 0:128])
    gp.affine_select(ident[:], ident[:], pattern=[[-1, 128]],
                     compare_op=OP.is_equal, fill=0.0, base=0,
                     channel_multiplier=1)
    # blockmask[b, n] = 1 if n in batch-b block
    blockmask = P_([2, 512], "blockmask")
    v.tensor_copy(blockmask[:], ones_full[0:2, :])
    gp.affine_select(blockmask[:], blockmask[:], pattern=[[1, 512]],
                     compare_op=OP.is_ge, fill=0.0, base=0,
                     channel_multiplier=-256)
    gp.affine_select(blockmask[:], blockmask[:], pattern=[[-1, 512]],
                     compare_op=OP.is_ge, fill=0.0, base=255,
                     channel_multiplier=256)
    jmask = []
    for c in range(2):
        jm = P_([128, 256], f"jmask{c}")
        gp.affine_select(jm[:], ones_full[:, 0:256], pattern=[[-1, 256]],
                         compare_op=OP.is_ge, fill=0.0, base=128 * c - 1,
                         channel_multiplier=1)
        jmask.append(jm)
    negblockmask = P_([2, 512], "negblockmask", FPR)
    v.tensor_scalar_mul(negblockmask[:], blockmask[:], -1.0)
    negones_row = P_([1, 128], "negones_row", FPR)
    v.tensor_scalar_mul(negones_row[:], ones_full[0:1, 0:128], -1.0)
    onesR = P_([128, 512], "onesR", FPR)
    v.tensor_copy(onesR[:], ones_full[:])
    identR = P_([128, 128], "identR", FPR)
    v.tensor_copy(identR[:], ident[:])
    cE12 = P_([128, 1], "cE12")
    v.memset(cE12[:], 1e-12)
    cE37 = P_([128, 1], "cE37")
    v.memset(cE37[:], 1e-37)
    # one-hot selectors
    selrow = []  # [1,2] rows for scatter (lhsT)
    for b in range(B):
        sf = P_([1, 2], f"selrowF{b}")
        v.memset(sf[:], 0.0)
        v.memset(sf[0:1, b:b + 1], 1.0)
        s = P_([1, 2], f"selrow{b}", FPR)
        v.tensor_copy(s[:], sf[:])
        selrow.append(s)
    selcol0 = P_([2, 1], "selcol0")
    v.memset(selcol0[:], 0.0)
    v.memset(selcol0[0:1, 0:1], 1.0)
    selcol1 = P_([2, 1], "selcol1")
    v.tensor_sub(selcol1[:], ones_full[0:2, 0:1], selcol0[:])
    selcol = [selcol0, selcol1]

    # ---------------- weights ----------------
    def load_w(dram, n_tiles, cols, name, row0=0, rows=128):
        out = []
        for k in range(n_tiles):
            t = P_([rows, cols], f"{name}{k}", FPR)
            nc.gpsimd.dma_start(out=t[:],
                                in_=dram.ap()[ds(row0 + k * rows, rows), :])
            out.append(t)
        return out

    wh_sb = load_w(wh_d, 4, H, "wh")
    wrv2 = load_w(wh_d, 2, H, "wrv2", row0=512, rows=128)
    # W_iface padded to even free size (f32r matmul ISA constraint)
    wi_sb = []
    for k in range(4):
        t = P_([128, IF + 1], f"wi{k}", FPR)
        v.tensor_scalar_mul(t[:], ones_full[:, 0:IF + 1], 0.0)
        nc.gpsimd.dma_start(out=t[:, 0:IF],
                            in_=wi_d.ap()[ds(k * 128, 128), :])
        wi_sb.append(t)
    wo_sb = load_w(wo_d, 4, O, "wo")
    wm2 = load_w(wm_d, 2, O, "wm2", rows=128)
    bh_sb = P_([1, H], "bh")
    dma(out=bh_sb[:], in_=bh_d.ap()[None, :])

    # ---------------- Xp precompute ----------------
    TB = T * B
    assert TB <= 128
    xnat = P_([128, I], "xnat")
    dma(out=xnat[:TB, :], in_=x_d.ap().rearrange("t b i -> (t b) i"))
    xt_sb = []
    for k in range(4):
        t = P_([128, 128], f"xt{k}", FPR)
        xtp = PS([128, 512], "ctrl")
        tp_(xtp[:, 0:TB], xnat[:TB, ts(k, 128)], ident[:TB, :TB])
        v.tensor_copy(t[:, :TB], xtp[:, 0:TB])
        xt_sb.append(t)
    xp_sb = P_([128, H], "xp", FPR)
    xp_ps = PS([128, H], "ctrl")
    for k in range(4):
        mmr(xp_ps[:TB, :], xt_sb[k][:, :TB], wh_sb[k][:], start=(k == 0),
            stop=False)
    mm(xp_ps[:TB, :], ones_full[0:1, :TB], bh_sb[:], start=False, stop=True)
    v.tensor_copy(xp_sb[:TB, :], xp_ps[:TB, :])

    # ---------------- carries (initial) ----------------
    MT = C_([64, 512], "MT", FPR)
    v.tensor_scalar_mul(MT[:], ones_full[0:64, :], 1e-6)
    Ms = []
    for c in range(2):
        m = C_([128, 128], f"Ms{c}", FPR)
        v.tensor_scalar_mul(m[:], ones_full[:, 0:128], 1e-6)
        Ms.append(m)
    L = []
    LT = []
    for c in range(2):
        l = C_([128, 512], f"L{c}", FPR)
        v.tensor_scalar_mul(l[:], ones_full[:], 0.0)
        L.append(l)
        lt = C_([128, 512], f"LT{c}", FPR)
        v.tensor_scalar_mul(lt[:], ones_full[:], 0.0)
        LT.append(lt)
    u_col = C_([128, 4], "u_col")
    v.memset(u_col[:], 0.0)
    ww_col = C_([128, 4], "ww_col")
    v.memset(ww_col[:], 0.0)
    ww_2r = C_([2, 256], "ww_2r", FPR)
    v.tensor_scalar_mul(ww_2r[:], ones_full[0:2, 0:256], 0.0)
    ww_row2 = C_([1, 512], "ww_row2", FPR)
    v.tensor_scalar_mul(ww_row2[:], ones_full[0:1, :], 0.0)
    ww_blk = C_([2, 512], "ww_blk", FPR)
    v.tensor_scalar_mul(ww_blk[:], ones_full[0:2, :], 0.0)
    p_2r = C_([2, 256], "p_2r", FPR)
    v.tensor_scalar_mul(p_2r[:], ones_full[0:2, 0:256], 0.0)
    p_blk = C_([2, 512], "p_blk", FPR)
    v.tensor_scalar_mul(p_blk[:], ones_full[0:2, :], 0.0)
    p_row2 = C_([1, 512], "p_row2", FPR)
    v.tensor_scalar_mul(p_row2[:], ones_full[0:1, :], 0.0)
    rw16 = C_([128, 16], "rw16", FPR)
    v.tensor_scalar_mul(rw16[:], ones_full[:, 0:16], 0.0)
    rvT128 = C_([128, 4], "rvT128", FPR)
    v.tensor_scalar_mul(rvT128[:], ones_full[:, 0:4], 0.0)
    rn_row2 = C_([1, 512], "rn_row2")
    v.memset(rn_row2[:], float((Wd * 1e-12 + 1e-12) ** -0.5))

    # smT column map (scratch PSUM bank, tag "sm"):
    SM_HTP, SM_RMG, SM_C12, SM_RST, SM_KT, SM_FGB = 0, 8, 14, 18, 22, 32
    SM_LNU, SM_AT, SM_WC, SM_RWT, SM_RVT = 48, 176, 304, 308, 324

    # ---------------- steps ----------------
    for t_step in range(T):
        last = (t_step == T - 1)
        smT = PS([128, 512], "sm", bufs=1)

        # ===== controller h =====
        h_ps = PS([2, H], "ctrl")
        for j in range(2):
            lhs = rvT128[:].rearrange("p (b j) -> p j b", j=2)[:, j, :]
            mmr(h_ps[:], lhs, wrv2[j][:], start=(j == 0), stop=False)
        mmr(h_ps[:], identR[:TB, ds(2 * t_step, 2)], xp_sb[:TB, :],
            start=False, stop=True)
        h_sb = T_([2, H], "h_sb")
        sc.activation(h_sb[:], h_ps[:], AF.Relu)
        for k in range(4):
            tp_(smT[:, ds(SM_HTP + 2 * k, 2)], h_sb[:, ts(k, 128)],
                ident[0:2, 0:2], skip_group_check=True)
        hT = T_([128, 8], "hT", FPR)
        v.tensor_copy(hT[:], smT[:, ds(SM_HTP, 8)])

        # ===== iface =====
        if_ps = PS([2, IF + 1], "ctrl")
        for k in range(4):
            mmr(if_ps[:], hT[:, ds(2 * k, 2)], wi_sb[k][:], start=(k == 0),
                stop=(k == 3))

        # -- iface activations (full 2-row ops only) --
        esig = T_([2, 134], "esig")
        sc.activation(esig[:], if_ps[:, C_EV:C_RM], AF.Exp, scale=-1.0)
        v.tensor_scalar_add(esig[:], esig[:], 1.0)
        sig = T_([2, 134], "sig")
        v.reciprocal(sig[:], esig[:])
        # sig: [,0:64]=ev  [,128:132]=fg  [,132:133]=ag  [,133:134]=wg

        rme = T_([2, 12], "rme")
        sc.activation(rme[:], if_ps[:, C_RM:C_RM + 12], AF.Exp)
        rmden = T_([2, 4], "rmden")
        v.tensor_reduce(rmden[:], rme[:].rearrange("b (r m) -> b r m", m=3),
                        axis=AX.X, op=OP.add)
        v.reciprocal(rmden[:], rmden[:])
        rmG = T_([2, 12], "rmG")
        v.tensor_tensor(
            out=rmG[:].rearrange("b (m r) -> b m r", r=4),
            in0=rme[:].rearrange("b (r m) -> b m r", m=3),
            in1=rmden[:].rearrange("b (u r) -> b u r", u=1).broadcast_to(
                [2, 3, 4]),
            op=OP.mult)
        for m3 in range(3):
            tp_(smT[0:4, ds(SM_RMG + 2 * m3, 2)], rmG[:, ds(4 * m3, 4)],
                ident[0:2, 0:2], skip_group_check=True)
        rm_m = []
        for m3 in range(3):
            rmt = T_([4, 2], f"rm_m{m3}")
            if m3 == 1:
                sc.activation(rmt[:], smT[0:4, ds(SM_RMG + 2 * m3, 2)],
                              AF.Copy)
            else:
                v.tensor_copy(rmt[:], smT[0:4, ds(SM_RMG + 2 * m3, 2)])
            rm_m.append(rmt)

        # gates -> transposed rows [1,2]
        c1 = T_([2, 1], "c1")
        v.tensor_tensor(c1[:], sig[:, 132:133], sig[:, 133:134], op=OP.mult)
        c2 = T_([2, 1], "c2")
        v.tensor_scalar(c2[:], sig[:, 132:133], -1.0, 1.0, op0=OP.mult,
                        op1=OP.add)
        v.tensor_mul(c2[:], c2[:], sig[:, 133:134])
        tp_(smT[0:1, ds(SM_C12, 2)], c1[:, 0:1], ident[0:2, 0:2],
            skip_group_check=True)
        tp_(smT[0:1, ds(SM_C12 + 2, 2)], c2[:, 0:1], ident[0:2, 0:2],
            skip_group_check=True)
        c1T = T_([1, 2], "c1T")
        c2T = T_([1, 2], "c2T")

        # oneplus(rb|wb) = 1 + ln(1+exp(x)); key norms; rs = (1+sp)/||k||
        bw5 = T_([2, 5], "bw5")
        sc.activation(bw5[:, 0:4], if_ps[:, C_RB:C_RB + 4], AF.Copy)
        sc.activation(bw5[:, 4:5], if_ps[:, C_WB:C_WB + 1], AF.Copy)
        sc.activation(bw5[:], bw5[:], AF.Exp)
        sc.activation(bw5[:], bw5[:], AF.Ln, bias=1.0)
        ifk = T_([2, 325], "ifk")
        v.tensor_copy(ifk[:], if_ps[:, 0:C_EV])
        ksq = T_([2, 325], "ksq")
        v.tensor_tensor(ksq[:], ifk[:], ifk[:], op=OP.mult)
        kn2 = T_([2, 5], "kn2")
        v.tensor_reduce(kn2[:, 0:4],
                        ksq[:, 0:256].rearrange("b (k w) -> b k w", w=64),
                        axis=AX.X, op=OP.add)
        v.tensor_reduce(kn2[:, 4:5], ksq[:, C_WK:C_WK + 64], axis=AX.X,
                        op=OP.add)
        sc.activation(kn2[:], kn2[:], AF.Ln, bias=cE12[0:2, 0:1])
        invkn = T_([2, 5], "invkn")
        sc.activation(invkn[:], kn2[:], AF.Exp, scale=-0.5)
        rs = T_([2, 5], "rs")
        v.scalar_tensor_tensor(rs[:], bw5[:], 1.0, invkn[:], op0=OP.add,
                               op1=OP.mult)
        # transpose read scales [2,4]->[4,2] and write scale [2,1]->[1,2]
        tp_(smT[0:4, ds(SM_RST, 2)], rs[:, 0:4], ident[0:2, 0:2],
            skip_group_check=True)
        tp_(smT[0:1, ds(SM_RST + 2, 2)], rs[:, 4:5], ident[0:2, 0:2],
            skip_group_check=True)
        rsRT = T_([4, 2], "rsRT")
        sc.activation(rsRT[:], smT[0:4, ds(SM_RST, 2)], AF.Copy)
        rsWT = T_([1, 2], "rsWT")
        sc.activation(rsWT[:], smT[0:1, ds(SM_RST + 2, 2)], AF.Copy)
        sc.activation(c1T[:], smT[0:1, ds(SM_C12, 2)], AF.Copy)
        sc.activation(c2T[:], smT[0:1, ds(SM_C12 + 2, 2)], AF.Copy)

        # ===== keys (raw; scales applied to sims) =====
        tp_(smT[:, ds(SM_KT, 2)], ifk[:, 0:128], ident[0:2, 0:2],
            skip_group_check=True)
        tp_(smT[:, ds(SM_KT + 2, 2)], ifk[:, 128:256], ident[0:2, 0:2],
            skip_group_check=True)
        tp_(smT[0:64, ds(SM_KT + 4, 2)], ifk[:, C_WK:C_WK + 64],
            ident[0:2, 0:2], skip_group_check=True)
        keysR = T_([64, 8], "keysR", FPR)
        keysW = T_([64, 8], "keysW", FPR)
        v.tensor_scalar_mul(keysW[:], ones_full[0:64, 0:8], 0.0)
        key_engs = [v, sc, v, sc]
        for kk in range(4):
            src = smT[ds(64 * (kk % 2), 64), ds(SM_KT + 2 * (kk // 2), 2)]
            eng = key_engs[kk]
            if eng is sc:
                sc.activation(
                    keysR[:].rearrange("w (b r) -> w r b", r=4)[:, kk, :],
                    src, AF.Copy)
            else:
                eng.tensor_copy(
                    keysR[:].rearrange("w (b r) -> w r b", r=4)[:, kk, :],
                    src)
        for b in range(B):
            v.tensor_copy(keysW[:, ds(4 * b, 1)],
                          smT[0:64, ds(SM_KT + 4 + b, 1)])

        # ===== cw on old M =====
        simw = []
        for b in range(B):
            swb = PS([2, 512], "ctrl")
            mmr(swb[:], keysW[:, ds(4 * b, 2)], MT[:], start=True, stop=True)
            simw.append(swb)
        shx = PS([128, 512], "shx", bufs=1)  # cw/rc rows 0:4,64:68; rn8 r32
        c2cw = []
        for b in range(B):
            r0 = ds(64 * b, 1)
            cwdb = T_([1, 1], f"cwd{b}")
            v.scalar_tensor_tensor(shx[r0, 0:256],
                                   simw[b][0:1, ds(256 * b, 256)],
                                   rsWT[0:1, b:b + 1],
                                   rn_row2[0:1, ds(256 * b, 256)],
                                   op0=OP.mult, op1=OP.mult)
            sc.activation(shx[r0, 256:512], shx[r0, 0:256], AF.Exp,
                          accum_out=cwdb[:])
            v.reciprocal(cwdb[:], cwdb[:])
            c2cwb = T_([1, 256], f"c2cw{b}")
            v.tensor_scalar(c2cwb[:], shx[r0, 256:512], cwdb[:],
                            c2T[0:1, b:b + 1], op0=OP.mult, op1=OP.mult)
            c2cw.append(c2cwb)

        # ===== usage =====
        fgrow = []
        for b in range(B):
            fgp = PS([1, 4], "ctrl")
            mm(fgp[:], selcol[b][:], sig[:, 128:132], start=True, stop=True)
            fgs = T_([1, 4], f"fgrow{b}")
            v.tensor_copy(fgs[:], fgp[:])
            fgrow.append(fgs)
        for c in range(2):
            for b in range(B):
                mm(smT[:, ds(SM_FGB + 8 * c + 4 * b, 4)],
                   ones_full[0:1, 0:128], fgrow[b][:], start=True, stop=True,
                   skip_group_check=True)
        m1 = T_([128, 16], "m1")
        v.scalar_tensor_tensor(m1[:], smT[:, ds(SM_FGB, 16)], -1.0, rw16[:],
                               op0=OP.mult, op1=OP.mult)
        m2 = T_([128, 16], "m2")
        sc.activation(m2[:], m1[:], AF.Identity, bias=1.0)
        q8 = T_([128, 8], "q8")
        gp.tensor_tensor(q8[:],
                        m2[:].rearrange("p (g r) -> p g r", r=2)[:, :, 0],
                        m2[:].rearrange("p (g r) -> p g r", r=2)[:, :, 1],
                        op=OP.mult)
        ret4 = T_([128, 4], "ret4")
        v.tensor_tensor(ret4[:],
                        q8[:].rearrange("p (h u) -> p h u", u=2)[:, :, 0],
                        q8[:].rearrange("p (h u) -> p h u", u=2)[:, :, 1],
                        op=OP.mult)
        t1 = T_([128, 4], "t1")
        gp.tensor_tensor(t1[:], u_col[:], ww_col[:], op=OP.mult)
        t2 = T_([128, 4], "t2")
        gp.tensor_add(t2[:], u_col[:], ww_col[:])
        v.tensor_sub(t2[:], t2[:], t1[:])
        un_col = C_([128, 4], "u_col")
        v.tensor_tensor(un_col[:], t2[:], ret4[:], op=OP.mult)

        # ===== allocation =====
        lnu_col = T_([128, 4], "lnu_col")
        sc.activation(lnu_col[:], un_col[:], AF.Ln, bias=cE37[:, 0:1])
        ut_ps = PS([1, 512], "ctrl")
        for j in range(4):
            b, c = j // 2, j % 2
            tp_(ut_ps[0:1, ds(128 * j, 128)],
                un_col[:, ds(2 * c + b, 1)], ident[:],
                skip_group_check=True)
        u_row2 = T_([1, 512], "u_row2")
        sc.activation(u_row2[:], ut_ps[:], AF.Copy)
        # per-batch PSUM bank: broadcast u_b; ln(u) goes to SBUF
        lnubc_sb = T_([128, 512], "lnubc_sb")
        ubcln = []
        for b in range(B):
            ub = PS([128, 256], "ubcln", bufs=1)
            mm(ub[:], ones_full[0:1, 0:128],
               u_row2[0:1, ds(256 * b, 256)], start=True, stop=True)
            sc.activation(lnubc_sb[:, ds(256 * b, 256)], ub[:], AF.Ln,
                          bias=cE37[:, 0:1])
            ubcln.append(ub)
        A1 = T_([128, 4], "A1")
        eqc = T_([128, 4], "eqc")
        for c in range(2):
            for b in range(B):
                col = ds(2 * c + b, 1)
                scr = T_([128, 256], f"scr{c}{b}")
                v.scalar_tensor_tensor(scr[:], ubcln[b][:],
                                       un_col[:, col],
                                       lnubc_sb[:, ds(256 * b, 256)],
                                       op0=OP.is_lt, op1=OP.mult,
                                       accum_out=A1[:, col])
                scr2 = T_([128, 256], f"scr2{c}{b}")
                v.scalar_tensor_tensor(scr2[:], ubcln[b][:],
                                       un_col[:, col], jmask[c][:],
                                       op0=OP.is_equal, op1=OP.mult,
                                       accum_out=eqc[:, col])
        A = T_([128, 4], "A")
        v.tensor_tensor(A[:], eqc[:], lnu_col[:], op=OP.mult)
        v.tensor_add(A[:], A[:], A1[:])
        cpx = T_([128, 4], "cpx")
        sc.activation(cpx[:], A[:], AF.Exp)
        onemu = T_([128, 4], "onemu")
        v.tensor_scalar(onemu[:], un_col[:], -1.0, 1.0, op0=OP.mult,
                        op1=OP.add)
        a_col = T_([128, 4], "a_col")
        v.tensor_tensor(a_col[:], onemu[:], cpx[:], op=OP.mult)

        # ===== ww (row space, written into [1,512] row) =====
        at_ps = PS([1, 512], "ctrl")
        for j in range(4):
            b, c = j // 2, j % 2
            tp_(at_ps[0:1, ds(128 * j, 128)], a_col[:, ds(2 * c + b, 1)],
                ident[:], skip_group_check=True)
        wwn_row2 = C_([1, 512], "ww_row2", FPR)
        wws4 = T_([1, 4], "wws4")
        for b in range(B):
            for c in range(2):
                v.scalar_tensor_tensor(
                    wwn_row2[0:1, ds(256 * b + 128 * c, 128)],
                    at_ps[0:1, ds(128 * (2 * b + c), 128)],
                    c1T[0:1, b:b + 1],
                    c2cw[b][0:1, ds(128 * c, 128)],
                    op0=OP.mult, op1=OP.add,
                    accum_out=wws4[0:1, ds(2 * b + c, 1)])
        wws2 = T_([1, 2], "wws2")
        v.tensor_reduce(wws2[:], wws4[:].rearrange("o (b c) -> o b c", c=2),
                        axis=AX.X, op=OP.add)
        # stacked [2,256] / [2,512] forms via selector-scatter in PSUM
        ww2r_ps = PS([2, 256], "ctrl")
        for b in range(B):
            mmr(ww2r_ps[:], selrow[b][:], wwn_row2[0:1, ds(256 * b, 256)],
                start=(b == 0), stop=(b == 1))
        wwn_2r = C_([2, 256], "ww_2r", FPR)
        v.tensor_copy(wwn_2r[:], ww2r_ps[:])
        wwblk_ps = PS([2, 512], "ctrl")
        for b in range(B):
            mmr(wwblk_ps[:, ds(256 * b, 256)], selrow[b][:],
                wwn_row2[0:1, ds(256 * b, 256)], start=True, stop=True,
                skip_group_check=True)
        wwn_blk = C_([2, 512], "ww_blk", FPR)
        sc.activation(wwn_blk[:], wwblk_ps[:], AF.Copy)
        # ww_col via transposes of stacked halves (cols come out as (b))
        for c in range(2):
            tp_(smT[:, ds(SM_WC + 2 * c, 2)].bitcast(FPR),
                wwn_2r[:, ds(128 * c, 128)], identR[0:2, 0:2],
                skip_group_check=True)
        wwn_col = C_([128, 4], "ww_col")
        v.tensor_copy(wwn_col[:], smT[:, ds(SM_WC, 4)])

        # ===== L / LT updates (old p as rhs) =====
        a2 = []
        for c in range(2):
            a2c = PS([128, 512], "a2", bufs=1)
            mmr(a2c[:], wwn_2r[:, ds(128 * c, 128)], negblockmask[:],
                start=True, stop=False)
            mmr(a2c[:], negones_row[:], wwn_row2[:], start=False,
                stop=True)
            a2.append(a2c)
        Ln = []
        LTn = []
        for c in range(2):
            b_c = PS([128, 512], "aux")
            mmr(b_c[:], wwn_2r[:, ds(128 * c, 128)], p_blk[:], start=True,
                stop=True)
            b2_c = PS([128, 512], "aux")
            mmr(b2_c[:], p_2r[:, ds(128 * c, 128)], wwn_blk[:], start=True,
                stop=True)
            lnc = C_([128, 512], f"L{c}", FPR)
            v.scalar_tensor_tensor(lnc[:], a2[c][:], 1.0, L[c][:], op0=OP.add,
                                   op1=OP.mult)
            v.tensor_add(lnc[:], lnc[:], b_c[:])
            gp.affine_select(lnc[:], lnc[:], pattern=[[0, 2], [-1, 256]],
                             compare_op=OP.not_equal, fill=0.0, base=128 * c,
                             channel_multiplier=1)
            Ln.append(lnc)
            ltc = C_([128, 512], f"LT{c}", FPR)
            v.scalar_tensor_tensor(ltc[:], a2[c][:], 1.0, LT[c][:],
                                   op0=OP.add, op1=OP.mult)
            v.tensor_add(ltc[:], ltc[:], b2_c[:])
            gp.affine_select(ltc[:], ltc[:], pattern=[[0, 2], [-1, 256]],
                             compare_op=OP.not_equal, fill=0.0, base=128 * c,
                             channel_multiplier=1)
            LTn.append(ltc)

        # ===== p update (row space + stacked forms) =====
        pn_row2 = C_([1, 512], "p_row2", FPR)
        nws2 = T_([1, 2], "nws2")
        sc.activation(nws2[:], wws2[:], AF.Identity, bias=1.0, scale=-1.0)
        for b in range(B):
            v.scalar_tensor_tensor(pn_row2[0:1, ds(256 * b, 256)],
                                   p_row2[0:1, ds(256 * b, 256)],
                                   nws2[0:1, b:b + 1],
                                   wwn_row2[0:1, ds(256 * b, 256)],
                                   op0=OP.mult, op1=OP.add)
        p2r_ps = PS([2, 256], "ctrl")
        for b in range(B):
            mmr(p2r_ps[:], selrow[b][:], pn_row2[0:1, ds(256 * b, 256)],
                start=(b == 0), stop=(b == 1))
        pn_2r = C_([2, 256], "p_2r", FPR)
        v.tensor_copy(pn_2r[:], p2r_ps[:])
        pblk_ps = PS([2, 512], "ctrl")
        for b in range(B):
            mmr(pblk_ps[:, ds(256 * b, 256)], selrow[b][:],
                pn_row2[0:1, ds(256 * b, 256)], start=True, stop=True,
                skip_group_check=True)
        pn_blk = C_([2, 512], "p_blk", FPR)
        sc.activation(pn_blk[:], pblk_ps[:], AF.Copy)

        # ===== M update =====
        negev_2r = T_([2, 64], "negev_2r", FPR)
        v.tensor_scalar_mul(negev_2r[:], sig[:, 0:64], -1.0)
        wv_2r = T_([2, 64], "wv_2r", FPR)
        v.tensor_copy(wv_2r[:], if_ps[:, C_WV:C_WV + 64])
        q1 = PS([64, 512], "aux")
        mmr(q1[:], negev_2r[:], wwn_blk[:], start=True, stop=True)
        q2 = PS([64, 512], "aux")
        mmr(q2[:], wv_2r[:], wwn_blk[:], start=True, stop=True)
        MTn = C_([64, 512], "MT", FPR)
        v.scalar_tensor_tensor(MTn[:], q1[:], 1.0, MT[:], op0=OP.add,
                               op1=OP.mult)
        v.tensor_add(MTn[:], MTn[:], q2[:])
        # Ms via transposes of MTn
        mst = PS([128, 512], "aux")
        for c in range(2):
            for b in range(B):
                tp_(mst[:, ds(64 * (2 * c + b), 64)].bitcast(FPR),
                    MTn[0:64, ds(256 * b + 128 * c, 128)],
                    identR[0:64, 0:64], skip_group_check=True)
        Msn = []
        for c in range(2):
            msc = C_([128, 128], f"Ms{c}", FPR)
            eng = v if c == 0 else sc
            if eng is sc:
                sc.activation(msc[:], mst[:, ds(128 * c, 128)], AF.Copy)
            else:
                v.tensor_copy(msc[:], mst[:, ds(128 * c, 128)])
            Msn.append(msc)

        # ===== rnorm (new M) =====
        mt2 = T_([64, 512], "mt2", FPR)
        sc.activation(mt2[:], MTn[:], AF.Square)
        nq = PS([2, 512], "aux")
        mmr(nq[:], onesR[0:64, 0:2], mt2[:], start=True, stop=True)
        rnln = T_([1, 512], "rnln")
        sc.activation(rnln[:], nq[0:1, :], AF.Ln, bias=cE12[0:1, 0:1])
        rnn_row2 = C_([1, 512], "rn_row2")
        sc.activation(rnn_row2[:], rnln[:], AF.Exp, scale=-0.5)

        # ===== rc on new M =====
        simr = []
        for b in range(B):
            srb = PS([4, 512], "ctrl")
            mmr(srb[:], keysR[:, ds(4 * b, 4)], MTn[:], start=True, stop=True)
            simr.append(srb)
        for b in range(B):
            mm(shx[ds(32, 4), ds(256 * b, 256)], ones_full[0:1, 0:4],
               rnn_row2[0:1, ds(256 * b, 256)], start=True, stop=True,
               skip_group_check=True)
        rn8_sb = T_([4, 512], "rn8_sb")
        sc.activation(rn8_sb[:], shx[ds(32, 4), :], AF.Copy)
        for b in range(B):
            rr = ds(64 * b, 4)  # rc rows reuse cw rows (consumed)
            v.scalar_tensor_tensor(shx[rr, 0:256],
                                   simr[b][:, ds(256 * b, 256)],
                                   rsRT[:, b:b + 1],
                                   rn8_sb[:, ds(256 * b, 256)],
                                   op0=OP.mult, op1=OP.mult)
            sc.activation(shx[rr, 256:512], shx[rr, 0:256], AF.Exp,
                          accum_out=smT[ds(64 * b, 4), ds(SM_LNU, 1)])
            v.reciprocal(smT[ds(64 * b, 4), ds(SM_LNU, 1)],
                         smT[ds(64 * b, 4), ds(SM_LNU, 1)])

        # ===== fwd / bwd / rw blend =====
        bwd = []
        fwd = []
        for b in range(B):
            bwb = PS([4, 512], "aux")
            for c in range(2):
                mmr(bwb[:], rw16[:, ds(8 * c + 4 * b, 4)], Ln[c][:],
                    start=(c == 0), stop=(c == 1))
            bwd.append(bwb)
        for b in range(B):
            fwb = PS([4, 512], "aux")
            for c in range(2):
                mmr(fwb[:], rw16[:, ds(8 * c + 4 * b, 4)], LTn[c][:],
                    start=(c == 0), stop=(c == 1))
            fwd.append(fwb)
        rwb = []
        for b in range(B):
            blk = ds(256 * b, 256)
            rwbb = T_([4, 256], f"rwb{b}")
            rm1c = T_([4, 1], f"rm1c{b}")
            v.tensor_tensor(rm1c[:], rm_m[1][:, b:b + 1],
                            smT[ds(64 * b, 4), ds(SM_LNU, 1)], op=OP.mult)
            v.tensor_scalar_mul(rwbb[:], bwd[b][:, blk],
                                rm_m[0][:, b:b + 1])
            v.scalar_tensor_tensor(rwbb[:], shx[ds(64 * b, 4), 256:512],
                                   rm1c[:], rwbb[:], op0=OP.mult, op1=OP.add)
            v.scalar_tensor_tensor(rwbb[:], fwd[b][:, blk],
                                   rm_m[2][:, b:b + 1], rwbb[:],
                                   op0=OP.mult, op1=OP.add)
            rwb.append(rwbb)
        for c in range(2):
            for b in range(B):
                tp_(smT[:, ds(SM_RWT + 8 * c + 4 * b, 4)],
                    rwb[b][:, ds(128 * c, 128)], ident[0:4, 0:4],
                    skip_group_check=True)
        rwn16 = C_([128, 16], "rw16", FPR)
        v.tensor_copy(rwn16[:], smT[:, ds(SM_RWT, 16)])

        # ===== rv =====
        rv_sb = []
        for b in range(B):
            rvb = PS([4, 64], "ctrl")
            for c in range(2):
                mmr(rvb[:], rwn16[:, ds(8 * c + 4 * b, 4)],
                    Msn[c][:, ds(64 * b, 64)], start=(c == 0), stop=(c == 1))
            rvsb = T_([4, 64], f"rv_sb{b}")
            v.tensor_copy(rvsb[:], rvb[:])
            rv_sb.append(rvsb)
        for b in range(B):
            tp_(smT[0:64, ds(SM_RVT + 4 * b, 4)], rv_sb[b][:],
                ident[0:4, 0:4], skip_group_check=True)
        rvn128 = C_([128, 4], "rvT128", FPR)
        for b in range(B):
            quad = smT[0:64, ds(SM_RVT + 4 * b, 4)].rearrange(
                "w (j k) -> w k j", k=2)
            v.tensor_copy(rvn128[0:64, ds(2 * b, 2)], quad[:, 0, :])
            v.tensor_copy(rvn128[64:128, ds(2 * b, 2)], quad[:, 1, :])

        # ===== output =====
        po = PS([2, O], "ctrl")
        for k in range(4):
            mmr(po[:], hT[:, ds(2 * k, 2)], wo_sb[k][:], start=(k == 0),
                stop=False)
        for j in range(2):
            lhs = rvn128[:].rearrange("p (b j) -> p j b", j=2)[:, j, :]
            mmr(po[:], lhs, wm2[j][:], start=False, stop=(j == 1))
        out_sb = T_([2, O], "out_sb")
        sc.activation(out_sb[:], po[:], AF.Copy)
        dma(out=out_d.ap().rearrange("t b o -> (t b) o")[ds(2 * t_step, 2), :],
            in_=out_sb[:])

        if dbg is not None and last:
            dma(out=dbg["h"].ap(), in_=h_sb[:])
            dma(out=dbg["sig"].ap(), in_=sig[:])
            dma(out=dbg["cw"].ap()[0:1], in_=c2cw[0][:])
            dma(out=dbg["cw"].ap()[1:2], in_=c2cw[1][:])
            dma(out=dbg["ret"].ap(), in_=ret4[:])
            dma(out=dbg["u"].ap(), in_=un_col[:])
            dma(out=dbg["a"].ap(), in_=a_col[:])
            dma(out=dbg["ww"].ap(), in_=wwn_row2[:])
            dma(out=dbg["mt"].ap(), in_=MTn[:])
            dma(out=dbg["rn"].ap(), in_=rnn_row2[:])
            dma(out=dbg["rc"].ap()[0:4], in_=shx[0:4, 256:512])
            dma(out=dbg["rc"].ap()[4:8], in_=shx[64:68, 256:512])
            dma(out=dbg["rw"].ap()[0:4], in_=rwb[0][:])
            dma(out=dbg["rw"].ap()[4:8], in_=rwb[1][:])
            dma(out=dbg["rv"].ap()[0:4], in_=rv_sb[0][:])
            dma(out=dbg["rv"].ap()[4:8], in_=rv_sb[1][:])
            dma(out=dbg["L0"].ap(), in_=Ln[0][:])
            dma(out=dbg["LT0"].ap(), in_=LTn[0][:])
            dma(out=dbg["p"].ap(), in_=pn_row2[:])
            dma(out=dbg["lnu"].ap(), in_=lnu_col[:])
            dma(out=dbg["eqc"].ap(), in_=eqc[:])
            dma(out=dbg["A1"].ap(), in_=A1[:])

        MT, Ms, L, LT = MTn, Msn, Ln, LTn
        u_col, ww_col = un_col, wwn_col
        ww_2r, ww_row2, ww_blk = wwn_2r, wwn_row2, wwn_blk
        p_2r, p_blk, p_row2 = pn_2r, pn_blk, pn_row2
        rw16, rvT128, rn_row2 = rwn16, rvn128, rnn_row2


# ---------------------------------------------------------------------------
# Public entry point
# ---------------------------------------------------------------------------
_T, _BFULL, _NCORES = 64, 16, 8
_cache = {}


def _get_nc(T=_T, debug=False, fix=True):
    key = ("nc", T, debug, fix)
    if key not in _cache:
        nc = bass.Bass("TRN2")
        build(nc, T, debug=debug)
        if fix:
            fix_sync_waits(nc)
        _cache[key] = nc
    return _cache[key]


def _get_jit():
    """Build the sharded PJRT executable once and reuse it across calls
    (run_bass_kernel_spmd re-traces jax.jit on every call)."""
    if "jit" in _cache:
        return _cache["jit"]
    import jax
    import numpy as _np
    from jax.sharding import Mesh, PartitionSpec
    from jax.experimental.shard_map import shard_map
    from concourse import bass2jax as _b2j
    from concourse import mybir as _mybir
    _b2j.install_neuronx_cc_hook()
    nc = _get_nc()
    partition_name = (nc.partition_id_tensor.name
                      if nc.partition_id_tensor else None)
    in_names, out_names, out_avals, zero_shapes = [], [], [], []
    for alloc in nc.m.functions[0].allocations:
        if not isinstance(alloc, _mybir.MemoryLocationSet):
            continue
        name = alloc.memorylocations[0].name
        if alloc.kind == "ExternalInput":
            if name != partition_name:
                in_names.append(name)
        elif alloc.kind == "ExternalOutput":
            shape = tuple(alloc.tensor_shape)
            dtype = _mybir.dt.np(alloc.dtype)
            out_names.append(name)
            out_avals.append(jax.core.ShapedArray(shape, dtype))
            zero_shapes.append((shape, dtype))
    n_params = len(in_names)
    n_outs = len(out_avals)
    all_names = list(in_names) + out_names
    if partition_name is not None:
        all_names.append(partition_name)

    def _body(*args):
        operands = list(args)
        if partition_name is not None:
            operands.append(_b2j.partition_id_tensor())
        outs = _b2j._bass_exec_p.bind(
            *operands, out_avals=tuple(out_avals), in_names=tuple(all_names),
            out_names=tuple(out_names), lowering_input_output_aliases=(),
            sim_require_finite=True, sim_require_nnan=True, nc=nc)
        return tuple(outs)

    devices = jax.devices()[:_NCORES]
    mesh = Mesh(_np.asarray(devices), ("core",))
    in_specs = (PartitionSpec("core"),) * (n_params + n_outs)
    out_specs = (PartitionSpec("core"),) * n_outs
    donate = tuple(range(n_params, n_params + n_outs))
    try:
        smapped = shard_map(_body, mesh=mesh, in_specs=in_specs,
                            out_specs=out_specs, check_rep=False)
    except TypeError:
        smapped = shard_map(_body, mesh=mesh, in_specs=in_specs,
                            out_specs=out_specs, check_vma=False)
    fn = jax.jit(smapped, donate_argnums=donate, keep_unused=True)
    _cache["mesh"] = mesh
    _cache["jit"] = (fn, in_names, out_names, out_avals, zero_shapes)
    return _cache["jit"]


def kernel(**inputs):
    x = np.ascontiguousarray(np.asarray(inputs["x"], dtype=np.float32))
    shared = {
        k: np.ascontiguousarray(np.asarray(inputs[k], dtype=np.float32))
        for k in ("W_hid", "b_hid", "W_iface", "W_out", "W_memout")
    }
    assert x.shape == (_T, _BFULL, I)
    in_maps = []
    for core in range(_NCORES):
        shard = np.ascontiguousarray(x[:, core * B:(core + 1) * B, :])
        m = {"x": shard}
        m.update(shared)
        in_maps.append(m)
    try:
        fn, in_names, out_names, out_avals, zero_shapes = _get_jit()
        import jax
        # Weights are replicated per core and rarely change between calls:
        # keep their device placement cached, revalidated by exact equality.
        wcache = _cache.setdefault("wdev", {})
        concat_in = []
        for name in in_names:
            if name == "x":
                concat_in.append(np.concatenate(
                    [in_maps[c][name] for c in range(_NCORES)], axis=0))
                continue
            # weights are identical across cores: validate against the
            # single-copy input, replicate only on cache miss
            single = in_maps[0][name]
            ent = wcache.get(name)
            if ent is not None and ent[0].shape == single.shape and \
                    np.array_equal(ent[0], single):
                concat_in.append(ent[1])
            else:
                from jax.sharding import NamedSharding, PartitionSpec as _P
                host = np.concatenate([single] * _NCORES, axis=0)
                shd = NamedSharding(_cache["mesh"], _P("core"))
                dev = jax.device_put(host, shd)
                dev.block_until_ready()
                wcache[name] = (single.copy(), dev)
                concat_in.append(dev)
        concat_zeros = [np.zeros((_NCORES * sh[0],) + tuple(sh[1:]), dt)
                        for sh, dt in zero_shapes]
        out_arrs = fn(*concat_in, *concat_zeros)
        oi = out_names.index("out")
        res = np.asarray(out_arrs[oi]).reshape(_NCORES, _T, B, O)
        out = np.empty((_T, _BFULL, O), dtype=np.float32)
        for core in range(_NCORES):
            out[:, core * B:(core + 1) * B, :] = res[core]
        return out
    except Exception:
        nc = _get_nc()
        res = run_bass_kernel_spmd(nc, in_maps,
                                   core_ids=list(range(_NCORES)))
        out = np.empty((_T, _BFULL, O), dtype=np.float32)
        for core in range(_NCORES):
            out[:, core * B:(core + 1) * B, :] = res.results[core]["out"]
        return out
